# revision 1
# baseline (speedup 1.0000x reference)
"""Trainium2 Bass kernel for a 16-head attention block (d_model=1024, seq=4096).

Sharding: tensor-parallel over heads. Each of the 8 cores computes QKV
projections, RMSNorm(q,k), full softmax(QK^T)V attention for its 2 heads,
and a partial O-projection (its heads' slice of the contraction). The host
sums the 8 partial outputs (bf16 partials) and adds the output bias.

Per-core dataflow (k-first, attention is ACT/exp-bound so everything else
is arranged to hide under it). x ships ONCE as bf16 and stays resident in
SBUF (8 chunk tiles, 1KB descriptor runs) — the cost model serializes all
DMA on a shared 360GB/s device, so halving x traffic halves the prefix:
  phase 1 (serial prefix): k,v projections (bf16 GEMM, moving dim 256),
           RMSNorm(k) with wq*wk folded into the k side, PE-transpose
           k_hat into [64, s] fp32r tiles, V -> bf16 [k, 65] chunks with a
           fused ones column. The q side for the first q-tile is
           interleaved into the prefix tail (ACT Sqrt path).
  phase 2 (ACT-bound steady state): per (q-tile 512, head):
           S[k,q] blocks via khatT.T @ qhatT (K=64 fp32r), exp on ACT ->
           bf16 probs, z[q,65] += probs_chunk.T @ V' with probs as the
           stationary operand (65-column moving operand halves PE time).
           Row 64 = softmax denominator; normalize in q-major on DVE,
           PE-transpose both heads at once into z_nT[d_local, s] bf16.
           The NEXT q-tile's q side runs under the exp shadow: bf16
           x @ Wq GEMM (N=128 bf16 runs at 1 cyc/row; fp32r would be 4x),
           RMSNorm(q) with a Newton-iteration rsqrt on DVE (keeps the
           ACT table on Exp), PE-transposes into qhatT.
  phase 3: out[s,dm] partial = z_nT.T @ WoT (bf16), PSUM->SBUF bf16 -> HBM,
           software-pipelined one q-tile behind attention.
"""

import numpy as np
from contextlib import ExitStack

import concourse.bass as bass
import concourse.tile as tile
from concourse import mybir
from concourse.masks import make_identity

F32 = mybir.dt.float32
F32R = mybir.dt.float32r
BF16 = mybir.dt.bfloat16
I32 = mybir.dt.int32
AF = mybir.ActivationFunctionType
ALU = mybir.AluOpType

D_MODEL = 1024
SEQ = 4096
N_HEADS = 16
D_HEAD = 64
N_CORES = 8
HEADS_LOCAL = 2
P = 128
DM_AUG = D_MODEL + P                     # 1152 rows: x^T plus ones-row block
NCH = DM_AUG // P                        # 9 contraction chunks
KV_LOCAL = 2 * HEADS_LOCAL * D_HEAD      # 256: [k0|k1|v0|v1]
Q_LOCAL = HEADS_LOCAL * D_HEAD           # 128: [q0|q1]
SB = SEQ // P                            # 32 s-blocks
QT = 8                                   # q-tiles of 512
QW = SEQ // QT                           # 512
QC = QW // P                             # 4 q-chunks of 128 per q-tile
KB = SEQ // P                            # 32 k-blocks
EXP_BATCH = 3
EPS = 1e-6
RSQRT_MAGIC = 0x5F3759DF


MAX_WAITS = 1


def _split_excess_waits(nc):
    """This walrus build rejects instructions carrying more than one or two
    sync-wait commands (CTRL and pseudo-DMA structs especially). Rewrite every
    instruction with more than MAX_WAITS waits into a chain of same-engine
    NoOps each carrying MAX_WAITS waits, followed by the original."""
    import bass_rust

    n_new = 0
    for f in nc.m.functions:
        for bb in f.blocks:
            changed = False
            out = []
            for ins in bb.instructions:
                si = ins.sync_info
                waits = list(si.on_wait) if si is not None and si.on_wait else []
                if len(waits) > MAX_WAITS:
                    changed = True
                    ncar = len(waits) - MAX_WAITS
                    for i in range(0, ncar, MAX_WAITS):
                        chunk = waits[i : min(i + MAX_WAITS, ncar)]
                        nop = mybir.InstNoOp(
                            name=f"{ins.name}-wsplit{i}", ins=[], outs=[]
                        )
                        nop.engine = ins.engine
                        nop.sync_info = bass_rust.SyncInfo(
                            on_wait=chunk, on_update=[]
                        )
                        out.append(nop)
                        n_new += 1
                    ins.sync_info = bass_rust.SyncInfo(
                        on_wait=waits[ncar:], on_update=si.on_update
                    )
                out.append(ins)
            if changed:
                bb.instructions = out
    return n_new


def build_core_kernel(split_waits=True):
    nc = bass.Bass()
    xtb = nc.declare_dram_parameter("xtb", [DM_AUG, SEQ], BF16, isOutput=False)
    wkvt = nc.declare_dram_parameter("wkvt", [DM_AUG, KV_LOCAL], BF16, isOutput=False)
    wqt = nc.declare_dram_parameter("wqt", [DM_AUG, Q_LOCAL], BF16, isOutput=False)
    wot = nc.declare_dram_parameter("wot", [P, D_MODEL], BF16, isOutput=False)
    wkc = nc.declare_dram_parameter("wkc", [D_HEAD, 1], F32, isOutput=False)
    out = nc.declare_dram_parameter("out", [SEQ, D_MODEL], BF16, isOutput=True)

    xtb_r = xtb.rearrange("(c p) s -> p c s", p=P)       # [128, 9, 4096]
    wkvt_r = wkvt.rearrange("(c p) f -> p c f", p=P)     # [128, 9, 256]
    wqt_r = wqt.rearrange("(c p) f -> p c f", p=P)       # [128, 9, 128]

    with ExitStack() as ctx:
        tc = ctx.enter_context(tile.TileContext(nc))

        const = ctx.enter_context(tc.tile_pool(name="const", bufs=1))
        persist = ctx.enter_context(tc.tile_pool(name="persist", bufs=1))

        # DMA order matters: the whole prefix waits on x chunk 0 + Wkv
        xb0 = const.tile([P, NCH, QW], BF16, name="xb0")
        wkv_sb = const.tile([P, NCH, KV_LOCAL], BF16)
        nc.sync.dma_start(wkv_sb[:, 0:1, :], wkvt_r[:, 0:1, :])
        nc.sync.dma_start(xb0[:, :, 0:QW // 2], xtb_r[:, :, 0 : QW // 2])
        nc.sync.dma_start(wkv_sb[:, 1:NCH, :], wkvt_r[:, 1:NCH, :])
        nc.sync.dma_start(xb0[:, :, QW // 2 : QW], xtb_r[:, :, QW // 2 : QW])
        ident_f = const.tile([P, P], F32)
        make_identity(nc, ident_f)
        ident_r = const.tile([P, P], F32R)
        nc.scalar.activation(ident_r[:], ident_f[:], AF.Copy)
        ident_b = const.tile([P, P], BF16)
        nc.vector.tensor_copy(ident_b[:], ident_f[:])
        wkc_sb = const.tile([D_HEAD, 1], F32)
        nc.sync.dma_start(wkc_sb[:], wkc[:])
        eps_t = const.tile([P, 1], F32)
        nc.gpsimd.memset(eps_t[:], EPS)
        wq_sb = const.tile([P, NCH, Q_LOCAL], BF16)
        wot_sb = const.tile([P, D_MODEL], BF16)

        # attention operands: q_hat/k_hat in [64, s] fp32r (K=64 contraction),
        # V' in [k, 65] bf16 per (head, k-block) with ones denominator column
        qhatT = [persist.tile([D_HEAD, SEQ], F32R, name=f"qhatT{h}") for h in range(2)]
        khatT = [persist.tile([D_HEAD, SEQ], F32R, name=f"khatT{h}") for h in range(2)]
        vp = persist.tile([P, HEADS_LOCAL, KB, D_HEAD + 1], BF16)
        nc.gpsimd.memset(vp[:, :, :, D_HEAD : D_HEAD + 1], 1.0)
        z_nT = persist.tile([P, SEQ], BF16)

        # resident bf16 copy of x^T (augmented): 8 chunk tiles of 512 s-cols,
        # loaded once (1KB descriptor runs; serves both kv- and q-GEMMs)
        xball = [xb0] + [
            persist.tile([P, NCH, QW], BF16, name=f"xb{d}") for d in range(1, QT)
        ]
        for d in range(1, QT):
            nc.sync.dma_start(xball[d][:], xtb_r[:, :, bass.ts(d, QW)])
            if d == 3:
                nc.sync.dma_start(wq_sb[:], wqt_r)
        nc.sync.dma_start(wot_sb[:], wot[:])

        qnorm = ctx.enter_context(tc.tile_pool(name="qnorm", bufs=3))

        def emit_q_side_sb(sb, qpool, tpool, rsqrt_act=False):
            """bf16 x @ Wq for one s-block + RMSNorm(q) + PE transposes into
            qhatT. rsqrt_act: use ACT Rsqrt (phase-1 table) instead of the
            DVE Newton rsqrt (phase 2 keeps the ACT table on Exp)."""
            if True:
                ssl = bass.ts(sb, P)
                xbt = xball[sb // QC]
                xsl = bass.ts(sb % QC, P)
                qps = qpool.tile([P, Q_LOCAL], F32, name="qps", tag=qpool.name_tag)
                for c in range(NCH):
                    nc.tensor.matmul(
                        qps[:],
                        lhsT=xbt[:, c, xsl],
                        rhs=wq_sb[:, c, :],
                        start=(c == 0),
                        stop=(c == NCH - 1),
                    )
                qsb = qnorm.tile([P, Q_LOCAL], F32, name="qsb", tag="qsb")
                nc.vector.tensor_copy(qsb[:], qps[:])
                qg = qsb[:].rearrange("p (g d) -> p g d", g=2)
                sq = qnorm.tile([P, 2, D_HEAD], F32, name="qsq", tag="qsq")
                nc.vector.tensor_tensor(sq[:], qg, qg, ALU.mult)
                ss = qnorm.tile([P, 2], F32, name="qss", tag="qss")
                nc.vector.tensor_reduce(
                    ss[:], sq[:], axis=mybir.AxisListType.X, op=ALU.add
                )
                # rr = rsqrt(ss/64 + eps) via bit-trick seed + 2 Newton steps
                if rsqrt_act:
                    yrs = qnorm.tile([P, 2], F32, name="qrs", tag="qrs")
                    nc.scalar.activation(
                        yrs[:], ss[:], AF.Sqrt, bias=eps_t[:], scale=1.0 / D_HEAD
                    )
                    yact = qnorm.tile([P, 2], F32, name="qra", tag="qra")
                    nc.vector.reciprocal(yact[:], yrs[:])
                    y = yact[:]
                else:
                    ms = qnorm.tile([P, 2], F32, name="qms", tag="qms")
                    nc.vector.tensor_scalar(
                        ms[:], ss[:], 1.0 / D_HEAD, EPS, op0=ALU.mult, op1=ALU.add
                    )
                    xh = qnorm.tile([P, 2], F32, name="qxh", tag="qxh")
                    nc.vector.tensor_scalar(xh[:], ms[:], 0.5, None, op0=ALU.mult)
                    iy = qnorm.tile([P, 2], I32, name="qiy", tag="qiy")
                    nc.vector.tensor_scalar(
                        iy[:], ms[:].bitcast(I32), 1, None, op0=ALU.logical_shift_right
                    )
                    nc.vector.tensor_scalar(
                        iy[:], iy[:], -1, RSQRT_MAGIC, op0=ALU.mult, op1=ALU.add
                    )
                    y = iy[:].bitcast(F32)
                    for it in range(2):
                        y2 = qnorm.tile([P, 2], F32, name=f"qy2_{it}", tag=f"qy2_{it}")
                        nc.vector.tensor_tensor(y2[:], y, y, ALU.mult)
                        nc.vector.tensor_tensor(y2[:], y2[:], xh[:], ALU.mult)
                        nc.vector.tensor_scalar(
                            y2[:], y2[:], -1.0, 1.5, op0=ALU.mult, op1=ALU.add
                        )
                        yn = qnorm.tile([P, 2], F32, name=f"qyn_{it}", tag=f"qyn_{it}")
                        nc.vector.tensor_tensor(yn[:], y, y2[:], ALU.mult)
                        y = yn[:]
                q_hat = qnorm.tile([P, 2, D_HEAD], F32R, name="qhat", tag="qhat")
                nc.vector.tensor_tensor(
                    q_hat[:], qg, y[:, :, None].to_broadcast((P, 2, D_HEAD)), ALU.mult
                )
                for h in range(2):
                    pt = tpool.tile(
                        [D_HEAD, P], F32R, name="qpt", tag=tpool.name_tag
                    )
                    nc.tensor.transpose(pt[:], q_hat[:, h, :], ident_r[:])
                    nc.vector.tensor_copy(qhatT[h][:, ssl], pt[:])

        # ------------- phase 1: K/V projections + RMSNorm(k) + V' -------------
        with ExitStack() as p1:
            norm = p1.enter_context(tc.tile_pool(name="norm", bufs=8))
            qkps = p1.enter_context(tc.tile_pool(name="kvps", bufs=6, space="PSUM"))
            tps = p1.enter_context(tc.tile_pool(name="tps", bufs=2, space="PSUM"))

            class _P1Pool:
                name_tag = "kvps"

                @staticmethod
                def tile(shape, dt, name=None, tag=None):
                    return qkps.tile(shape, dt, name=name, tag="kvps")

            class _P1TPool:
                name_tag = "tps"

                @staticmethod
                def tile(shape, dt, name=None, tag=None):
                    return tps.tile(shape, dt, name=name, tag="tps")

            for sb in range(SB):
                ssl = bass.ts(sb, P)
                xbt = xball[sb // QC]
                xsl = bass.ts(sb % QC, P)

                kv_ps = qkps.tile([P, KV_LOCAL], F32, name="kv_ps", tag="kvps")
                for c in range(NCH):
                    nc.tensor.matmul(
                        kv_ps[:],
                        lhsT=xbt[:, c, xsl],
                        rhs=wkv_sb[:, c, :],
                        start=(c == 0),
                        stop=(c == NCH - 1),
                    )

                # RMSNorm stats for the 2 k heads
                k_ps = kv_ps[:, 0 : 2 * D_HEAD].rearrange("p (g d) -> p g d", g=2)
                sq = norm.tile([P, 2, D_HEAD], F32)
                nc.scalar.activation(sq[:], k_ps, AF.Square)
                ss = norm.tile([P, 2], F32)
                nc.vector.tensor_reduce(
                    ss[:], sq[:], axis=mybir.AxisListType.X, op=ALU.add
                )
                rs = norm.tile([P, 2], F32)
                nc.scalar.activation(
                    rs[:], ss[:], AF.Sqrt, bias=eps_t[:], scale=1.0 / D_HEAD
                )
                rr = norm.tile([P, 2], F32)
                nc.vector.reciprocal(rr[:], rs[:])

                k_hat = norm.tile([P, 2, D_HEAD], F32R)
                nc.vector.tensor_tensor(
                    k_hat[:],
                    k_ps,
                    rr[:, :, None].to_broadcast((P, 2, D_HEAD)),
                    ALU.mult,
                )

                # V chunks for both heads -> bf16 (ACT: same table as Square)
                nc.scalar.activation(
                    vp[:, :, sb, 0:D_HEAD],
                    kv_ps[:, 2 * D_HEAD : 4 * D_HEAD].rearrange(
                        "p (h d) -> p h d", h=2
                    ),
                    AF.Copy,
                )

                # transposes into [d, s]; wq*wk folded into the k side
                for h in range(2):
                    pt = tps.tile([D_HEAD, P], F32R, name="pt", tag="tps")
                    nc.tensor.transpose(pt[:], k_hat[:, h, :], ident_r[:])
                    if h == 0:
                        nc.scalar.activation(
                            khatT[h][:, ssl], pt[:], AF.Copy, scale=wkc_sb[:]
                        )
                    else:
                        nc.vector.tensor_scalar_mul(
                            khatT[h][:, ssl], pt[:], wkc_sb[:]
                        )

                # q side of the first q-tile, interleaved into the prefix
                # tail (ACT Sqrt: phase 1 owns the sqrt table)
                if sb >= 17 and (sb - 17) % 4 == 0:
                    emit_q_side_sb((sb - 17) // 4, _P1Pool, _P1TPool,
                                   rsqrt_act=True)




        # ---------- phase 2+3: attention with inlined O-projection ----------
        # PSUM banks: 2 score slots x3 banks, z accumulator 1 bank, shared
        # utility bank ("ops": O-proj / next-q-tile GEMM+transposes / ztp) = 8
        with ExitStack() as p2:
            spool = p2.enter_context(tc.tile_pool(name="sps", bufs=2, space="PSUM"))
            zqpool = p2.enter_context(tc.tile_pool(name="zqps", bufs=1, space="PSUM"))
            opool = p2.enter_context(tc.tile_pool(name="ops", bufs=1, space="PSUM"))
            ppool = p2.enter_context(tc.tile_pool(name="probs", bufs=4))
            znpool = p2.enter_context(tc.tile_pool(name="zn", bufs=3))
            rpool = p2.enter_context(tc.tile_pool(name="rcp", bufs=3))
            osb = p2.enter_context(tc.tile_pool(name="osb", bufs=4))

            class _P2Pool:
                name_tag = "ops"

                @staticmethod
                def tile(shape, dt, name=None, tag=None):
                    return opool.tile(shape, dt, name=name, tag="ops")

            def emit_oproj(qt, final=False):
                # final q-tile: S slots are free, so pipeline the matmuls
                # 2-wide through them and put half the copies on the idle ACT
                for sbl in range(QC):
                    sb = qt * QC + sbl
                    ot = osb.tile([P, D_MODEL], BF16, name="ot", tag="ot")
                    for half in range(2):
                        if final:
                            ops = spool.tile([P, QW], F32, name="ops", tag="sps")
                        else:
                            ops = opool.tile([P, QW], F32, name="ops", tag="ops")
                        nc.tensor.matmul(
                            ops[:],
                            lhsT=z_nT[:, bass.ts(sb, P)],
                            rhs=wot_sb[:, bass.ts(half, QW)],
                            start=True,
                            stop=True,
                        )
                        if final and half == 0:
                            nc.scalar.activation(
                                ot[:, bass.ts(half, QW)], ops[:], AF.Copy
                            )
                        else:
                            nc.vector.tensor_copy(ot[:, bass.ts(half, QW)], ops[:])
                        if final:
                            nc.sync.dma_start(
                                out[bass.ts(sb, P), bass.ts(half, QW)],
                                ot[:, bass.ts(half, QW)],
                            )
                    if not final:
                        nc.sync.dma_start(out[bass.ts(sb, P), :], ot[:])

            for qt in range(QT):
                qsl = bass.ts(qt, QW)
                zn = znpool.tile([P, QC, P], BF16, name="zn", tag="zn")
                for h in range(HEADS_LOCAL):
                    zq = zqpool.tile([P, QC, D_HEAD + 1], F32, name="zq", tag="zq")
                    for kb0 in [0] + list(range(2, KB, EXP_BATCH)):
                        nb = 2 if kb0 == 0 else min(EXP_BATCH, KB - kb0)
                        sps = spool.tile(
                            [P, EXP_BATCH, QW], F32, name="sps", tag="sps"
                        )
                        for j in range(nb):
                            kb = kb0 + j
                            nc.tensor.matmul(
                                sps[:, j, :],
                                lhsT=khatT[h][:, bass.ts(kb, P)],
                                rhs=qhatT[h][:, qsl],
                                start=True,
                                stop=True,
                            )
                        probs = ppool.tile(
                            [P, EXP_BATCH, QW], BF16, name="probs", tag="probs"
                        )
                        nc.scalar.activation(
                            probs[:, 0:nb, :], sps[:, 0:nb, :], AF.Exp
                        )
                        # all 128 PV matmuls form ONE PSUM accumulation group
                        # (zq spans a single 2KB zero region): start marks the
                        # whole region pending-zero, each chunk's first touch
                        # overwrites, everything else accumulates
                        for j in range(nb):
                            kb = kb0 + j
                            for qc in range(QC):
                                nc.tensor.matmul(
                                    zq[:, qc, :],
                                    lhsT=probs[:, j, bass.ts(qc, P)],
                                    rhs=vp[:, h, kb, :],
                                    start=(kb == 0 and qc == 0),
                                    stop=(kb == KB - 1 and qc == QC - 1),
                                    skip_group_check=True,
                                )
                        # software-pipelined work emitted under the exp shadow:
                        # h0: O-projection of the previous q-tile
                        # h1: q side (GEMM+norm+transposes) of the next q-tile
                        if kb0 == 2 and h == 0 and qt > 0:
                            emit_oproj(qt - 1)
                        if kb0 == 2 and h == 1 and qt < QT - 1:
                            for sbl in range(QC):
                                emit_q_side_sb((qt + 1) * QC + sbl, _P2Pool, _P2Pool)
                    # normalize in q-major: z = z / rowsum (col 64)
                    rcp = rpool.tile([P, QC], F32, name="rcp", tag="rcp")
                    nc.vector.reciprocal(rcp[:], zq[:, :, D_HEAD])
                    nc.vector.tensor_tensor(
                        zn[:, :, bass.ts(h, D_HEAD)],
                        zq[:, :, 0:D_HEAD],
                        rcp[:, :, None].to_broadcast((P, QC, D_HEAD)),
                        ALU.mult,
                    )
                # transpose both heads at once into z_nT[d_local, s]
                ztp = zqpool.tile([P, QC, P], BF16, name="ztp", tag="zq")
                for qc in range(QC):
                    nc.tensor.transpose(ztp[:, qc, :], zn[:, qc, :], ident_b[:])
                    nc.vector.tensor_copy(
                        z_nT[:, qt * QW + qc * P : qt * QW + (qc + 1) * P],
                        ztp[:, qc, :],
                    )
            emit_oproj(QT - 1, final=True)

    if split_waits:
        _split_excess_waits(nc)
    return nc


def shard_inputs(x, Wqkv, bqkv, Wo, bo, wq, wk):
    import ml_dtypes

    x2 = np.ascontiguousarray(np.asarray(x, dtype=np.float32).reshape(SEQ, D_MODEL))
    Wqkv = np.asarray(Wqkv, dtype=np.float32)
    bqkv = np.asarray(bqkv, dtype=np.float32)
    Wo = np.asarray(Wo, dtype=np.float32)
    wq = np.asarray(wq, dtype=np.float32)
    wk = np.asarray(wk, dtype=np.float32)

    xta = np.zeros((DM_AUG, SEQ), np.float32)
    xta[:D_MODEL] = x2.T
    xta[D_MODEL] = 1.0
    xtb = np.ascontiguousarray(xta.astype(ml_dtypes.bfloat16))

    wkc = np.ascontiguousarray((wq * wk).reshape(D_HEAD, 1))

    in_maps = []
    for c in range(N_CORES):
        rows, brows = [], []
        for part in range(3):
            for h in (HEADS_LOCAL * c, HEADS_LOCAL * c + 1):
                sl = slice(part * D_MODEL + h * D_HEAD, part * D_MODEL + (h + 1) * D_HEAD)
                rows.append(Wqkv[sl])
                brows.append(bqkv[sl])
        Wl = np.concatenate(rows, 0)          # [384, 1024] rows [q0|q1|k0|k1|v0|v1]
        bl = np.concatenate(brows, 0)         # [384]
        wqkvta = np.zeros((DM_AUG, 384), np.float32)
        wqkvta[:D_MODEL] = Wl.T
        wqkvta[D_MODEL] = bl
        wkvt = np.ascontiguousarray(
            wqkvta[:, Q_LOCAL:].astype(ml_dtypes.bfloat16)
        )                                                              # [1152, 256]
        wqt = np.ascontiguousarray(
            wqkvta[:, :Q_LOCAL].astype(ml_dtypes.bfloat16)
        )                                                              # [1152, 128]
        cols = slice(HEADS_LOCAL * c * D_HEAD, (HEADS_LOCAL * c + HEADS_LOCAL) * D_HEAD)
        wotc = np.ascontiguousarray(Wo[:, cols].T.astype(ml_dtypes.bfloat16))
        in_maps.append(
            {
                "xtb": xtb,
                "wkvt": wkvt,
                "wqt": wqt,
                "wot": wotc,
                "wkc": wkc,
            }
        )
    return in_maps


_NC_CACHE = {}
LAST_RESULT = None


def kernel(x, Wqkv, bqkv, Wo, bo, wq, wk):
    import os
    from concourse.bass_utils import run_bass_kernel_spmd

    global LAST_RESULT
    assert np.asarray(x).shape == (1, SEQ, D_MODEL)
    in_maps = shard_inputs(x, Wqkv, bqkv, Wo, bo, wq, wk)
    if "nc" not in _NC_CACHE:
        _NC_CACHE["nc"] = build_core_kernel()
    nc = _NC_CACHE["nc"]
    trace = bool(int(os.environ.get("BASS_KERNEL_TRACE", "0")))
    res = run_bass_kernel_spmd(nc, in_maps, list(range(N_CORES)), trace=trace)
    LAST_RESULT = res
    acc = np.zeros((SEQ, D_MODEL), np.float64)
    for c in range(N_CORES):
        acc += res.results[c]["out"].astype(np.float64)
    acc += np.asarray(bo, dtype=np.float64)
    return acc.astype(np.float32).reshape(1, SEQ, D_MODEL)



# revision 42
# speedup vs baseline: 1.0284x; 1.0284x over previous
"""Trainium2 Bass kernel for a 16-head attention block (d_model=1024, seq=4096).

Sharding: tensor-parallel over heads. Each of the 8 cores computes QKV
projections, RMSNorm(q,k), full softmax(QK^T)V attention for its 2 heads,
and a partial O-projection (its heads' slice of the contraction). The host
sums the 8 partial outputs (bf16 partials) and adds the output bias.

Per-core dataflow (k-first, attention is ACT/exp-bound so everything else
is arranged to hide under it). x ships ONCE as bf16 and stays resident in
SBUF (8 chunk tiles, 1KB descriptor runs) — the cost model serializes all
DMA on a shared 360GB/s device, so halving x traffic halves the prefix:
  phase 1 (serial prefix): k,v projections (bf16 GEMM, moving dim 256),
           RMSNorm(k) with wq*wk folded into the k side, PE-transpose
           k_hat into [64, s] fp32r tiles, V -> bf16 [k, 65] chunks with a
           fused ones column. The q side for the first q-tile is
           interleaved into the prefix tail (ACT Sqrt path).
  phase 2 (ACT-bound steady state): per (q-tile 512, head):
           S[k,q] blocks via khatT.T @ qhatT (K=64 fp32r), exp on ACT ->
           bf16 probs, z[q,65] += probs_chunk.T @ V' with probs as the
           stationary operand (65-column moving operand halves PE time).
           Row 64 = softmax denominator; normalize in q-major on DVE,
           PE-transpose both heads at once into z_nT[d_local, s] bf16.
           The NEXT q-tile's q side runs under the exp shadow: bf16
           x @ Wq GEMM (N=128 bf16 runs at 1 cyc/row; fp32r would be 4x),
           RMSNorm(q) with a Newton-iteration rsqrt on DVE (keeps the
           ACT table on Exp), PE-transposes into qhatT.
  phase 3: out[s,dm] partial = z_nT.T @ WoT (bf16), PSUM->SBUF bf16 -> HBM,
           software-pipelined one q-tile behind attention.
"""

import numpy as np
from contextlib import ExitStack

import concourse.bass as bass
import concourse.tile as tile
from concourse import mybir
from concourse.masks import make_identity

F32 = mybir.dt.float32
F32R = mybir.dt.float32r
BF16 = mybir.dt.bfloat16
I32 = mybir.dt.int32
AF = mybir.ActivationFunctionType
ALU = mybir.AluOpType

D_MODEL = 1024
SEQ = 4096
N_HEADS = 16
D_HEAD = 64
N_CORES = 8
HEADS_LOCAL = 2
P = 128
DM_AUG = D_MODEL + P                     # 1152 rows: x^T plus ones-row block
NCH = DM_AUG // P                        # 9 contraction chunks
KV_LOCAL = 2 * HEADS_LOCAL * D_HEAD      # 256: [k0|k1|v0|v1]
Q_LOCAL = HEADS_LOCAL * D_HEAD           # 128: [q0|q1]
SB = SEQ // P                            # 32 s-blocks
QT = 8                                   # q-tiles of 512
QW = SEQ // QT                           # 512
QC = QW // P                             # 4 q-chunks of 128 per q-tile
KB = SEQ // P                            # 32 k-blocks
EXP_BATCH = 3
PREFIX_SLOT_MS = 0.0025
EPS = 1e-6
RSQRT_MAGIC = 0x5F3759DF


MAX_WAITS = 1


def _split_excess_waits(nc):
    """This walrus build rejects instructions carrying more than one or two
    sync-wait commands (CTRL and pseudo-DMA structs especially). Rewrite every
    instruction with more than MAX_WAITS waits into a chain of same-engine
    NoOps each carrying MAX_WAITS waits, followed by the original."""
    import bass_rust

    n_new = 0
    for f in nc.m.functions:
        for bb in f.blocks:
            changed = False
            out = []
            for ins in bb.instructions:
                si = ins.sync_info
                waits = list(si.on_wait) if si is not None and si.on_wait else []
                if len(waits) > MAX_WAITS:
                    changed = True
                    ncar = len(waits) - MAX_WAITS
                    for i in range(0, ncar, MAX_WAITS):
                        chunk = waits[i : min(i + MAX_WAITS, ncar)]
                        nop = mybir.InstNoOp(
                            name=f"{ins.name}-wsplit{i}", ins=[], outs=[]
                        )
                        nop.engine = ins.engine
                        nop.sync_info = bass_rust.SyncInfo(
                            on_wait=chunk, on_update=[]
                        )
                        out.append(nop)
                        n_new += 1
                    ins.sync_info = bass_rust.SyncInfo(
                        on_wait=waits[ncar:], on_update=si.on_update
                    )
                out.append(ins)
            if changed:
                bb.instructions = out
    return n_new


def build_core_kernel(split_waits=True):
    nc = bass.Bass()
    xtb = nc.declare_dram_parameter("xtb", [DM_AUG, SEQ], BF16, isOutput=False)
    wkvt = nc.declare_dram_parameter("wkvt", [DM_AUG, KV_LOCAL], BF16, isOutput=False)
    wqt = nc.declare_dram_parameter("wqt", [DM_AUG, Q_LOCAL], BF16, isOutput=False)
    wot = nc.declare_dram_parameter("wot", [P, D_MODEL], BF16, isOutput=False)
    wkc = nc.declare_dram_parameter("wkc", [P, 1], F32, isOutput=False)
    out = nc.declare_dram_parameter("out", [SEQ, D_MODEL], BF16, isOutput=True)

    xtb_r = xtb.rearrange("(c p) s -> p c s", p=P)       # [128, 9, 4096]
    wkvt_r = wkvt.rearrange("(c p) f -> p c f", p=P)     # [128, 9, 256]
    wqt_r = wqt.rearrange("(c p) f -> p c f", p=P)       # [128, 9, 128]

    with ExitStack() as ctx:
        tc = ctx.enter_context(tile.TileContext(nc))

        const = ctx.enter_context(tc.tile_pool(name="const", bufs=1))
        persist = ctx.enter_context(tc.tile_pool(name="persist", bufs=1))

        # DMA order matters: q side of qt0 runs first (xb0 + wq), then the
        # k side needs the full wkv
        xb0 = const.tile([P, NCH, QW], BF16, name="xb0")
        wkv_sb = const.tile([P, NCH, KV_LOCAL], BF16)
        wq_sb = const.tile([P, NCH, Q_LOCAL], BF16)
        nc.sync.dma_start(wkv_sb[:, 0:1, :], wkvt_r[:, 0:1, :])
        nc.sync.dma_start(xb0[:, :, 0:QW // 2], xtb_r[:, :, 0 : QW // 2])
        nc.sync.dma_start(wkv_sb[:, 1:NCH, :], wkvt_r[:, 1:NCH, :])
        nc.sync.dma_start(xb0[:, :, QW // 2 : QW], xtb_r[:, :, QW // 2 : QW])
        nc.sync.dma_start(wq_sb[:], wqt_r)
        ident_f = const.tile([P, P], F32)
        make_identity(nc, ident_f)
        ident_r = const.tile([P, P], F32R)
        nc.scalar.activation(ident_r[:], ident_f[:], AF.Copy)
        ident_b = const.tile([P, P], BF16)
        nc.vector.tensor_copy(ident_b[:], ident_f[:])
        wkc_sb = const.tile([P, 1], F32)
        nc.sync.dma_start(wkc_sb[:], wkc[:])
        wot_sb = const.tile([P, D_MODEL], BF16)

        # attention operands packed 2-heads-per-tile: khat_all/qhat_all
        # [128, s] fp32r with h0 in partitions 0-63, h1 in 64-127 (scores use
        # partition-offset matmuls, K=64). V' in [k, 65] bf16 per (head,
        # k-block) with a fused ones denominator column.
        qhat_all = persist.tile([P, SEQ], F32R, name="qhat_all")
        khat_all = persist.tile([P, SEQ], F32R, name="khat_all")
        vp = persist.tile([P, HEADS_LOCAL, KB, D_HEAD + 1], BF16)
        nc.gpsimd.memset(vp[:, :, :, D_HEAD : D_HEAD + 1], 1.0)
        z_nT = persist.tile([P, SEQ], BF16)

        # resident bf16 copy of x^T (augmented): 8 chunk tiles of 512 s-cols,
        # loaded once (1KB descriptor runs; serves both kv- and q-GEMMs).
        # wq comes right after xb0 so the first q-tile's q side can run at
        # the top of the prefix.
        xball = [xb0] + [
            persist.tile([P, NCH, QW], BF16, name=f"xb{d}") for d in range(1, QT)
        ]
        for d in range(1, QT):
            nc.sync.dma_start(xball[d][:], xtb_r[:, :, bass.ts(d, QW)])
        nc.sync.dma_start(wot_sb[:], wot[:])

        qnorm = ctx.enter_context(tc.tile_pool(name="qnorm", bufs=3))

        def emit_rsqrt_dve(pool, ss, n, tag=""):
            """rr = rsqrt(ss/64 + eps) via bit-trick seed + 2 Newton steps,
            all on DVE (keeps ACT free for exp). ss/rr: [P, n] fp32."""
            ms = pool.tile([P, n], F32, name="rms" + tag, tag="rms" + tag)
            nc.vector.tensor_scalar(
                ms[:], ss, 1.0 / D_HEAD, EPS, op0=ALU.mult, op1=ALU.add
            )
            xh = pool.tile([P, n], F32, name="rxh" + tag, tag="rxh" + tag)
            nc.vector.tensor_scalar(xh[:], ms[:], 0.5, None, op0=ALU.mult)
            iy = pool.tile([P, n], I32, name="riy" + tag, tag="riy" + tag)
            nc.vector.tensor_scalar(
                iy[:], ms[:].bitcast(I32), 1, None, op0=ALU.logical_shift_right
            )
            nc.vector.tensor_scalar(
                iy[:], iy[:], -1, RSQRT_MAGIC, op0=ALU.mult, op1=ALU.add
            )
            y = iy[:].bitcast(F32)
            for it in range(2):
                y2 = pool.tile([P, n], F32, name=f"ry2_{it}" + tag, tag=f"ry2_{it}" + tag)
                nc.vector.tensor_tensor(y2[:], y, y, ALU.mult)
                nc.vector.tensor_tensor(y2[:], y2[:], xh[:], ALU.mult)
                nc.vector.tensor_scalar(
                    y2[:], y2[:], -1.0, 1.5, op0=ALU.mult, op1=ALU.add
                )
                yn = pool.tile([P, n], F32, name=f"ryn_{it}" + tag, tag=f"ryn_{it}" + tag)
                nc.vector.tensor_tensor(yn[:], y, y2[:], ALU.mult)
                y = yn[:]
            return y

        def emit_q_side_group(qt, qpool, tpool):
            """bf16 x @ Wq for one q-tile (4 s-blocks) + RMSNorm(q) + PE
            transposes into qhatT. Stats batched across the 4 s-blocks so the
            DVE Newton-rsqrt chain runs once on [P, 8]. All elementwise work
            on DVE/Pool (ACT stays on Exp)."""
            xbt = xball[qt]
            qsball = qnorm.tile([P, QC, Q_LOCAL], F32, name="qsb", tag="qsb")
            for j in range(QC):
                qps = qpool.tile([P, Q_LOCAL], F32, name="qps", tag=qpool.name_tag)
                for c in range(NCH):
                    nc.tensor.matmul(
                        qps[:],
                        lhsT=xbt[:, c, bass.ts(j, P)],
                        rhs=wq_sb[:, c, :],
                        start=(c == 0),
                        stop=(c == NCH - 1),
                    )
                if j % 2 == 0:
                    nc.vector.tensor_copy(qsball[:, j, :], qps[:])
                else:
                    nc.vector.tensor_copy(qsball[:, j, :], qps[:])
            qg = qsball[:].rearrange("p j (g d) -> p j g d", g=2)
            sq = qnorm.tile([P, QC, 2, D_HEAD], F32, name="qsq", tag="qsq")
            nc.vector.tensor_tensor(sq[:], qg, qg, ALU.mult)
            ss = qnorm.tile([P, QC, 2], F32, name="qss", tag="qss")
            nc.vector.tensor_reduce(
                ss[:], sq[:], axis=mybir.AxisListType.X, op=ALU.add
            )
            y = emit_rsqrt_dve(
                qnorm, ss[:].rearrange("p j g -> p (j g)"), QC * 2, tag="q"
            )
            yb = y.rearrange("p (j g) -> p j g", g=2)
            q_hat = qnorm.tile([P, QC, 2, D_HEAD], F32R, name="qhat", tag="qhat")
            nc.vector.tensor_tensor(
                q_hat[:],
                qg,
                yb[:, :, :, None].to_broadcast((P, QC, 2, D_HEAD)),
                ALU.mult,
            )
            for j in range(QC):
                ssl = bass.ts(qt * QC + j, P)
                pt = tpool.tile([P, P], F32R, name="qpt", tag=tpool.name_tag)
                nc.tensor.transpose(
                    pt[:], q_hat[:, j].rearrange("p g d -> p (g d)"), ident_r[:]
                )
                if j % 2 == 0:
                    nc.vector.tensor_copy(qhat_all[:, ssl], pt[:])
                else:
                    nc.vector.tensor_copy(qhat_all[:, ssl], pt[:])

        # shared pools alive for the whole kernel
        opool = ctx.enter_context(tc.tile_pool(name="ops", bufs=1, space="PSUM"))
        ppool = ctx.enter_context(tc.tile_pool(name="probs", bufs=4))
        znpool = ctx.enter_context(tc.tile_pool(name="zn", bufs=3))
        rpool = ctx.enter_context(tc.tile_pool(name="rcp", bufs=3))
        osb = ctx.enter_context(tc.tile_pool(name="osb", bufs=4))
        norm = ctx.enter_context(tc.tile_pool(name="norm", bufs=3))

        class _OpsPool:
            name_tag = "ops"

            @staticmethod
            def tile(shape, dt, name=None, tag=None):
                return opool.tile(shape, dt, name=name, tag="ops")

        def emit_score_exp(h, kb, sps, probs, qsl):
            """S matmul (partition-offset by head) + ACT exp -> bf16 probs.
            sps/probs: [P, QW] APs. Returns the exp instruction."""
            hsl = slice(D_HEAD * h, D_HEAD * (h + 1))
            nc.tensor.matmul(
                sps,
                lhsT=khat_all[hsl, bass.ts(kb, P)],
                rhs=qhat_all[hsl, qsl],
                start=True,
                stop=True,
            )
            return nc.scalar.activation(probs, sps, AF.Exp)

        def emit_pv(h, kb, zq, probs, last):
            """4 PV matmuls accumulating [P, QC, 65] into zq (one PSUM
            zero-region per (qt, h))."""
            for qc in range(QC):
                nc.tensor.matmul(
                    zq[:, qc, :],
                    lhsT=probs[:, bass.ts(qc, P)],
                    rhs=vp[:, h, kb, :],
                    start=(kb == 0 and qc == 0),
                    stop=(last and qc == QC - 1),
                    skip_group_check=True,
                )

        def emit_qt_finish(qt, h_zq_pairs, ztp):
            """normalize z (q-major), pack both heads, PE-transpose into
            z_nT[d_local, s]."""
            zn = znpool.tile([P, QC, P], BF16, name="zn", tag="zn")
            for h, zq in h_zq_pairs:
                rcp = rpool.tile([P, QC], F32, name="rcp", tag="rcp")
                nc.vector.reciprocal(rcp[:], zq[:, :, D_HEAD])
                nc.vector.tensor_tensor(
                    zn[:, :, bass.ts(h, D_HEAD)],
                    zq[:, :, 0:D_HEAD],
                    rcp[:, :, None].to_broadcast((P, QC, D_HEAD)),
                    ALU.mult,
                )
            for qc in range(QC):
                nc.tensor.transpose(ztp[:, qc, :], zn[:, qc, :], ident_b[:])
                nc.vector.tensor_copy(
                    z_nT[:, qt * QW + qc * P : qt * QW + (qc + 1) * P],
                    ztp[:, qc, :],
                )

        def emit_oproj(qt, spool=None, final=False):
            # final q-tile: S slots are free, so pipeline the matmuls
            # 2-wide through them and put half the copies on the idle ACT
            for sbl in range(QC):
                sb = qt * QC + sbl
                ot = osb.tile([P, D_MODEL], BF16, name="ot", tag="ot")
                for half in range(2):
                    if final:
                        ops = spool.tile([P, QW], F32, name="ops", tag="sps")
                    else:
                        ops = opool.tile([P, QW], F32, name="ops", tag="ops")
                    nc.tensor.matmul(
                        ops[:],
                        lhsT=z_nT[:, bass.ts(sb, P)],
                        rhs=wot_sb[:, bass.ts(half, QW)],
                        start=True,
                        stop=True,
                    )
                    if final and half == 0:
                        nc.scalar.activation(
                            ot[:, bass.ts(half, QW)], ops[:], AF.Copy
                        )
                    elif final:
                        nc.vector.tensor_copy(ot[:, bass.ts(half, QW)], ops[:])
                    else:
                        nc.vector.tensor_copy(ot[:, bass.ts(half, QW)], ops[:])
                    if final:
                        nc.sync.dma_start(
                            out[bass.ts(sb, P), bass.ts(half, QW)],
                            ot[:, bass.ts(half, QW)],
                        )
                if not final:
                    nc.sync.dma_start(out[bass.ts(sb, P), :], ot[:])

        # ---- streaming prefix: K/V projections with qt0's attention (both
        # heads) interleaved so ACT starts exp within a few us of launch.
        # PSUM banks: kvps 2 + tps 1 + sps1 2 + zqA 1 + zqB 1 + ops 1 = 8
        with ExitStack() as p1:
            qkps = p1.enter_context(tc.tile_pool(name="kvps", bufs=2, space="PSUM"))
            tps = p1.enter_context(tc.tile_pool(name="tps", bufs=1, space="PSUM"))
            sps1 = p1.enter_context(tc.tile_pool(name="sps1", bufs=2, space="PSUM"))
            zqAp = p1.enter_context(tc.tile_pool(name="zqA", bufs=1, space="PSUM"))
            zqBp = p1.enter_context(tc.tile_pool(name="zqB", bufs=1, space="PSUM"))

            class _TpsPool:
                name_tag = "tps"

                @staticmethod
                def tile(shape, dt, name=None, tag=None):
                    return tps.tile(shape, dt, name=name, tag="tps")

            def emit_k_tail(pend):
                """k_hat mult + packed PE transposes + khat_all copies for a
                finished segment (software-pipelined one segment behind)."""
                g, ksb, rr, jlo, jhi = pend
                n = jhi - jlo
                kg = ksb[:, jlo:jhi].rearrange("p j (g d) -> p j g d", g=2)
                k_hat = norm.tile([P, n, 2, D_HEAD], F32R, name="khat", tag="khat")
                nc.vector.tensor_tensor(
                    k_hat[:],
                    kg,
                    rr.rearrange("p (j g) -> p j g", g=2)[
                        :, :, :, None
                    ].to_broadcast((P, n, 2, D_HEAD)),
                    ALU.mult,
                )
                for j in range(n):
                    ssl = bass.ts(g * QC + jlo + j, P)
                    pt = tps.tile([P, P], F32R, name="pt", tag="tps")
                    nc.tensor.transpose(
                        pt[:], k_hat[:, j].rearrange("p g d -> p (g d)"), ident_r[:]
                    )
                    if j % 2 == 0:
                        nc.vector.tensor_scalar_mul(khat_all[:, ssl], pt[:], wkc_sb[:])
                    else:
                        nc.vector.tensor_scalar_mul(khat_all[:, ssl], pt[:], wkc_sb[:])

            zqA = zqAp.tile([P, QC, D_HEAD + 1], F32, name="zqA")
            zqB = zqBp.tile([P, QC, D_HEAD + 1], F32, name="zqB")
            qsl0 = bass.ts(0, QW)

            def emit_prefix_attn(kblo, kbhi):
                """qt0 attention windows for k-blocks [kblo, kbhi), both
                heads, EB=1. Returns the last exp instruction (used as a
                scheduler ordering anchor)."""
                anchor = None
                for h, zq in ((0, zqA), (1, zqB)):
                    for kb in range(kblo, kbhi):
                        sps = sps1.tile([P, QW], F32, name="sps1", tag="sps1")
                        probs = ppool.tile([P, QW], BF16, name="probs1", tag="probs1")
                        anchor = emit_score_exp(h, kb, sps[:], probs[:], qsl0)
                        emit_pv(h, kb, zq, probs[:], last=(kb == KB - 1))
                return anchor

            def emit_kv_gemms(g, ksb, sqg, jlo, jhi):
                xbt = xball[g]
                for j in range(jlo, jhi):
                    sb = g * QC + j
                    kv_ps = qkps.tile([P, KV_LOCAL], F32, name="kv_ps", tag="kvps")
                    for c in range(NCH):
                        nc.tensor.matmul(
                            kv_ps[:],
                            lhsT=xbt[:, c, bass.ts(j, P)],
                            rhs=wkv_sb[:, c, :],
                            start=(c == 0),
                            stop=(c == NCH - 1),
                        )
                    # stage K + V' to SBUF on DVE (GPSIMD cannot touch PSUM;
                    # ACT would pace the PSUM bank release behind queued exps)
                    nc.vector.tensor_copy(ksb[:, j, :], kv_ps[:, 0 : 2 * D_HEAD])
                    nc.vector.tensor_copy(
                        vp[:, :, sb, 0:D_HEAD],
                        kv_ps[:, 2 * D_HEAD : 4 * D_HEAD].rearrange(
                            "p (h d) -> p h d", h=2
                        ),
                    )
                    kgj = ksb[:, j, :].rearrange("p (g d) -> p g d", g=2)
                    nc.vector.tensor_tensor(sqg[:, j], kgj, kgj, ALU.mult)

            def emit_k_stats(g, ksb, sqg, jlo, jhi):
                n = jhi - jlo
                ssg = norm.tile([P, n, 2], F32, name="ssg", tag="ssg")
                nc.vector.tensor_reduce(
                    ssg[:], sqg[:, jlo:jhi], axis=mybir.AxisListType.X, op=ALU.add
                )
                rr = emit_rsqrt_dve(
                    norm, ssg[:].rearrange("p j g -> p (j g)"), n * 2, tag="k"
                )
                return (g, ksb, rr, jlo, jhi)

            # group 0 is split into two pairs so the first attention windows
            # (and with them ACT's exp stream) start as early as possible
            segs = [(0, 0, 2), (0, 2, 4)] + [(g, 0, QC) for g in range(1, SB // QC)]
            ktiles = {}
            pending = None
            for si, (g, jlo, jhi) in enumerate(segs):
                if jlo == 0:
                    ktiles[g] = (
                        norm.tile([P, QC, 2 * D_HEAD], F32, name="ksb", tag="ksb"),
                        norm.tile([P, QC, 2, D_HEAD], F32, name="sqg", tag="sqg"),
                    )
                ksb, sqg = ktiles[g]
                emit_kv_gemms(g, ksb, sqg, jlo, jhi)
                if si == 0:
                    # q side of qt0: overlaps the first k pair's stats chain
                    emit_q_side_group(0, _OpsPool, _TpsPool)
                if pending is not None:
                    pg, _, _, pjlo, pjhi = pending
                    emit_k_tail(pending)
                    emit_prefix_attn(pg * QC + pjlo, pg * QC + pjhi)
                pending = emit_k_stats(g, ksb, sqg, jlo, jhi)
                if si == 0:
                    # no pipelining for the very first pair: its khat (and the
                    # first exp windows) are the critical path
                    emit_k_tail(pending)
                    emit_prefix_attn(0, 2)
                    pending = None
                if (g, jlo) == (3, 0):
                    emit_q_side_group(1, _OpsPool, _TpsPool)
            emit_k_tail(pending)
            emit_prefix_attn(SB - QC, SB)
            ztp0 = zqBp.tile([P, QC, P], BF16, name="ztp0", tag="zqB")
            emit_qt_finish(0, ((0, zqA), (1, zqB)), ztp0)

        # ---- steady state: q-tiles 1..7, ACT-saturated exp pipeline.
        # PSUM banks: 2 score slots x3 banks, z accumulator 1, ops 1 = 8
        with ExitStack() as p2:
            spool = p2.enter_context(tc.tile_pool(name="sps", bufs=2, space="PSUM"))
            zqpool = p2.enter_context(tc.tile_pool(name="zqps", bufs=1, space="PSUM"))

            for qt in range(1, QT):
                qsl = bass.ts(qt, QW)
                zn = znpool.tile([P, QC, P], BF16, name="zn", tag="zn")
                for h in range(HEADS_LOCAL):
                    zq = zqpool.tile([P, QC, D_HEAD + 1], F32, name="zq", tag="zq")
                    for kb0 in [0] + list(range(2, KB, EXP_BATCH)):
                        nb = 2 if kb0 == 0 else min(EXP_BATCH, KB - kb0)
                        sps = spool.tile(
                            [P, EXP_BATCH, QW], F32, name="sps", tag="sps"
                        )
                        probs = ppool.tile(
                            [P, EXP_BATCH, QW], BF16, name="probs", tag="probs"
                        )
                        for j in range(nb):
                            kb = kb0 + j
                            hsl = slice(D_HEAD * h, D_HEAD * (h + 1))
                            nc.tensor.matmul(
                                sps[:, j, :],
                                lhsT=khat_all[hsl, bass.ts(kb, P)],
                                rhs=qhat_all[hsl, qsl],
                                start=True,
                                stop=True,
                            )
                        nc.scalar.activation(
                            probs[:, 0:nb, :], sps[:, 0:nb, :], AF.Exp
                        )
                        # all 128 PV matmuls form ONE PSUM accumulation group
                        # (zq spans a single 2KB zero region)
                        for j in range(nb):
                            kb = kb0 + j
                            emit_pv(h, kb, zq, probs[:, j, :], last=(kb == KB - 1))
                        # software-pipelined work emitted under the exp shadow:
                        # h0: O-projection of the previous q-tile
                        # h1: q side (GEMM+norm+transposes) of the next q-tile
                        if kb0 == 2 and h == 0:
                            emit_oproj(qt - 1)
                        if kb0 == 2 and h == 1 and qt < QT - 1:
                            emit_q_side_group(qt + 1, _OpsPool, _OpsPool)
                    # normalize in q-major: z = z / rowsum (col 64)
                    rcp = rpool.tile([P, QC], F32, name="rcp", tag="rcp")
                    nc.vector.reciprocal(rcp[:], zq[:, :, D_HEAD])
                    nc.vector.tensor_tensor(
                        zn[:, :, bass.ts(h, D_HEAD)],
                        zq[:, :, 0:D_HEAD],
                        rcp[:, :, None].to_broadcast((P, QC, D_HEAD)),
                        ALU.mult,
                    )
                # transpose both heads at once into z_nT[d_local, s]
                ztp = zqpool.tile([P, QC, P], BF16, name="ztp", tag="zq")
                for qc in range(QC):
                    nc.tensor.transpose(ztp[:, qc, :], zn[:, qc, :], ident_b[:])
                    nc.vector.tensor_copy(
                        z_nT[:, qt * QW + qc * P : qt * QW + (qc + 1) * P],
                        ztp[:, qc, :],
                    )
            emit_oproj(QT - 1, spool, final=True)

    if split_waits:
        _split_excess_waits(nc)
    return nc


def shard_inputs(x, Wqkv, bqkv, Wo, bo, wq, wk):
    import ml_dtypes

    x2 = np.ascontiguousarray(np.asarray(x, dtype=np.float32).reshape(SEQ, D_MODEL))
    Wqkv = np.asarray(Wqkv, dtype=np.float32)
    bqkv = np.asarray(bqkv, dtype=np.float32)
    Wo = np.asarray(Wo, dtype=np.float32)
    wq = np.asarray(wq, dtype=np.float32)
    wk = np.asarray(wk, dtype=np.float32)

    xta = np.zeros((DM_AUG, SEQ), np.float32)
    xta[:D_MODEL] = x2.T
    xta[D_MODEL] = 1.0
    xtb = np.ascontiguousarray(xta.astype(ml_dtypes.bfloat16))

    # per-partition scale for khat_all's packed [d0|d1] feature rows
    wkc = np.ascontiguousarray(np.tile((wq * wk).reshape(D_HEAD), 2).reshape(P, 1))

    in_maps = []
    for c in range(N_CORES):
        rows, brows = [], []
        for part in range(3):
            for h in (HEADS_LOCAL * c, HEADS_LOCAL * c + 1):
                sl = slice(part * D_MODEL + h * D_HEAD, part * D_MODEL + (h + 1) * D_HEAD)
                rows.append(Wqkv[sl])
                brows.append(bqkv[sl])
        Wl = np.concatenate(rows, 0)          # [384, 1024] rows [q0|q1|k0|k1|v0|v1]
        bl = np.concatenate(brows, 0)         # [384]
        wqkvta = np.zeros((DM_AUG, 384), np.float32)
        wqkvta[:D_MODEL] = Wl.T
        wqkvta[D_MODEL] = bl
        wkvt = np.ascontiguousarray(
            wqkvta[:, Q_LOCAL:].astype(ml_dtypes.bfloat16)
        )                                                              # [1152, 256]
        wqt = np.ascontiguousarray(
            wqkvta[:, :Q_LOCAL].astype(ml_dtypes.bfloat16)
        )                                                              # [1152, 128]
        cols = slice(HEADS_LOCAL * c * D_HEAD, (HEADS_LOCAL * c + HEADS_LOCAL) * D_HEAD)
        wotc = np.ascontiguousarray(Wo[:, cols].T.astype(ml_dtypes.bfloat16))
        in_maps.append(
            {
                "xtb": xtb,
                "wkvt": wkvt,
                "wqt": wqt,
                "wot": wotc,
                "wkc": wkc,
            }
        )
    return in_maps


_NC_CACHE = {}
LAST_RESULT = None


def kernel(x, Wqkv, bqkv, Wo, bo, wq, wk):
    import os
    from concourse.bass_utils import run_bass_kernel_spmd

    global LAST_RESULT
    assert np.asarray(x).shape == (1, SEQ, D_MODEL)
    in_maps = shard_inputs(x, Wqkv, bqkv, Wo, bo, wq, wk)
    if "nc" not in _NC_CACHE:
        _NC_CACHE["nc"] = build_core_kernel()
    nc = _NC_CACHE["nc"]
    trace = bool(int(os.environ.get("BASS_KERNEL_TRACE", "0")))
    res = run_bass_kernel_spmd(nc, in_maps, list(range(N_CORES)), trace=trace)
    LAST_RESULT = res
    acc = np.zeros((SEQ, D_MODEL), np.float64)
    for c in range(N_CORES):
        acc += res.results[c]["out"].astype(np.float64)
    acc += np.asarray(bo, dtype=np.float64)
    return acc.astype(np.float32).reshape(1, SEQ, D_MODEL)



# revision 48
# speedup vs baseline: 1.0408x; 1.0120x over previous
"""Trainium2 Bass kernel for a 16-head attention block (d_model=1024, seq=4096).

Sharding: tensor-parallel over heads. Each of the 8 cores computes QKV
projections, RMSNorm(q,k), full softmax(QK^T)V attention for its 2 heads,
and a partial O-projection (its heads' slice of the contraction). The host
sums the 8 partial outputs (bf16 partials) and adds the output bias.

Per-core dataflow (k-first, attention is ACT/exp-bound so everything else
is arranged to hide under it). x ships ONCE as bf16 and stays resident in
SBUF (8 chunk tiles, 1KB descriptor runs) — the cost model serializes all
DMA on a shared 360GB/s device, so halving x traffic halves the prefix:
  phase 1 (serial prefix): k,v projections (bf16 GEMM, moving dim 256),
           RMSNorm(k) with wq*wk folded into the k side, PE-transpose
           k_hat into [64, s] fp32r tiles, V -> bf16 [k, 65] chunks with a
           fused ones column. The q side for the first q-tile is
           interleaved into the prefix tail (ACT Sqrt path).
  phase 2 (ACT-bound steady state): per (q-tile 512, head):
           S[k,q] blocks via khatT.T @ qhatT (K=64 fp32r), exp on ACT ->
           bf16 probs, z[q,65] += probs_chunk.T @ V' with probs as the
           stationary operand (65-column moving operand halves PE time).
           Row 64 = softmax denominator; normalize in q-major on DVE,
           PE-transpose both heads at once into z_nT[d_local, s] bf16.
           The NEXT q-tile's q side runs under the exp shadow: bf16
           x @ Wq GEMM (N=128 bf16 runs at 1 cyc/row; fp32r would be 4x),
           RMSNorm(q) with a Newton-iteration rsqrt on DVE (keeps the
           ACT table on Exp), PE-transposes into qhatT.
  phase 3: out[s,dm] partial = z_nT.T @ WoT (bf16), PSUM->SBUF bf16 -> HBM,
           software-pipelined one q-tile behind attention.
"""

import numpy as np
from contextlib import ExitStack

import concourse.bass as bass
import concourse.tile as tile
from concourse import mybir
from concourse.masks import make_identity

F32 = mybir.dt.float32
F32R = mybir.dt.float32r
BF16 = mybir.dt.bfloat16
I32 = mybir.dt.int32
AF = mybir.ActivationFunctionType
ALU = mybir.AluOpType

D_MODEL = 1024
SEQ = 4096
N_HEADS = 16
D_HEAD = 64
N_CORES = 8
HEADS_LOCAL = 2
P = 128
DM_AUG = D_MODEL + P                     # 1152 rows: x^T plus ones-row block
NCH = DM_AUG // P                        # 9 contraction chunks
KV_LOCAL = 2 * HEADS_LOCAL * D_HEAD      # 256: [k0|k1|v0|v1]
Q_LOCAL = HEADS_LOCAL * D_HEAD           # 128: [q0|q1]
SB = SEQ // P                            # 32 s-blocks
QT = 8                                   # q-tiles of 512
QW = SEQ // QT                           # 512
QC = QW // P                             # 4 q-chunks of 128 per q-tile
KB = SEQ // P                            # 32 k-blocks
EXP_BATCH = 3
PREFIX_SLOT_MS = 0.0025
EPS = 1e-6
RSQRT_MAGIC = 0x5F3759DF


MAX_WAITS = 1


def _split_excess_waits(nc):
    """This walrus build rejects instructions carrying more than one or two
    sync-wait commands (CTRL and pseudo-DMA structs especially). Rewrite every
    instruction with more than MAX_WAITS waits into a chain of same-engine
    NoOps each carrying MAX_WAITS waits, followed by the original."""
    import bass_rust

    n_new = 0
    for f in nc.m.functions:
        for bb in f.blocks:
            changed = False
            out = []
            for ins in bb.instructions:
                si = ins.sync_info
                waits = list(si.on_wait) if si is not None and si.on_wait else []
                if len(waits) > MAX_WAITS:
                    changed = True
                    ncar = len(waits) - MAX_WAITS
                    for i in range(0, ncar, MAX_WAITS):
                        chunk = waits[i : min(i + MAX_WAITS, ncar)]
                        nop = mybir.InstNoOp(
                            name=f"{ins.name}-wsplit{i}", ins=[], outs=[]
                        )
                        nop.engine = ins.engine
                        nop.sync_info = bass_rust.SyncInfo(
                            on_wait=chunk, on_update=[]
                        )
                        out.append(nop)
                        n_new += 1
                    ins.sync_info = bass_rust.SyncInfo(
                        on_wait=waits[ncar:], on_update=si.on_update
                    )
                out.append(ins)
            if changed:
                bb.instructions = out
    return n_new


def build_core_kernel(split_waits=True):
    nc = bass.Bass()
    xtb = nc.declare_dram_parameter("xtb", [DM_AUG, SEQ], BF16, isOutput=False)
    wkvt = nc.declare_dram_parameter("wkvt", [DM_AUG, KV_LOCAL], BF16, isOutput=False)
    wqt = nc.declare_dram_parameter("wqt", [DM_AUG, Q_LOCAL], BF16, isOutput=False)
    wot = nc.declare_dram_parameter("wot", [P, D_MODEL], BF16, isOutput=False)
    wkc = nc.declare_dram_parameter("wkc", [P, 1], F32, isOutput=False)
    out = nc.declare_dram_parameter("out", [SEQ, D_MODEL], BF16, isOutput=True)

    xtb_r = xtb.rearrange("(c p) s -> p c s", p=P)       # [128, 9, 4096]
    wkvt_r = wkvt.rearrange("(c p) f -> p c f", p=P)     # [128, 9, 256]
    wqt_r = wqt.rearrange("(c p) f -> p c f", p=P)       # [128, 9, 128]

    with ExitStack() as ctx:
        tc = ctx.enter_context(tile.TileContext(nc))

        const = ctx.enter_context(tc.tile_pool(name="const", bufs=1))
        persist = ctx.enter_context(tc.tile_pool(name="persist", bufs=1))

        # DMA order matters: q side of qt0 runs first (xb0 + wq), then the
        # k side needs the full wkv
        xb0 = const.tile([P, NCH, QW], BF16, name="xb0")
        wkv_sb = const.tile([P, NCH, KV_LOCAL], BF16)
        wq_sb = const.tile([P, NCH, Q_LOCAL], BF16)
        nc.sync.dma_start(wkv_sb[:, 0:1, :], wkvt_r[:, 0:1, :])
        nc.sync.dma_start(xb0[:, :, 0:QW // 2], xtb_r[:, :, 0 : QW // 2])
        nc.sync.dma_start(wkv_sb[:, 1:NCH, :], wkvt_r[:, 1:NCH, :])
        nc.sync.dma_start(xb0[:, :, QW // 2 : QW], xtb_r[:, :, QW // 2 : QW])
        nc.sync.dma_start(wq_sb[:], wqt_r)
        ident_f = const.tile([P, P], F32)
        make_identity(nc, ident_f)
        ident_r = const.tile([P, P], F32R)
        nc.scalar.activation(ident_r[:], ident_f[:], AF.Copy)
        ident_b = const.tile([P, P], BF16)
        nc.vector.tensor_copy(ident_b[:], ident_f[:])
        wkc_sb = const.tile([P, 1], F32)
        nc.sync.dma_start(wkc_sb[:], wkc[:])
        wot_sb = const.tile([P, D_MODEL], BF16)

        # attention operands packed 2-heads-per-tile: khat_all/qhat_all
        # [128, s] fp32r with h0 in partitions 0-63, h1 in 64-127 (scores use
        # partition-offset matmuls, K=64). V' in [k, 65] bf16 per (head,
        # k-block) with a fused ones denominator column.
        qhat_all = persist.tile([P, SEQ], F32R, name="qhat_all")
        khat_all = persist.tile([P, SEQ], F32R, name="khat_all")
        vp = persist.tile([P, HEADS_LOCAL, KB, D_HEAD + 1], BF16)
        nc.gpsimd.memset(vp[:, :, :, D_HEAD : D_HEAD + 1], 1.0)
        z_nT = persist.tile([P, SEQ], BF16)

        # resident bf16 copy of x^T (augmented): 8 chunk tiles of 512 s-cols,
        # loaded once (1KB descriptor runs; serves both kv- and q-GEMMs).
        # wq comes right after xb0 so the first q-tile's q side can run at
        # the top of the prefix.
        xball = [xb0] + [
            persist.tile([P, NCH, QW], BF16, name=f"xb{d}") for d in range(1, QT)
        ]
        for d in range(1, QT):
            nc.sync.dma_start(xball[d][:], xtb_r[:, :, bass.ts(d, QW)])
        nc.sync.dma_start(wot_sb[:], wot[:])

        qnorm = ctx.enter_context(tc.tile_pool(name="qnorm", bufs=3))

        def emit_rsqrt_dve(pool, ss, n, tag=""):
            """rr = rsqrt(ss/64 + eps) via bit-trick seed + 2 Newton steps,
            all on DVE (keeps ACT free for exp). ss/rr: [P, n] fp32."""
            ms = pool.tile([P, n], F32, name="rms" + tag, tag="rms" + tag)
            nc.vector.tensor_scalar(
                ms[:], ss, 1.0 / D_HEAD, EPS, op0=ALU.mult, op1=ALU.add
            )
            xh = pool.tile([P, n], F32, name="rxh" + tag, tag="rxh" + tag)
            nc.vector.tensor_scalar(xh[:], ms[:], 0.5, None, op0=ALU.mult)
            iy = pool.tile([P, n], I32, name="riy" + tag, tag="riy" + tag)
            nc.vector.tensor_scalar(
                iy[:], ms[:].bitcast(I32), 1, None, op0=ALU.logical_shift_right
            )
            nc.vector.tensor_scalar(
                iy[:], iy[:], -1, RSQRT_MAGIC, op0=ALU.mult, op1=ALU.add
            )
            y = iy[:].bitcast(F32)
            for it in range(2):
                y2 = pool.tile([P, n], F32, name=f"ry2_{it}" + tag, tag=f"ry2_{it}" + tag)
                nc.vector.tensor_tensor(y2[:], y, y, ALU.mult)
                nc.vector.tensor_tensor(y2[:], y2[:], xh[:], ALU.mult)
                nc.vector.tensor_scalar(
                    y2[:], y2[:], -1.0, 1.5, op0=ALU.mult, op1=ALU.add
                )
                yn = pool.tile([P, n], F32, name=f"ryn_{it}" + tag, tag=f"ryn_{it}" + tag)
                nc.vector.tensor_tensor(yn[:], y, y2[:], ALU.mult)
                y = yn[:]
            return y

        def emit_q_side_group(qt, qpool, tpool, act_stage=False):
            """bf16 x @ Wq for one q-tile (4 s-blocks) + RMSNorm(q) + PE
            transposes into qhatT. Stats batched across the 4 s-blocks so the
            DVE Newton-rsqrt chain runs once on [P, 8]. act_stage: route the
            PSUM->SBUF staging copies through ACT (only safe before the exp
            stream starts, when ACT is idle)."""
            xbt = xball[qt]
            qsball = qnorm.tile([P, QC, Q_LOCAL], F32, name="qsb", tag="qsb")
            for j in range(QC):
                qps = qpool.tile([P, Q_LOCAL], F32, name="qps", tag=qpool.name_tag)
                for c in range(NCH):
                    nc.tensor.matmul(
                        qps[:],
                        lhsT=xbt[:, c, bass.ts(j, P)],
                        rhs=wq_sb[:, c, :],
                        start=(c == 0),
                        stop=(c == NCH - 1),
                    )
                if act_stage:
                    nc.scalar.activation(qsball[:, j, :], qps[:], AF.Copy)
                else:
                    nc.vector.tensor_copy(qsball[:, j, :], qps[:])
            qg = qsball[:].rearrange("p j (g d) -> p j g d", g=2)
            sq = qnorm.tile([P, QC, 2, D_HEAD], F32, name="qsq", tag="qsq")
            nc.vector.tensor_tensor(sq[:], qg, qg, ALU.mult)
            ss = qnorm.tile([P, QC, 2], F32, name="qss", tag="qss")
            nc.vector.tensor_reduce(
                ss[:], sq[:], axis=mybir.AxisListType.X, op=ALU.add
            )
            y = emit_rsqrt_dve(
                qnorm, ss[:].rearrange("p j g -> p (j g)"), QC * 2, tag="q"
            )
            yb = y.rearrange("p (j g) -> p j g", g=2)
            q_hat = qnorm.tile([P, QC, 2, D_HEAD], F32R, name="qhat", tag="qhat")
            nc.vector.tensor_tensor(
                q_hat[:],
                qg,
                yb[:, :, :, None].to_broadcast((P, QC, 2, D_HEAD)),
                ALU.mult,
            )
            for j in range(QC):
                ssl = bass.ts(qt * QC + j, P)
                pt = tpool.tile([P, P], F32R, name="qpt", tag=tpool.name_tag)
                nc.tensor.transpose(
                    pt[:], q_hat[:, j].rearrange("p g d -> p (g d)"), ident_r[:]
                )
                if act_stage:
                    nc.scalar.activation(qhat_all[:, ssl], pt[:], AF.Copy)
                else:
                    nc.vector.tensor_copy(qhat_all[:, ssl], pt[:])

        # shared pools alive for the whole kernel
        opool = ctx.enter_context(tc.tile_pool(name="ops", bufs=1, space="PSUM"))
        ppool = ctx.enter_context(tc.tile_pool(name="probs", bufs=4))
        znpool = ctx.enter_context(tc.tile_pool(name="zn", bufs=3))
        rpool = ctx.enter_context(tc.tile_pool(name="rcp", bufs=3))
        osb = ctx.enter_context(tc.tile_pool(name="osb", bufs=4))
        norm = ctx.enter_context(tc.tile_pool(name="norm", bufs=3))

        class _OpsPool:
            name_tag = "ops"

            @staticmethod
            def tile(shape, dt, name=None, tag=None):
                return opool.tile(shape, dt, name=name, tag="ops")

        def emit_score_exp(h, kb, sps, probs, qsl):
            """S matmul (partition-offset by head) + ACT exp -> bf16 probs.
            sps/probs: [P, QW] APs. Returns the exp instruction."""
            hsl = slice(D_HEAD * h, D_HEAD * (h + 1))
            nc.tensor.matmul(
                sps,
                lhsT=khat_all[hsl, bass.ts(kb, P)],
                rhs=qhat_all[hsl, qsl],
                start=True,
                stop=True,
            )
            return nc.scalar.activation(probs, sps, AF.Exp)

        def emit_pv(h, kb, zq, probs, last):
            """4 PV matmuls accumulating [P, QC, 65] into zq (one PSUM
            zero-region per (qt, h))."""
            for qc in range(QC):
                nc.tensor.matmul(
                    zq[:, qc, :],
                    lhsT=probs[:, bass.ts(qc, P)],
                    rhs=vp[:, h, kb, :],
                    start=(kb == 0 and qc == 0),
                    stop=(last and qc == QC - 1),
                    skip_group_check=True,
                )

        def emit_qt_finish(qt, h_zq_pairs, ztp):
            """normalize z (q-major), pack both heads, PE-transpose into
            z_nT[d_local, s]."""
            zn = znpool.tile([P, QC, P], BF16, name="zn", tag="zn")
            for h, zq in h_zq_pairs:
                rcp = rpool.tile([P, QC], F32, name="rcp", tag="rcp")
                nc.vector.reciprocal(rcp[:], zq[:, :, D_HEAD])
                nc.vector.tensor_tensor(
                    zn[:, :, bass.ts(h, D_HEAD)],
                    zq[:, :, 0:D_HEAD],
                    rcp[:, :, None].to_broadcast((P, QC, D_HEAD)),
                    ALU.mult,
                )
            for qc in range(QC):
                nc.tensor.transpose(ztp[:, qc, :], zn[:, qc, :], ident_b[:])
                nc.vector.tensor_copy(
                    z_nT[:, qt * QW + qc * P : qt * QW + (qc + 1) * P],
                    ztp[:, qc, :],
                )

        def emit_oproj(qt, spool=None, final=False):
            # final q-tile: S slots are free, so pipeline the matmuls
            # 2-wide through them and put half the copies on the idle ACT
            for sbl in range(QC):
                sb = qt * QC + sbl
                ot = osb.tile([P, D_MODEL], BF16, name="ot", tag="ot")
                for half in range(2):
                    if final:
                        ops = spool.tile([P, QW], F32, name="ops", tag="sps")
                    else:
                        ops = opool.tile([P, QW], F32, name="ops", tag="ops")
                    nc.tensor.matmul(
                        ops[:],
                        lhsT=z_nT[:, bass.ts(sb, P)],
                        rhs=wot_sb[:, bass.ts(half, QW)],
                        start=True,
                        stop=True,
                    )
                    if final and half == 0:
                        nc.scalar.activation(
                            ot[:, bass.ts(half, QW)], ops[:], AF.Copy
                        )
                    elif final:
                        nc.vector.tensor_copy(ot[:, bass.ts(half, QW)], ops[:])
                    else:
                        nc.vector.tensor_copy(ot[:, bass.ts(half, QW)], ops[:])
                    if final:
                        nc.sync.dma_start(
                            out[bass.ts(sb, P), bass.ts(half, QW)],
                            ot[:, bass.ts(half, QW)],
                        )
                if not final:
                    nc.sync.dma_start(out[bass.ts(sb, P), :], ot[:])

        # ---- streaming prefix: K/V projections with qt0's attention (both
        # heads) interleaved so ACT starts exp within a few us of launch.
        # PSUM banks: kvps 2 + tps 1 + sps1 2 + zqA 1 + zqB 1 + ops 1 = 8
        with ExitStack() as p1:
            qkps = p1.enter_context(tc.tile_pool(name="kvps", bufs=2, space="PSUM"))
            tps = p1.enter_context(tc.tile_pool(name="tps", bufs=1, space="PSUM"))
            sps1 = p1.enter_context(tc.tile_pool(name="sps1", bufs=2, space="PSUM"))
            zqAp = p1.enter_context(tc.tile_pool(name="zqA", bufs=1, space="PSUM"))
            zqBp = p1.enter_context(tc.tile_pool(name="zqB", bufs=1, space="PSUM"))

            class _TpsPool:
                name_tag = "tps"

                @staticmethod
                def tile(shape, dt, name=None, tag=None):
                    return tps.tile(shape, dt, name=name, tag="tps")

            def emit_k_tail(pend, act_stage=False):
                """k_hat mult + packed PE transposes + khat_all copies for a
                finished segment (software-pipelined one segment behind)."""
                g, ksb, rr, jlo, jhi = pend
                n = jhi - jlo
                kg = ksb[:, jlo:jhi].rearrange("p j (g d) -> p j g d", g=2)
                k_hat = norm.tile([P, n, 2, D_HEAD], F32R, name="khat", tag="khat")
                nc.vector.tensor_tensor(
                    k_hat[:],
                    kg,
                    rr.rearrange("p (j g) -> p j g", g=2)[
                        :, :, :, None
                    ].to_broadcast((P, n, 2, D_HEAD)),
                    ALU.mult,
                )
                for j in range(n):
                    ssl = bass.ts(g * QC + jlo + j, P)
                    pt = tps.tile([P, P], F32R, name="pt", tag="tps")
                    nc.tensor.transpose(
                        pt[:], k_hat[:, j].rearrange("p g d -> p (g d)"), ident_r[:]
                    )
                    if act_stage:
                        nc.scalar.activation(
                            khat_all[:, ssl], pt[:], AF.Copy, scale=wkc_sb[:]
                        )
                    else:
                        nc.vector.tensor_scalar_mul(khat_all[:, ssl], pt[:], wkc_sb[:])

            zqA = zqAp.tile([P, QC, D_HEAD + 1], F32, name="zqA")
            zqB = zqBp.tile([P, QC, D_HEAD + 1], F32, name="zqB")
            qsl0 = bass.ts(0, QW)

            def emit_prefix_attn(kblo, kbhi):
                """qt0 attention windows for k-blocks [kblo, kbhi), both
                heads, EB=1. Returns the last exp instruction (used as a
                scheduler ordering anchor)."""
                anchor = None
                for h, zq in ((0, zqA), (1, zqB)):
                    for kb in range(kblo, kbhi):
                        sps = sps1.tile([P, QW], F32, name="sps1", tag="sps1")
                        probs = ppool.tile([P, QW], BF16, name="probs1", tag="probs1")
                        anchor = emit_score_exp(h, kb, sps[:], probs[:], qsl0)
                        emit_pv(h, kb, zq, probs[:], last=(kb == KB - 1))
                return anchor

            def emit_kv_gemms(g, ksb, sqg, jlo, jhi, act_stage=False):
                xbt = xball[g]
                for j in range(jlo, jhi):
                    sb = g * QC + j
                    kv_ps = qkps.tile([P, KV_LOCAL], F32, name="kv_ps", tag="kvps")
                    for c in range(NCH):
                        nc.tensor.matmul(
                            kv_ps[:],
                            lhsT=xbt[:, c, bass.ts(j, P)],
                            rhs=wkv_sb[:, c, :],
                            start=(c == 0),
                            stop=(c == NCH - 1),
                        )
                    # stage K + V' to SBUF on DVE (GPSIMD cannot touch PSUM;
                    # ACT would pace the PSUM bank release behind queued exps
                    # -- except at the very start, when ACT is idle)
                    if act_stage:
                        nc.scalar.activation(
                            ksb[:, j, :], kv_ps[:, 0 : 2 * D_HEAD], AF.Copy
                        )
                        nc.scalar.activation(
                            vp[:, :, sb, 0:D_HEAD],
                            kv_ps[:, 2 * D_HEAD : 4 * D_HEAD].rearrange(
                                "p (h d) -> p h d", h=2
                            ),
                            AF.Copy,
                        )
                    else:
                        nc.vector.tensor_copy(ksb[:, j, :], kv_ps[:, 0 : 2 * D_HEAD])
                        nc.vector.tensor_copy(
                            vp[:, :, sb, 0:D_HEAD],
                            kv_ps[:, 2 * D_HEAD : 4 * D_HEAD].rearrange(
                                "p (h d) -> p h d", h=2
                            ),
                        )
                    kgj = ksb[:, j, :].rearrange("p (g d) -> p g d", g=2)
                    nc.vector.tensor_tensor(sqg[:, j], kgj, kgj, ALU.mult)

            def emit_k_stats(g, ksb, sqg, jlo, jhi):
                n = jhi - jlo
                ssg = norm.tile([P, n, 2], F32, name="ssg", tag="ssg")
                nc.vector.tensor_reduce(
                    ssg[:], sqg[:, jlo:jhi], axis=mybir.AxisListType.X, op=ALU.add
                )
                rr = emit_rsqrt_dve(
                    norm, ssg[:].rearrange("p j g -> p (j g)"), n * 2, tag="k"
                )
                return (g, ksb, rr, jlo, jhi)

            # group 0 is split into two pairs so the first attention windows
            # (and with them ACT's exp stream) start as early as possible
            segs = [(0, 0, 2), (0, 2, 4)] + [(g, 0, QC) for g in range(1, SB // QC)]
            ktiles = {}
            pending = None
            for si, (g, jlo, jhi) in enumerate(segs):
                if jlo == 0:
                    ktiles[g] = (
                        norm.tile([P, QC, 2 * D_HEAD], F32, name="ksb", tag="ksb"),
                        norm.tile([P, QC, 2, D_HEAD], F32, name="sqg", tag="sqg"),
                    )
                ksb, sqg = ktiles[g]
                # de-prioritize far-ahead kv GEMMs for the tile scheduler so
                # ready attention windows always win the PE; pure scheduling
                # hint (no runtime waits), PE still takes GEMMs when idle
                with tc.high_priority(offset=-(si * 800)):
                    emit_kv_gemms(g, ksb, sqg, jlo, jhi, act_stage=(si == 0))
                if si == 0:
                    # q side of qt0: overlaps the first k pair's stats chain
                    emit_q_side_group(0, _OpsPool, _TpsPool, act_stage=True)
                if pending is not None:
                    pg, _, _, pjlo, pjhi = pending
                    emit_k_tail(pending)
                    emit_prefix_attn(pg * QC + pjlo, pg * QC + pjhi)
                pending = emit_k_stats(g, ksb, sqg, jlo, jhi)
                if si == 0:
                    # no pipelining for the very first pair: its khat (and the
                    # first exp windows) are the critical path
                    emit_k_tail(pending, act_stage=True)
                    emit_prefix_attn(0, 2)
                    pending = None
                if (g, jlo) == (3, 0):
                    with tc.high_priority(offset=-2400):
                        emit_q_side_group(1, _OpsPool, _TpsPool)
            emit_k_tail(pending)
            emit_prefix_attn(SB - QC, SB)
            ztp0 = zqBp.tile([P, QC, P], BF16, name="ztp0", tag="zqB")
            emit_qt_finish(0, ((0, zqA), (1, zqB)), ztp0)

        # ---- steady state: q-tiles 1..7, ACT-saturated exp pipeline.
        # PSUM banks: 2 score slots x3 banks, z accumulator 1, ops 1 = 8
        with ExitStack() as p2:
            spool = p2.enter_context(tc.tile_pool(name="sps", bufs=2, space="PSUM"))
            zqpool = p2.enter_context(tc.tile_pool(name="zqps", bufs=1, space="PSUM"))

            for qt in range(1, QT):
                qsl = bass.ts(qt, QW)
                zn = znpool.tile([P, QC, P], BF16, name="zn", tag="zn")
                for h in range(HEADS_LOCAL):
                    zq = zqpool.tile([P, QC, D_HEAD + 1], F32, name="zq", tag="zq")
                    for kb0 in [0] + list(range(2, KB, EXP_BATCH)):
                        nb = 2 if kb0 == 0 else min(EXP_BATCH, KB - kb0)
                        sps = spool.tile(
                            [P, EXP_BATCH, QW], F32, name="sps", tag="sps"
                        )
                        probs = ppool.tile(
                            [P, EXP_BATCH, QW], BF16, name="probs", tag="probs"
                        )
                        for j in range(nb):
                            kb = kb0 + j
                            hsl = slice(D_HEAD * h, D_HEAD * (h + 1))
                            nc.tensor.matmul(
                                sps[:, j, :],
                                lhsT=khat_all[hsl, bass.ts(kb, P)],
                                rhs=qhat_all[hsl, qsl],
                                start=True,
                                stop=True,
                            )
                        nc.scalar.activation(
                            probs[:, 0:nb, :], sps[:, 0:nb, :], AF.Exp
                        )
                        # all 128 PV matmuls form ONE PSUM accumulation group
                        # (zq spans a single 2KB zero region)
                        for j in range(nb):
                            kb = kb0 + j
                            emit_pv(h, kb, zq, probs[:, j, :], last=(kb == KB - 1))
                        # software-pipelined work emitted under the exp shadow:
                        # h0: O-projection of the previous q-tile
                        # h1: q side (GEMM+norm+transposes) of the next q-tile
                        if kb0 == 2 and h == 0:
                            emit_oproj(qt - 1)
                        if kb0 == 2 and h == 1 and qt < QT - 1:
                            emit_q_side_group(qt + 1, _OpsPool, _OpsPool)
                    # normalize in q-major: z = z / rowsum (col 64)
                    rcp = rpool.tile([P, QC], F32, name="rcp", tag="rcp")
                    nc.vector.reciprocal(rcp[:], zq[:, :, D_HEAD])
                    nc.vector.tensor_tensor(
                        zn[:, :, bass.ts(h, D_HEAD)],
                        zq[:, :, 0:D_HEAD],
                        rcp[:, :, None].to_broadcast((P, QC, D_HEAD)),
                        ALU.mult,
                    )
                # transpose both heads at once into z_nT[d_local, s]
                ztp = zqpool.tile([P, QC, P], BF16, name="ztp", tag="zq")
                for qc in range(QC):
                    nc.tensor.transpose(ztp[:, qc, :], zn[:, qc, :], ident_b[:])
                    nc.vector.tensor_copy(
                        z_nT[:, qt * QW + qc * P : qt * QW + (qc + 1) * P],
                        ztp[:, qc, :],
                    )
            emit_oproj(QT - 1, spool, final=True)

    if split_waits:
        _split_excess_waits(nc)
    return nc


def shard_inputs(x, Wqkv, bqkv, Wo, bo, wq, wk):
    import ml_dtypes

    x2 = np.ascontiguousarray(np.asarray(x, dtype=np.float32).reshape(SEQ, D_MODEL))
    Wqkv = np.asarray(Wqkv, dtype=np.float32)
    bqkv = np.asarray(bqkv, dtype=np.float32)
    Wo = np.asarray(Wo, dtype=np.float32)
    wq = np.asarray(wq, dtype=np.float32)
    wk = np.asarray(wk, dtype=np.float32)

    xta = np.zeros((DM_AUG, SEQ), np.float32)
    xta[:D_MODEL] = x2.T
    xta[D_MODEL] = 1.0
    xtb = np.ascontiguousarray(xta.astype(ml_dtypes.bfloat16))

    # per-partition scale for khat_all's packed [d0|d1] feature rows
    wkc = np.ascontiguousarray(np.tile((wq * wk).reshape(D_HEAD), 2).reshape(P, 1))

    in_maps = []
    for c in range(N_CORES):
        rows, brows = [], []
        for part in range(3):
            for h in (HEADS_LOCAL * c, HEADS_LOCAL * c + 1):
                sl = slice(part * D_MODEL + h * D_HEAD, part * D_MODEL + (h + 1) * D_HEAD)
                rows.append(Wqkv[sl])
                brows.append(bqkv[sl])
        Wl = np.concatenate(rows, 0)          # [384, 1024] rows [q0|q1|k0|k1|v0|v1]
        bl = np.concatenate(brows, 0)         # [384]
        wqkvta = np.zeros((DM_AUG, 384), np.float32)
        wqkvta[:D_MODEL] = Wl.T
        wqkvta[D_MODEL] = bl
        wkvt = np.ascontiguousarray(
            wqkvta[:, Q_LOCAL:].astype(ml_dtypes.bfloat16)
        )                                                              # [1152, 256]
        wqt = np.ascontiguousarray(
            wqkvta[:, :Q_LOCAL].astype(ml_dtypes.bfloat16)
        )                                                              # [1152, 128]
        cols = slice(HEADS_LOCAL * c * D_HEAD, (HEADS_LOCAL * c + HEADS_LOCAL) * D_HEAD)
        wotc = np.ascontiguousarray(Wo[:, cols].T.astype(ml_dtypes.bfloat16))
        in_maps.append(
            {
                "xtb": xtb,
                "wkvt": wkvt,
                "wqt": wqt,
                "wot": wotc,
                "wkc": wkc,
            }
        )
    return in_maps


_NC_CACHE = {}
LAST_RESULT = None


def kernel(x, Wqkv, bqkv, Wo, bo, wq, wk):
    import os
    from concourse.bass_utils import run_bass_kernel_spmd

    global LAST_RESULT
    assert np.asarray(x).shape == (1, SEQ, D_MODEL)
    in_maps = shard_inputs(x, Wqkv, bqkv, Wo, bo, wq, wk)
    if "nc" not in _NC_CACHE:
        _NC_CACHE["nc"] = build_core_kernel()
    nc = _NC_CACHE["nc"]
    trace = bool(int(os.environ.get("BASS_KERNEL_TRACE", "0")))
    res = run_bass_kernel_spmd(nc, in_maps, list(range(N_CORES)), trace=trace)
    LAST_RESULT = res
    acc = np.zeros((SEQ, D_MODEL), np.float64)
    for c in range(N_CORES):
        acc += res.results[c]["out"].astype(np.float64)
    acc += np.asarray(bo, dtype=np.float64)
    return acc.astype(np.float32).reshape(1, SEQ, D_MODEL)



# revision 54
# speedup vs baseline: 1.0409x; 1.0001x over previous
"""Trainium2 Bass kernel for a 16-head attention block (d_model=1024, seq=4096).

Sharding: tensor-parallel over heads. Each of the 8 cores computes QKV
projections, RMSNorm(q,k), full softmax(QK^T)V attention for its 2 heads,
and a partial O-projection (its heads' slice of the contraction). The host
sums the 8 partial outputs (bf16 partials) and adds the output bias.

Per-core dataflow (k-first, attention is ACT/exp-bound so everything else
is arranged to hide under it). x ships ONCE as bf16 and stays resident in
SBUF (8 chunk tiles, 1KB descriptor runs) — the cost model serializes all
DMA on a shared 360GB/s device, so halving x traffic halves the prefix:
  phase 1 (serial prefix): k,v projections (bf16 GEMM, moving dim 256),
           RMSNorm(k) with wq*wk folded into the k side, PE-transpose
           k_hat into [64, s] fp32r tiles, V -> bf16 [k, 65] chunks with a
           fused ones column. The q side for the first q-tile is
           interleaved into the prefix tail (ACT Sqrt path).
  phase 2 (ACT-bound steady state): per (q-tile 512, head):
           S[k,q] blocks via khatT.T @ qhatT (K=64 fp32r), exp on ACT ->
           bf16 probs, z[q,65] += probs_chunk.T @ V' with probs as the
           stationary operand (65-column moving operand halves PE time).
           Row 64 = softmax denominator; normalize in q-major on DVE,
           PE-transpose both heads at once into z_nT[d_local, s] bf16.
           The NEXT q-tile's q side runs under the exp shadow: bf16
           x @ Wq GEMM (N=128 bf16 runs at 1 cyc/row; fp32r would be 4x),
           RMSNorm(q) with a Newton-iteration rsqrt on DVE (keeps the
           ACT table on Exp), PE-transposes into qhatT.
  phase 3: out[s,dm] partial = z_nT.T @ WoT (bf16), PSUM->SBUF bf16 -> HBM,
           software-pipelined one q-tile behind attention.
"""

import numpy as np
from contextlib import ExitStack

import concourse.bass as bass
import concourse.tile as tile
from concourse import mybir
from concourse.masks import make_identity

F32 = mybir.dt.float32
F32R = mybir.dt.float32r
BF16 = mybir.dt.bfloat16
I32 = mybir.dt.int32
AF = mybir.ActivationFunctionType
ALU = mybir.AluOpType

D_MODEL = 1024
SEQ = 4096
N_HEADS = 16
D_HEAD = 64
N_CORES = 8
HEADS_LOCAL = 2
P = 128
DM_AUG = D_MODEL + P                     # 1152 rows: x^T plus ones-row block
NCH = DM_AUG // P                        # 9 contraction chunks
KV_LOCAL = 2 * HEADS_LOCAL * D_HEAD      # 256: [k0|k1|v0|v1]
Q_LOCAL = HEADS_LOCAL * D_HEAD           # 128: [q0|q1]
SB = SEQ // P                            # 32 s-blocks
QT = 8                                   # q-tiles of 512
QW = SEQ // QT                           # 512
QC = QW // P                             # 4 q-chunks of 128 per q-tile
KB = SEQ // P                            # 32 k-blocks
EXP_BATCH = 3
PREFIX_SLOT_MS = 0.0025
EPS = 1e-6
RSQRT_MAGIC = 0x5F3759DF


MAX_WAITS = 1


def _split_excess_waits(nc):
    """This walrus build rejects instructions carrying more than one or two
    sync-wait commands (CTRL and pseudo-DMA structs especially). Rewrite every
    instruction with more than MAX_WAITS waits into a chain of same-engine
    NoOps each carrying MAX_WAITS waits, followed by the original."""
    import bass_rust

    n_new = 0
    for f in nc.m.functions:
        for bb in f.blocks:
            changed = False
            out = []
            for ins in bb.instructions:
                si = ins.sync_info
                waits = list(si.on_wait) if si is not None and si.on_wait else []
                if len(waits) > MAX_WAITS:
                    changed = True
                    ncar = len(waits) - MAX_WAITS
                    for i in range(0, ncar, MAX_WAITS):
                        chunk = waits[i : min(i + MAX_WAITS, ncar)]
                        nop = mybir.InstNoOp(
                            name=f"{ins.name}-wsplit{i}", ins=[], outs=[]
                        )
                        nop.engine = ins.engine
                        nop.sync_info = bass_rust.SyncInfo(
                            on_wait=chunk, on_update=[]
                        )
                        out.append(nop)
                        n_new += 1
                    ins.sync_info = bass_rust.SyncInfo(
                        on_wait=waits[ncar:], on_update=si.on_update
                    )
                out.append(ins)
            if changed:
                bb.instructions = out
    return n_new


def build_core_kernel(split_waits=True):
    nc = bass.Bass()
    xtb = nc.declare_dram_parameter("xtb", [DM_AUG, SEQ], BF16, isOutput=False)
    wkvt = nc.declare_dram_parameter("wkvt", [DM_AUG, KV_LOCAL], BF16, isOutput=False)
    wqt = nc.declare_dram_parameter("wqt", [DM_AUG, Q_LOCAL], BF16, isOutput=False)
    wot = nc.declare_dram_parameter("wot", [P, D_MODEL], BF16, isOutput=False)
    wkc = nc.declare_dram_parameter("wkc", [P, 1], F32, isOutput=False)
    out = nc.declare_dram_parameter("out", [SEQ, D_MODEL], BF16, isOutput=True)

    xtb_r = xtb.rearrange("(c p) s -> p c s", p=P)       # [128, 9, 4096]
    wkvt_r = wkvt.rearrange("(c p) f -> p c f", p=P)     # [128, 9, 256]
    wqt_r = wqt.rearrange("(c p) f -> p c f", p=P)       # [128, 9, 128]

    with ExitStack() as ctx:
        tc = ctx.enter_context(tile.TileContext(nc))

        const = ctx.enter_context(tc.tile_pool(name="const", bufs=1))
        persist = ctx.enter_context(tc.tile_pool(name="persist", bufs=1))

        # DMA order matters: q side of qt0 runs first (xb0 + wq), then the
        # k side needs the full wkv
        xb0 = const.tile([P, NCH, QW], BF16, name="xb0")
        wkv_sb = const.tile([P, NCH, KV_LOCAL], BF16)
        wq_sb = const.tile([P, NCH, Q_LOCAL], BF16)
        nc.sync.dma_start(wkv_sb[:, 0:1, :], wkvt_r[:, 0:1, :])
        nc.sync.dma_start(xb0[:, :, 0:QW // 2], xtb_r[:, :, 0 : QW // 2])
        nc.sync.dma_start(wkv_sb[:, 1:NCH, :], wkvt_r[:, 1:NCH, :])
        nc.sync.dma_start(xb0[:, :, QW // 2 : QW], xtb_r[:, :, QW // 2 : QW])
        nc.sync.dma_start(wq_sb[:], wqt_r)
        ident_f = const.tile([P, P], F32)
        make_identity(nc, ident_f)
        ident_r = const.tile([P, P], F32R)
        nc.scalar.activation(ident_r[:], ident_f[:], AF.Copy)
        ident_b = const.tile([P, P], BF16)
        nc.vector.tensor_copy(ident_b[:], ident_f[:])
        wkc_sb = const.tile([P, 1], F32)
        nc.sync.dma_start(wkc_sb[:], wkc[:])
        wot_sb = const.tile([P, D_MODEL], BF16)

        # attention operands packed 2-heads-per-tile: khat_all/qhat_all
        # [128, s] fp32r with h0 in partitions 0-63, h1 in 64-127 (scores use
        # partition-offset matmuls, K=64). V' in [k, 65] bf16 per (head,
        # k-block) with a fused ones denominator column.
        qhat_all = persist.tile([P, SEQ], F32R, name="qhat_all")
        khat_all = persist.tile([P, SEQ], F32R, name="khat_all")
        vp = persist.tile([P, HEADS_LOCAL, KB, D_HEAD + 1], BF16)
        nc.gpsimd.memset(vp[:, :, :, D_HEAD : D_HEAD + 1], 1.0)
        z_nT = persist.tile([P, SEQ], BF16)

        # resident bf16 copy of x^T (augmented): 8 chunk tiles of 512 s-cols,
        # loaded once (1KB descriptor runs; serves both kv- and q-GEMMs).
        # wq comes right after xb0 so the first q-tile's q side can run at
        # the top of the prefix.
        xball = [xb0] + [
            persist.tile([P, NCH, QW], BF16, name=f"xb{d}") for d in range(1, QT)
        ]
        for d in range(1, QT):
            nc.sync.dma_start(xball[d][:], xtb_r[:, :, bass.ts(d, QW)])
        nc.sync.dma_start(wot_sb[:], wot[:])

        qnorm = ctx.enter_context(tc.tile_pool(name="qnorm", bufs=3))

        def emit_rsqrt_dve(pool, ss, n, tag=""):
            """rr = rsqrt(ss/64 + eps) via bit-trick seed + 2 Newton steps,
            all on DVE (keeps ACT free for exp). ss/rr: [P, n] fp32."""
            ms = pool.tile([P, n], F32, name="rms" + tag, tag="rms" + tag)
            nc.vector.tensor_scalar(
                ms[:], ss, 1.0 / D_HEAD, EPS, op0=ALU.mult, op1=ALU.add
            )
            xh = pool.tile([P, n], F32, name="rxh" + tag, tag="rxh" + tag)
            nc.vector.tensor_scalar(xh[:], ms[:], 0.5, None, op0=ALU.mult)
            iy = pool.tile([P, n], I32, name="riy" + tag, tag="riy" + tag)
            nc.vector.tensor_scalar(
                iy[:], ms[:].bitcast(I32), 1, None, op0=ALU.logical_shift_right
            )
            nc.vector.tensor_scalar(
                iy[:], iy[:], -1, RSQRT_MAGIC, op0=ALU.mult, op1=ALU.add
            )
            y = iy[:].bitcast(F32)
            for it in range(2):
                y2 = pool.tile([P, n], F32, name=f"ry2_{it}" + tag, tag=f"ry2_{it}" + tag)
                nc.vector.tensor_tensor(y2[:], y, y, ALU.mult)
                nc.vector.tensor_tensor(y2[:], y2[:], xh[:], ALU.mult)
                nc.vector.tensor_scalar(
                    y2[:], y2[:], -1.0, 1.5, op0=ALU.mult, op1=ALU.add
                )
                yn = pool.tile([P, n], F32, name=f"ryn_{it}" + tag, tag=f"ryn_{it}" + tag)
                nc.vector.tensor_tensor(yn[:], y, y2[:], ALU.mult)
                y = yn[:]
            return y

        def emit_q_side_a(qt, qpool, act_stage=False):
            """part A of the q side: GEMMs + staging + stats + rsqrt."""
            xbt = xball[qt]
            qsball = qnorm.tile([P, QC, Q_LOCAL], F32, name="qsb", tag="qsb")
            for j in range(QC):
                qps = qpool.tile([P, Q_LOCAL], F32, name="qps", tag=qpool.name_tag)
                for c in range(NCH):
                    nc.tensor.matmul(
                        qps[:],
                        lhsT=xbt[:, c, bass.ts(j, P)],
                        rhs=wq_sb[:, c, :],
                        start=(c == 0),
                        stop=(c == NCH - 1),
                    )
                if act_stage:
                    nc.scalar.activation(qsball[:, j, :], qps[:], AF.Copy)
                else:
                    nc.vector.tensor_copy(qsball[:, j, :], qps[:])
            qg = qsball[:].rearrange("p j (g d) -> p j g d", g=2)
            sq = qnorm.tile([P, QC, 2, D_HEAD], F32, name="qsq", tag="qsq")
            nc.vector.tensor_tensor(sq[:], qg, qg, ALU.mult)
            ss = qnorm.tile([P, QC, 2], F32, name="qss", tag="qss")
            nc.vector.tensor_reduce(
                ss[:], sq[:], axis=mybir.AxisListType.X, op=ALU.add
            )
            y = emit_rsqrt_dve(
                qnorm, ss[:].rearrange("p j g -> p (j g)"), QC * 2, tag="q"
            )
            return (qt, qsball, y)

        def emit_q_side_b(part_a, tpool, act_stage=False):
            """part B: q_hat mult + packed transposes + qhat_all copies."""
            qt, qsball, y = part_a
            qg = qsball[:].rearrange("p j (g d) -> p j g d", g=2)
            yb = y.rearrange("p (j g) -> p j g", g=2)
            q_hat = qnorm.tile([P, QC, 2, D_HEAD], F32R, name="qhat", tag="qhat")
            nc.vector.tensor_tensor(
                q_hat[:],
                qg,
                yb[:, :, :, None].to_broadcast((P, QC, 2, D_HEAD)),
                ALU.mult,
            )
            for j in range(QC):
                ssl = bass.ts(qt * QC + j, P)
                pt = tpool.tile([P, P], F32R, name="qpt", tag=tpool.name_tag)
                nc.tensor.transpose(
                    pt[:], q_hat[:, j].rearrange("p g d -> p (g d)"), ident_r[:]
                )
                if act_stage:
                    nc.scalar.activation(qhat_all[:, ssl], pt[:], AF.Copy)
                else:
                    nc.vector.tensor_copy(qhat_all[:, ssl], pt[:])

        def emit_q_side_group(qt, qpool, tpool, act_stage=False):
            emit_q_side_b(
                emit_q_side_a(qt, qpool, act_stage), tpool, act_stage
            )

        def _unused_q_side(qt, qpool, tpool, act_stage=False):
            """bf16 x @ Wq for one q-tile (4 s-blocks) + RMSNorm(q) + PE
            transposes into qhatT. Stats batched across the 4 s-blocks so the
            DVE Newton-rsqrt chain runs once on [P, 8]. act_stage: route the
            PSUM->SBUF staging copies through ACT (only safe before the exp
            stream starts, when ACT is idle)."""
            xbt = xball[qt]
            qsball = qnorm.tile([P, QC, Q_LOCAL], F32, name="qsb", tag="qsb")
            for j in range(QC):
                qps = qpool.tile([P, Q_LOCAL], F32, name="qps", tag=qpool.name_tag)
                for c in range(NCH):
                    nc.tensor.matmul(
                        qps[:],
                        lhsT=xbt[:, c, bass.ts(j, P)],
                        rhs=wq_sb[:, c, :],
                        start=(c == 0),
                        stop=(c == NCH - 1),
                    )
                if act_stage:
                    nc.scalar.activation(qsball[:, j, :], qps[:], AF.Copy)
                else:
                    nc.vector.tensor_copy(qsball[:, j, :], qps[:])
            qg = qsball[:].rearrange("p j (g d) -> p j g d", g=2)
            sq = qnorm.tile([P, QC, 2, D_HEAD], F32, name="qsq", tag="qsq")
            nc.vector.tensor_tensor(sq[:], qg, qg, ALU.mult)
            ss = qnorm.tile([P, QC, 2], F32, name="qss", tag="qss")
            nc.vector.tensor_reduce(
                ss[:], sq[:], axis=mybir.AxisListType.X, op=ALU.add
            )
            y = emit_rsqrt_dve(
                qnorm, ss[:].rearrange("p j g -> p (j g)"), QC * 2, tag="q"
            )
            yb = y.rearrange("p (j g) -> p j g", g=2)
            q_hat = qnorm.tile([P, QC, 2, D_HEAD], F32R, name="qhat", tag="qhat")
            nc.vector.tensor_tensor(
                q_hat[:],
                qg,
                yb[:, :, :, None].to_broadcast((P, QC, 2, D_HEAD)),
                ALU.mult,
            )
            for j in range(QC):
                ssl = bass.ts(qt * QC + j, P)
                pt = tpool.tile([P, P], F32R, name="qpt", tag=tpool.name_tag)
                nc.tensor.transpose(
                    pt[:], q_hat[:, j].rearrange("p g d -> p (g d)"), ident_r[:]
                )
                if act_stage:
                    nc.scalar.activation(qhat_all[:, ssl], pt[:], AF.Copy)
                else:
                    nc.vector.tensor_copy(qhat_all[:, ssl], pt[:])

        # shared pools alive for the whole kernel
        opool = ctx.enter_context(tc.tile_pool(name="ops", bufs=1, space="PSUM"))
        ppool = ctx.enter_context(tc.tile_pool(name="probs", bufs=4))
        znpool = ctx.enter_context(tc.tile_pool(name="zn", bufs=3))
        rpool = ctx.enter_context(tc.tile_pool(name="rcp", bufs=3))
        osb = ctx.enter_context(tc.tile_pool(name="osb", bufs=4))
        norm = ctx.enter_context(tc.tile_pool(name="norm", bufs=3))

        class _OpsPool:
            name_tag = "ops"

            @staticmethod
            def tile(shape, dt, name=None, tag=None):
                return opool.tile(shape, dt, name=name, tag="ops")

        def emit_score_exp(h, kb, sps, probs, qsl):
            """S matmul (partition-offset by head) + ACT exp -> bf16 probs.
            sps/probs: [P, QW] APs. Returns the exp instruction."""
            hsl = slice(D_HEAD * h, D_HEAD * (h + 1))
            smm = nc.tensor.matmul(
                sps,
                lhsT=khat_all[hsl, bass.ts(kb, P)],
                rhs=qhat_all[hsl, qsl],
                start=True,
                stop=True,
            )
            return smm, nc.scalar.activation(probs, sps, AF.Exp)

        def emit_pv(h, kb, zq, probs, last):
            """4 PV matmuls accumulating [P, QC, 65] into zq (one PSUM
            zero-region per (qt, h))."""
            for qc in range(QC):
                nc.tensor.matmul(
                    zq[:, qc, :],
                    lhsT=probs[:, bass.ts(qc, P)],
                    rhs=vp[:, h, kb, :],
                    start=(kb == 0 and qc == 0),
                    stop=(last and qc == QC - 1),
                    skip_group_check=True,
                )

        def emit_qt_finish(qt, h_zq_pairs, ztp):
            """normalize z (q-major), pack both heads, PE-transpose into
            z_nT[d_local, s]."""
            zn = znpool.tile([P, QC, P], BF16, name="zn", tag="zn")
            for h, zq in h_zq_pairs:
                rcp = rpool.tile([P, QC], F32, name="rcp", tag="rcp")
                nc.vector.reciprocal(rcp[:], zq[:, :, D_HEAD])
                nc.vector.tensor_tensor(
                    zn[:, :, bass.ts(h, D_HEAD)],
                    zq[:, :, 0:D_HEAD],
                    rcp[:, :, None].to_broadcast((P, QC, D_HEAD)),
                    ALU.mult,
                )
            for qc in range(QC):
                nc.tensor.transpose(ztp[:, qc, :], zn[:, qc, :], ident_b[:])
                nc.vector.tensor_copy(
                    z_nT[:, qt * QW + qc * P : qt * QW + (qc + 1) * P],
                    ztp[:, qc, :],
                )

        def emit_oproj(qt, spool=None, final=False):
            # final q-tile: S slots are free, so pipeline the matmuls
            # 2-wide through them and put half the copies on the idle ACT
            for sbl in range(QC):
                sb = qt * QC + sbl
                ot = osb.tile([P, D_MODEL], BF16, name="ot", tag="ot")
                for half in range(2):
                    if final:
                        ops = spool.tile([P, QW], F32, name="ops", tag="sps")
                    else:
                        ops = opool.tile([P, QW], F32, name="ops", tag="ops")
                    nc.tensor.matmul(
                        ops[:],
                        lhsT=z_nT[:, bass.ts(sb, P)],
                        rhs=wot_sb[:, bass.ts(half, QW)],
                        start=True,
                        stop=True,
                    )
                    if final and half == 0:
                        nc.scalar.activation(
                            ot[:, bass.ts(half, QW)], ops[:], AF.Copy
                        )
                    elif final:
                        nc.vector.tensor_copy(ot[:, bass.ts(half, QW)], ops[:])
                    else:
                        nc.vector.tensor_copy(ot[:, bass.ts(half, QW)], ops[:])
                    if final:
                        nc.sync.dma_start(
                            out[bass.ts(sb, P), bass.ts(half, QW)],
                            ot[:, bass.ts(half, QW)],
                        )
                if not final:
                    nc.sync.dma_start(out[bass.ts(sb, P), :], ot[:])

        # ---- streaming prefix: K/V projections with qt0's attention (both
        # heads) interleaved so ACT starts exp within a few us of launch.
        # PSUM banks: kvps 2 + tps 1 + sps1 2 + zqA 1 + zqB 1 + ops 1 = 8
        with ExitStack() as p1:
            qkps = p1.enter_context(tc.tile_pool(name="kvps", bufs=2, space="PSUM"))
            tps = p1.enter_context(tc.tile_pool(name="tps", bufs=1, space="PSUM"))
            sps1 = p1.enter_context(tc.tile_pool(name="sps1", bufs=2, space="PSUM"))
            zqAp = p1.enter_context(tc.tile_pool(name="zqA", bufs=1, space="PSUM"))
            zqBp = p1.enter_context(tc.tile_pool(name="zqB", bufs=1, space="PSUM"))

            class _TpsPool:
                name_tag = "tps"

                @staticmethod
                def tile(shape, dt, name=None, tag=None):
                    return tps.tile(shape, dt, name=name, tag="tps")

            def emit_k_tail(pend, act_stage=False):
                """k_hat mult + packed PE transposes + khat_all copies for a
                finished segment (software-pipelined one segment behind)."""
                g, ksb, rr, jlo, jhi = pend
                n = jhi - jlo
                kg = ksb[:, jlo:jhi].rearrange("p j (g d) -> p j g d", g=2)
                k_hat = norm.tile([P, n, 2, D_HEAD], F32R, name="khat", tag="khat")
                nc.vector.tensor_tensor(
                    k_hat[:],
                    kg,
                    rr.rearrange("p (j g) -> p j g", g=2)[
                        :, :, :, None
                    ].to_broadcast((P, n, 2, D_HEAD)),
                    ALU.mult,
                )
                last_tp = None
                for j in range(n):
                    ssl = bass.ts(g * QC + jlo + j, P)
                    pt = tps.tile([P, P], F32R, name="pt", tag="tps")
                    last_tp = nc.tensor.transpose(
                        pt[:], k_hat[:, j].rearrange("p g d -> p (g d)"), ident_r[:]
                    )
                    if act_stage:
                        nc.scalar.activation(
                            khat_all[:, ssl], pt[:], AF.Copy, scale=wkc_sb[:]
                        )
                    else:
                        nc.vector.tensor_scalar_mul(khat_all[:, ssl], pt[:], wkc_sb[:])
                return last_tp

            zqA = zqAp.tile([P, QC, D_HEAD + 1], F32, name="zqA")
            zqB = zqBp.tile([P, QC, D_HEAD + 1], F32, name="zqB")
            qsl0 = bass.ts(0, QW)

            def emit_prefix_attn(kblo, kbhi):
                """qt0 attention windows for k-blocks [kblo, kbhi), both
                heads, EB=1. Returns the last exp instruction (used as a
                scheduler ordering anchor)."""
                anchor = None
                for h, zq in ((0, zqA), (1, zqB)):
                    for kb in range(kblo, kbhi):
                        sps = sps1.tile([P, QW], F32, name="sps1", tag="sps1")
                        probs = ppool.tile([P, QW], BF16, name="probs1", tag="probs1")
                        anchor, _ = emit_score_exp(h, kb, sps[:], probs[:], qsl0)
                        emit_pv(h, kb, zq, probs[:], last=(kb == KB - 1))
                return anchor

            def emit_kv_gemms(g, ksb, sqg, jlo, jhi, act_stage=False,
                              pe_anchors=()):
                import bass_rust

                xbt = xball[g]
                for j in range(jlo, jhi):
                    sb = g * QC + j
                    kv_ps = qkps.tile([P, KV_LOCAL], F32, name="kv_ps", tag="kvps")
                    for c in range(NCH):
                        mm = nc.tensor.matmul(
                            kv_ps[:],
                            lhsT=xbt[:, c, bass.ts(j, P)],
                            rhs=wkv_sb[:, c, :],
                            start=(c == 0),
                            stop=(c == NCH - 1),
                        )
                        if pe_anchors:
                            # PE->PE ordering-only deps: keep far-ahead GEMMs
                            # behind older attention windows in the static PE
                            # stream (free at runtime: same-engine order)
                            dset = bass_rust.InstructionNameOrderedSet()
                            for a in pe_anchors:
                                if a is not None:
                                    dset.add(a.ins.name)
                            mm.ins.add_nosync_dependencies_from(dset)
                            pe_anchors = ()
                    # stage K + V' to SBUF on DVE (GPSIMD cannot touch PSUM;
                    # ACT would pace the PSUM bank release behind queued exps
                    # -- except at the very start, when ACT is idle)
                    if act_stage:
                        nc.scalar.activation(
                            ksb[:, j, :], kv_ps[:, 0 : 2 * D_HEAD], AF.Copy
                        )
                        nc.scalar.activation(
                            vp[:, :, sb, 0:D_HEAD],
                            kv_ps[:, 2 * D_HEAD : 4 * D_HEAD].rearrange(
                                "p (h d) -> p h d", h=2
                            ),
                            AF.Copy,
                        )
                    else:
                        nc.vector.tensor_copy(ksb[:, j, :], kv_ps[:, 0 : 2 * D_HEAD])
                        nc.vector.tensor_copy(
                            vp[:, :, sb, 0:D_HEAD],
                            kv_ps[:, 2 * D_HEAD : 4 * D_HEAD].rearrange(
                                "p (h d) -> p h d", h=2
                            ),
                        )
                    kgj = ksb[:, j, :].rearrange("p (g d) -> p g d", g=2)
                    nc.vector.tensor_tensor(sqg[:, j], kgj, kgj, ALU.mult)

            def emit_k_stats(g, ksb, sqg, jlo, jhi):
                n = jhi - jlo
                ssg = norm.tile([P, n, 2], F32, name="ssg", tag="ssg")
                nc.vector.tensor_reduce(
                    ssg[:], sqg[:, jlo:jhi], axis=mybir.AxisListType.X, op=ALU.add
                )
                rr = emit_rsqrt_dve(
                    norm, ssg[:].rearrange("p j g -> p (j g)"), n * 2, tag="k"
                )
                return (g, ksb, rr, jlo, jhi)

            # group 0 is split into two pairs so the first attention windows
            # (and with them ACT's exp stream) start as early as possible
            segs = [(0, 0, 2), (0, 2, 4)] + [(g, 0, QC) for g in range(1, SB // QC)]
            ktiles = {}
            pending = None
            pe_anch = {}
            for si, (g, jlo, jhi) in enumerate(segs):
                if jlo == 0:
                    ktiles[g] = (
                        norm.tile([P, QC, 2 * D_HEAD], F32, name="ksb", tag="ksb"),
                        norm.tile([P, QC, 2, D_HEAD], F32, name="sqg", tag="sqg"),
                    )
                ksb, sqg = ktiles[g]
                # de-prioritize far-ahead kv GEMMs for the tile scheduler so
                # ready attention windows always win the PE; PE->PE nosync
                # anchors also keep them behind older windows in the static
                # stream (free at runtime: same-engine program order)
                with tc.high_priority(offset=-(si * 800)):
                    emit_kv_gemms(g, ksb, sqg, jlo, jhi, act_stage=(si == 0))
                if si == 0:
                    # q side of qt0: overlaps the first k pair's stats chain
                    emit_q_side_group(0, _OpsPool, _TpsPool, act_stage=True)
                if pending is not None:
                    pg, _, _, pjlo, pjhi = pending
                    tp_a = emit_k_tail(pending)
                    s_a = emit_prefix_attn(pg * QC + pjlo, pg * QC + pjhi)
                    pe_anch[si - 1] = (tp_a, s_a)
                pending = emit_k_stats(g, ksb, sqg, jlo, jhi)
                if si == 0:
                    # no pipelining for the very first pair: its khat (and the
                    # first exp windows) are the critical path
                    tp_a = emit_k_tail(pending, act_stage=True)
                    s_a = emit_prefix_attn(0, 2)
                    pe_anch[0] = (tp_a, s_a)
                    pending = None
                if (g, jlo) == (3, 0):
                    with tc.high_priority(offset=-2400):
                        qs1 = emit_q_side_a(1, _OpsPool)
                if (g, jlo) == (4, 0):
                    with tc.high_priority(offset=-2400):
                        emit_q_side_b(qs1, _TpsPool)
            emit_k_tail(pending)
            emit_prefix_attn(SB - QC, SB)
            ztp0 = zqBp.tile([P, QC, P], BF16, name="ztp0", tag="zqB")
            emit_qt_finish(0, ((0, zqA), (1, zqB)), ztp0)

        # ---- steady state: q-tiles 1..7, ACT-saturated exp pipeline.
        # PSUM banks: 2 score slots x3 banks, z accumulator 1, ops 1 = 8
        with ExitStack() as p2:
            spool = p2.enter_context(tc.tile_pool(name="sps", bufs=2, space="PSUM"))
            zqpool = p2.enter_context(tc.tile_pool(name="zqps", bufs=1, space="PSUM"))

            for qt in range(1, QT):
                qsl = bass.ts(qt, QW)
                zn = znpool.tile([P, QC, P], BF16, name="zn", tag="zn")
                for h in range(HEADS_LOCAL):
                    zq = zqpool.tile([P, QC, D_HEAD + 1], F32, name="zq", tag="zq")
                    for kb0 in [0] + list(range(2, KB, EXP_BATCH)):
                        nb = 2 if kb0 == 0 else min(EXP_BATCH, KB - kb0)
                        sps = spool.tile(
                            [P, EXP_BATCH, QW], F32, name="sps", tag="sps"
                        )
                        probs = ppool.tile(
                            [P, EXP_BATCH, QW], BF16, name="probs", tag="probs"
                        )
                        for j in range(nb):
                            kb = kb0 + j
                            hsl = slice(D_HEAD * h, D_HEAD * (h + 1))
                            nc.tensor.matmul(
                                sps[:, j, :],
                                lhsT=khat_all[hsl, bass.ts(kb, P)],
                                rhs=qhat_all[hsl, qsl],
                                start=True,
                                stop=True,
                            )
                        nc.scalar.activation(
                            probs[:, 0:nb, :], sps[:, 0:nb, :], AF.Exp
                        )
                        # all 128 PV matmuls form ONE PSUM accumulation group
                        # (zq spans a single 2KB zero region)
                        for j in range(nb):
                            kb = kb0 + j
                            emit_pv(h, kb, zq, probs[:, j, :], last=(kb == KB - 1))
                        # software-pipelined work emitted under the exp shadow:
                        # h0: O-projection of the previous q-tile
                        # h1: q side (GEMM+norm+transposes) of the next q-tile
                        if kb0 == 2 and h == 0:
                            emit_oproj(qt - 1)
                        if kb0 == 2 and h == 1 and qt < QT - 1:
                            emit_q_side_group(qt + 1, _OpsPool, _OpsPool)
                    # normalize in q-major: z = z / rowsum (col 64)
                    rcp = rpool.tile([P, QC], F32, name="rcp", tag="rcp")
                    nc.vector.reciprocal(rcp[:], zq[:, :, D_HEAD])
                    nc.vector.tensor_tensor(
                        zn[:, :, bass.ts(h, D_HEAD)],
                        zq[:, :, 0:D_HEAD],
                        rcp[:, :, None].to_broadcast((P, QC, D_HEAD)),
                        ALU.mult,
                    )
                # transpose both heads at once into z_nT[d_local, s]
                ztp = zqpool.tile([P, QC, P], BF16, name="ztp", tag="zq")
                for qc in range(QC):
                    nc.tensor.transpose(ztp[:, qc, :], zn[:, qc, :], ident_b[:])
                    nc.vector.tensor_copy(
                        z_nT[:, qt * QW + qc * P : qt * QW + (qc + 1) * P],
                        ztp[:, qc, :],
                    )
            emit_oproj(QT - 1, spool, final=True)

    if split_waits:
        _split_excess_waits(nc)
    return nc


def shard_inputs(x, Wqkv, bqkv, Wo, bo, wq, wk):
    import ml_dtypes

    x2 = np.ascontiguousarray(np.asarray(x, dtype=np.float32).reshape(SEQ, D_MODEL))
    Wqkv = np.asarray(Wqkv, dtype=np.float32)
    bqkv = np.asarray(bqkv, dtype=np.float32)
    Wo = np.asarray(Wo, dtype=np.float32)
    wq = np.asarray(wq, dtype=np.float32)
    wk = np.asarray(wk, dtype=np.float32)

    xta = np.zeros((DM_AUG, SEQ), np.float32)
    xta[:D_MODEL] = x2.T
    xta[D_MODEL] = 1.0
    xtb = np.ascontiguousarray(xta.astype(ml_dtypes.bfloat16))

    # per-partition scale for khat_all's packed [d0|d1] feature rows
    wkc = np.ascontiguousarray(np.tile((wq * wk).reshape(D_HEAD), 2).reshape(P, 1))

    in_maps = []
    for c in range(N_CORES):
        rows, brows = [], []
        for part in range(3):
            for h in (HEADS_LOCAL * c, HEADS_LOCAL * c + 1):
                sl = slice(part * D_MODEL + h * D_HEAD, part * D_MODEL + (h + 1) * D_HEAD)
                rows.append(Wqkv[sl])
                brows.append(bqkv[sl])
        Wl = np.concatenate(rows, 0)          # [384, 1024] rows [q0|q1|k0|k1|v0|v1]
        bl = np.concatenate(brows, 0)         # [384]
        wqkvta = np.zeros((DM_AUG, 384), np.float32)
        wqkvta[:D_MODEL] = Wl.T
        wqkvta[D_MODEL] = bl
        wkvt = np.ascontiguousarray(
            wqkvta[:, Q_LOCAL:].astype(ml_dtypes.bfloat16)
        )                                                              # [1152, 256]
        wqt = np.ascontiguousarray(
            wqkvta[:, :Q_LOCAL].astype(ml_dtypes.bfloat16)
        )                                                              # [1152, 128]
        cols = slice(HEADS_LOCAL * c * D_HEAD, (HEADS_LOCAL * c + HEADS_LOCAL) * D_HEAD)
        wotc = np.ascontiguousarray(Wo[:, cols].T.astype(ml_dtypes.bfloat16))
        in_maps.append(
            {
                "xtb": xtb,
                "wkvt": wkvt,
                "wqt": wqt,
                "wot": wotc,
                "wkc": wkc,
            }
        )
    return in_maps


_NC_CACHE = {}
LAST_RESULT = None


def kernel(x, Wqkv, bqkv, Wo, bo, wq, wk):
    import os
    from concourse.bass_utils import run_bass_kernel_spmd

    global LAST_RESULT
    assert np.asarray(x).shape == (1, SEQ, D_MODEL)
    in_maps = shard_inputs(x, Wqkv, bqkv, Wo, bo, wq, wk)
    if "nc" not in _NC_CACHE:
        _NC_CACHE["nc"] = build_core_kernel()
    nc = _NC_CACHE["nc"]
    trace = bool(int(os.environ.get("BASS_KERNEL_TRACE", "0")))
    res = run_bass_kernel_spmd(nc, in_maps, list(range(N_CORES)), trace=trace)
    LAST_RESULT = res
    acc = np.zeros((SEQ, D_MODEL), np.float64)
    for c in range(N_CORES):
        acc += res.results[c]["out"].astype(np.float64)
    acc += np.asarray(bo, dtype=np.float64)
    return acc.astype(np.float32).reshape(1, SEQ, D_MODEL)



# revision 60
# speedup vs baseline: 1.0488x; 1.0076x over previous
"""Trainium2 Bass kernel for a 16-head attention block (d_model=1024, seq=4096).

Sharding: tensor-parallel over heads. Each of the 8 cores computes QKV
projections, RMSNorm(q,k), full softmax(QK^T)V attention for its 2 heads,
and a partial O-projection (its heads' slice of the contraction). The host
sums the 8 partial outputs (bf16 partials) and adds the output bias.

Per-core dataflow (k-first, attention is ACT/exp-bound so everything else
is arranged to hide under it). x ships ONCE as bf16 and stays resident in
SBUF (8 chunk tiles, 1KB descriptor runs) — the cost model serializes all
DMA on a shared 360GB/s device, so halving x traffic halves the prefix:
  phase 1 (serial prefix): k,v projections (bf16 GEMM, moving dim 256),
           RMSNorm(k) with wq*wk folded into the k side, PE-transpose
           k_hat into [64, s] fp32r tiles, V -> bf16 [k, 65] chunks with a
           fused ones column. The q side for the first q-tile is
           interleaved into the prefix tail (ACT Sqrt path).
  phase 2 (ACT-bound steady state): per (q-tile 512, head):
           S[k,q] blocks via khatT.T @ qhatT (K=64 fp32r), exp on ACT ->
           bf16 probs, z[q,65] += probs_chunk.T @ V' with probs as the
           stationary operand (65-column moving operand halves PE time).
           Row 64 = softmax denominator; normalize in q-major on DVE,
           PE-transpose both heads at once into z_nT[d_local, s] bf16.
           The NEXT q-tile's q side runs under the exp shadow: bf16
           x @ Wq GEMM (N=128 bf16 runs at 1 cyc/row; fp32r would be 4x),
           RMSNorm(q) with a Newton-iteration rsqrt on DVE (keeps the
           ACT table on Exp), PE-transposes into qhatT.
  phase 3: out[s,dm] partial = z_nT.T @ WoT (bf16), PSUM->SBUF bf16 -> HBM,
           software-pipelined one q-tile behind attention.
"""

import numpy as np
from contextlib import ExitStack

import concourse.bass as bass
import concourse.tile as tile
from concourse import mybir
from concourse.masks import make_identity

F32 = mybir.dt.float32
F32R = mybir.dt.float32r
BF16 = mybir.dt.bfloat16
I32 = mybir.dt.int32
AF = mybir.ActivationFunctionType
ALU = mybir.AluOpType

D_MODEL = 1024
SEQ = 4096
N_HEADS = 16
D_HEAD = 64
N_CORES = 8
HEADS_LOCAL = 2
P = 128
DM_AUG = D_MODEL + P                     # 1152 rows: x^T plus ones-row block
NCH = DM_AUG // P                        # 9 contraction chunks
KV_LOCAL = 2 * HEADS_LOCAL * D_HEAD      # 256: [k0|k1|v0|v1]
Q_LOCAL = HEADS_LOCAL * D_HEAD           # 128: [q0|q1]
SB = SEQ // P                            # 32 s-blocks
QT = 8                                   # q-tiles of 512
QW = SEQ // QT                           # 512
QC = QW // P                             # 4 q-chunks of 128 per q-tile
KB = SEQ // P                            # 32 k-blocks
EXP_BATCH = 3
PREFIX_SLOT_MS = 0.0025
EPS = 1e-6
RSQRT_MAGIC = 0x5F3759DF


MAX_WAITS = 1


def _split_excess_waits(nc):
    """This walrus build rejects instructions carrying more than one or two
    sync-wait commands (CTRL and pseudo-DMA structs especially). Rewrite every
    instruction with more than MAX_WAITS waits into a chain of same-engine
    NoOps each carrying MAX_WAITS waits, followed by the original."""
    import bass_rust

    n_new = 0
    for f in nc.m.functions:
        for bb in f.blocks:
            changed = False
            out = []
            for ins in bb.instructions:
                si = ins.sync_info
                waits = list(si.on_wait) if si is not None and si.on_wait else []
                if len(waits) > MAX_WAITS:
                    changed = True
                    ncar = len(waits) - MAX_WAITS
                    for i in range(0, ncar, MAX_WAITS):
                        chunk = waits[i : min(i + MAX_WAITS, ncar)]
                        nop = mybir.InstNoOp(
                            name=f"{ins.name}-wsplit{i}", ins=[], outs=[]
                        )
                        nop.engine = ins.engine
                        nop.sync_info = bass_rust.SyncInfo(
                            on_wait=chunk, on_update=[]
                        )
                        out.append(nop)
                        n_new += 1
                    ins.sync_info = bass_rust.SyncInfo(
                        on_wait=waits[ncar:], on_update=si.on_update
                    )
                out.append(ins)
            if changed:
                bb.instructions = out
    return n_new


def build_core_kernel(split_waits=True):
    nc = bass.Bass()
    xtb = nc.declare_dram_parameter("xtb", [DM_AUG, SEQ], BF16, isOutput=False)
    wkvt = nc.declare_dram_parameter("wkvt", [DM_AUG, KV_LOCAL], BF16, isOutput=False)
    wqt = nc.declare_dram_parameter("wqt", [DM_AUG, Q_LOCAL], BF16, isOutput=False)
    wot = nc.declare_dram_parameter("wot", [P, D_MODEL], BF16, isOutput=False)
    wkc = nc.declare_dram_parameter("wkc", [P, 1], F32, isOutput=False)
    out = nc.declare_dram_parameter("out", [SEQ, D_MODEL], BF16, isOutput=True)

    xtb_r = xtb.rearrange("(c p) s -> p c s", p=P)       # [128, 9, 4096]
    wkvt_r = wkvt.rearrange("(c p) f -> p c f", p=P)     # [128, 9, 256]
    wqt_r = wqt.rearrange("(c p) f -> p c f", p=P)       # [128, 9, 128]

    with ExitStack() as ctx:
        tc = ctx.enter_context(tile.TileContext(nc))

        const = ctx.enter_context(tc.tile_pool(name="const", bufs=1))
        persist = ctx.enter_context(tc.tile_pool(name="persist", bufs=1))

        # DMA order matters: q side of qt0 runs first (xb0 + wq), then the
        # k side needs the full wkv
        xb0 = const.tile([P, NCH, QW], BF16, name="xb0")
        wkv_sb = const.tile([P, NCH, KV_LOCAL], BF16)
        wq_sb = const.tile([P, NCH, Q_LOCAL], BF16)
        nc.sync.dma_start(wkv_sb[:, 0:1, :], wkvt_r[:, 0:1, :])
        nc.sync.dma_start(xb0[:, :, 0:QW // 2], xtb_r[:, :, 0 : QW // 2])
        nc.sync.dma_start(wkv_sb[:, 1:NCH, :], wkvt_r[:, 1:NCH, :])
        nc.sync.dma_start(xb0[:, :, QW // 2 : QW], xtb_r[:, :, QW // 2 : QW])
        nc.sync.dma_start(wq_sb[:], wqt_r)
        ident_f = const.tile([P, P], F32)
        make_identity(nc, ident_f)
        ident_r = const.tile([P, P], F32R)
        nc.scalar.activation(ident_r[:], ident_f[:], AF.Copy)
        ident_b = const.tile([P, P], BF16)
        nc.vector.tensor_copy(ident_b[:], ident_f[:])
        wkc_sb = const.tile([P, 1], F32)
        nc.sync.dma_start(wkc_sb[:], wkc[:])
        wot_sb = const.tile([P, D_MODEL], BF16)
        # bias rows live in chunk 8 / partition 0 of the augmented weights;
        # broadcast them across partitions once (Pool) so the projections can
        # skip the 9th GEMM chunk and fold the bias into the staging TT-add
        ones1 = const.tile([1, P], BF16, name="ones1")
        nc.gpsimd.memset(ones1[:], 1.0)
        kvb_sb = const.tile([P, KV_LOCAL], F32)
        qb_sb = const.tile([P, Q_LOCAL], F32)

        # attention operands packed 2-heads-per-tile: khat_all/qhat_all
        # [128, s] fp32r with h0 in partitions 0-63, h1 in 64-127 (scores use
        # partition-offset matmuls, K=64). V' in [k, 65] bf16 per (head,
        # k-block) with a fused ones denominator column.
        qhat_all = persist.tile([P, SEQ], F32R, name="qhat_all")
        khat_all = persist.tile([P, SEQ], F32R, name="khat_all")
        vp = persist.tile([P, HEADS_LOCAL, KB, D_HEAD + 1], BF16)
        nc.gpsimd.memset(vp[:, :, :, D_HEAD : D_HEAD + 1], 1.0)
        z_nT = persist.tile([P, SEQ], BF16)

        # resident bf16 copy of x^T (augmented): 8 chunk tiles of 512 s-cols,
        # loaded once (1KB descriptor runs; serves both kv- and q-GEMMs).
        # wq comes right after xb0 so the first q-tile's q side can run at
        # the top of the prefix.
        xball = [xb0] + [
            persist.tile([P, NCH, QW], BF16, name=f"xb{d}") for d in range(1, QT)
        ]
        for d in range(1, QT):
            nc.sync.dma_start(xball[d][:], xtb_r[:, :, bass.ts(d, QW)])
        nc.sync.dma_start(wot_sb[:], wot[:])

        qnorm = ctx.enter_context(tc.tile_pool(name="qnorm", bufs=3))

        def emit_rsqrt_dve(pool, ss, n, tag=""):
            """rr = rsqrt(ss/64 + eps) via bit-trick seed + 2 Newton steps,
            all on DVE (keeps ACT free for exp). ss/rr: [P, n] fp32."""
            ms = pool.tile([P, n], F32, name="rms" + tag, tag="rms" + tag)
            nc.vector.tensor_scalar(
                ms[:], ss, 1.0 / D_HEAD, EPS, op0=ALU.mult, op1=ALU.add
            )
            xh = pool.tile([P, n], F32, name="rxh" + tag, tag="rxh" + tag)
            nc.vector.tensor_scalar(xh[:], ms[:], 0.5, None, op0=ALU.mult)
            iy = pool.tile([P, n], I32, name="riy" + tag, tag="riy" + tag)
            nc.vector.tensor_scalar(
                iy[:], ms[:].bitcast(I32), 1, None, op0=ALU.logical_shift_right
            )
            nc.vector.tensor_scalar(
                iy[:], iy[:], -1, RSQRT_MAGIC, op0=ALU.mult, op1=ALU.add
            )
            y = iy[:].bitcast(F32)
            for it in range(2):
                y2 = pool.tile([P, n], F32, name=f"ry2_{it}" + tag, tag=f"ry2_{it}" + tag)
                nc.vector.tensor_tensor(y2[:], y, y, ALU.mult)
                nc.vector.tensor_tensor(y2[:], y2[:], xh[:], ALU.mult)
                nc.vector.tensor_scalar(
                    y2[:], y2[:], -1.0, 1.5, op0=ALU.mult, op1=ALU.add
                )
                yn = pool.tile([P, n], F32, name=f"ryn_{it}" + tag, tag=f"ryn_{it}" + tag)
                nc.vector.tensor_tensor(yn[:], y, y2[:], ALU.mult)
                y = yn[:]
            return y

        def emit_q_side_a(qt, qpool, act_stage=False):
            """part A of the q side: GEMMs + staging + stats + rsqrt."""
            xbt = xball[qt]
            qsball = qnorm.tile([P, QC, Q_LOCAL], F32, name="qsb", tag="qsb")
            for j in range(QC):
                qps = qpool.tile([P, Q_LOCAL], F32, name="qps", tag=qpool.name_tag)
                for c in range(NCH - 1):
                    nc.tensor.matmul(
                        qps[:],
                        lhsT=xbt[:, c, bass.ts(j, P)],
                        rhs=wq_sb[:, c, :],
                        start=(c == 0),
                        stop=(c == NCH - 2),
                    )
                nc.vector.tensor_tensor(
                    qsball[:, j, :], qps[:], qb_sb[:], ALU.add
                )
            qg = qsball[:].rearrange("p j (g d) -> p j g d", g=2)
            sq = qnorm.tile([P, QC, 2, D_HEAD], F32, name="qsq", tag="qsq")
            nc.vector.tensor_tensor(sq[:], qg, qg, ALU.mult)
            ss = qnorm.tile([P, QC, 2], F32, name="qss", tag="qss")
            nc.vector.tensor_reduce(
                ss[:], sq[:], axis=mybir.AxisListType.X, op=ALU.add
            )
            y = emit_rsqrt_dve(
                qnorm, ss[:].rearrange("p j g -> p (j g)"), QC * 2, tag="q"
            )
            return (qt, qsball, y)

        def emit_q_side_b(part_a, tpool, act_stage=False):
            """part B: q_hat mult + packed transposes + qhat_all copies."""
            qt, qsball, y = part_a
            qg = qsball[:].rearrange("p j (g d) -> p j g d", g=2)
            yb = y.rearrange("p (j g) -> p j g", g=2)
            q_hat = qnorm.tile([P, QC, 2, D_HEAD], F32R, name="qhat", tag="qhat")
            nc.vector.tensor_tensor(
                q_hat[:],
                qg,
                yb[:, :, :, None].to_broadcast((P, QC, 2, D_HEAD)),
                ALU.mult,
            )
            for j in range(QC):
                ssl = bass.ts(qt * QC + j, P)
                pt = tpool.tile([P, P], F32R, name="qpt", tag=tpool.name_tag)
                nc.tensor.transpose(
                    pt[:], q_hat[:, j].rearrange("p g d -> p (g d)"), ident_r[:]
                )
                if act_stage:
                    nc.scalar.activation(qhat_all[:, ssl], pt[:], AF.Copy)
                else:
                    nc.vector.tensor_copy(qhat_all[:, ssl], pt[:])

        def emit_q_side_group(qt, qpool, tpool, act_stage=False):
            emit_q_side_b(
                emit_q_side_a(qt, qpool, act_stage), tpool, act_stage
            )

        def _unused_q_side(qt, qpool, tpool, act_stage=False):
            """bf16 x @ Wq for one q-tile (4 s-blocks) + RMSNorm(q) + PE
            transposes into qhatT. Stats batched across the 4 s-blocks so the
            DVE Newton-rsqrt chain runs once on [P, 8]. act_stage: route the
            PSUM->SBUF staging copies through ACT (only safe before the exp
            stream starts, when ACT is idle)."""
            xbt = xball[qt]
            qsball = qnorm.tile([P, QC, Q_LOCAL], F32, name="qsb", tag="qsb")
            for j in range(QC):
                qps = qpool.tile([P, Q_LOCAL], F32, name="qps", tag=qpool.name_tag)
                for c in range(NCH - 1):
                    nc.tensor.matmul(
                        qps[:],
                        lhsT=xbt[:, c, bass.ts(j, P)],
                        rhs=wq_sb[:, c, :],
                        start=(c == 0),
                        stop=(c == NCH - 2),
                    )
                nc.vector.tensor_tensor(
                    qsball[:, j, :], qps[:], qb_sb[:], ALU.add
                )
            qg = qsball[:].rearrange("p j (g d) -> p j g d", g=2)
            sq = qnorm.tile([P, QC, 2, D_HEAD], F32, name="qsq", tag="qsq")
            nc.vector.tensor_tensor(sq[:], qg, qg, ALU.mult)
            ss = qnorm.tile([P, QC, 2], F32, name="qss", tag="qss")
            nc.vector.tensor_reduce(
                ss[:], sq[:], axis=mybir.AxisListType.X, op=ALU.add
            )
            y = emit_rsqrt_dve(
                qnorm, ss[:].rearrange("p j g -> p (j g)"), QC * 2, tag="q"
            )
            yb = y.rearrange("p (j g) -> p j g", g=2)
            q_hat = qnorm.tile([P, QC, 2, D_HEAD], F32R, name="qhat", tag="qhat")
            nc.vector.tensor_tensor(
                q_hat[:],
                qg,
                yb[:, :, :, None].to_broadcast((P, QC, 2, D_HEAD)),
                ALU.mult,
            )
            for j in range(QC):
                ssl = bass.ts(qt * QC + j, P)
                pt = tpool.tile([P, P], F32R, name="qpt", tag=tpool.name_tag)
                nc.tensor.transpose(
                    pt[:], q_hat[:, j].rearrange("p g d -> p (g d)"), ident_r[:]
                )
                if act_stage:
                    nc.scalar.activation(qhat_all[:, ssl], pt[:], AF.Copy)
                else:
                    nc.vector.tensor_copy(qhat_all[:, ssl], pt[:])

        # shared pools alive for the whole kernel
        opool = ctx.enter_context(tc.tile_pool(name="ops", bufs=1, space="PSUM"))
        ppool = ctx.enter_context(tc.tile_pool(name="probs", bufs=4))
        znpool = ctx.enter_context(tc.tile_pool(name="zn", bufs=3))
        rpool = ctx.enter_context(tc.tile_pool(name="rcp", bufs=3))
        osb = ctx.enter_context(tc.tile_pool(name="osb", bufs=4))
        norm = ctx.enter_context(tc.tile_pool(name="norm", bufs=3))

        class _OpsPool:
            name_tag = "ops"

            @staticmethod
            def tile(shape, dt, name=None, tag=None):
                return opool.tile(shape, dt, name=name, tag="ops")

        def emit_score_exp(h, kb, sps, probs, qsl):
            """S matmul (partition-offset by head) + ACT exp -> bf16 probs.
            sps/probs: [P, QW] APs. Returns the exp instruction."""
            hsl = slice(D_HEAD * h, D_HEAD * (h + 1))
            smm = nc.tensor.matmul(
                sps,
                lhsT=khat_all[hsl, bass.ts(kb, P)],
                rhs=qhat_all[hsl, qsl],
                start=True,
                stop=True,
            )
            return smm, nc.scalar.activation(probs, sps, AF.Exp)

        def emit_pv(h, kb, zq, probs, last):
            """4 PV matmuls accumulating [P, QC, 65] into zq (one PSUM
            zero-region per (qt, h))."""
            for qc in range(QC):
                nc.tensor.matmul(
                    zq[:, qc, :],
                    lhsT=probs[:, bass.ts(qc, P)],
                    rhs=vp[:, h, kb, :],
                    start=(kb == 0 and qc == 0),
                    stop=(last and qc == QC - 1),
                    skip_group_check=True,
                )

        def emit_qt_finish(qt, h_zq_pairs, ztp):
            """normalize z (q-major), pack both heads, PE-transpose into
            z_nT[d_local, s]."""
            zn = znpool.tile([P, QC, P], BF16, name="zn", tag="zn")
            for h, zq in h_zq_pairs:
                rcp = rpool.tile([P, QC], F32, name="rcp", tag="rcp")
                nc.vector.reciprocal(rcp[:], zq[:, :, D_HEAD])
                nc.vector.tensor_tensor(
                    zn[:, :, bass.ts(h, D_HEAD)],
                    zq[:, :, 0:D_HEAD],
                    rcp[:, :, None].to_broadcast((P, QC, D_HEAD)),
                    ALU.mult,
                )
            for qc in range(QC):
                nc.tensor.transpose(ztp[:, qc, :], zn[:, qc, :], ident_b[:])
                nc.vector.tensor_copy(
                    z_nT[:, qt * QW + qc * P : qt * QW + (qc + 1) * P],
                    ztp[:, qc, :],
                )

        def emit_oproj(qt, spool=None, final=False):
            # final q-tile: S slots are free, so pipeline the matmuls
            # 2-wide through them and put half the copies on the idle ACT
            for sbl in range(QC):
                sb = qt * QC + sbl
                ot = osb.tile([P, D_MODEL], BF16, name="ot", tag="ot")
                for half in range(2):
                    if final:
                        ops = spool.tile([P, QW], F32, name="ops", tag="sps")
                    else:
                        ops = opool.tile([P, QW], F32, name="ops", tag="ops")
                    nc.tensor.matmul(
                        ops[:],
                        lhsT=z_nT[:, bass.ts(sb, P)],
                        rhs=wot_sb[:, bass.ts(half, QW)],
                        start=True,
                        stop=True,
                    )
                    if final and half == 0:
                        nc.scalar.activation(
                            ot[:, bass.ts(half, QW)], ops[:], AF.Copy
                        )
                    elif final:
                        nc.vector.tensor_copy(ot[:, bass.ts(half, QW)], ops[:])
                    else:
                        nc.vector.tensor_copy(ot[:, bass.ts(half, QW)], ops[:])
                    if final:
                        nc.sync.dma_start(
                            out[bass.ts(sb, P), bass.ts(half, QW)],
                            ot[:, bass.ts(half, QW)],
                        )
                if not final:
                    nc.sync.dma_start(out[bass.ts(sb, P), :], ot[:])

        # ---- streaming prefix: K/V projections with qt0's attention (both
        # heads) interleaved so ACT starts exp within a few us of launch.
        # PSUM banks: kvps 2 + tps 1 + sps1 2 + zqA 1 + zqB 1 + ops 1 = 8
        with ExitStack() as p1:
            qkps = p1.enter_context(tc.tile_pool(name="kvps", bufs=2, space="PSUM"))
            tps = p1.enter_context(tc.tile_pool(name="tps", bufs=1, space="PSUM"))
            sps1 = p1.enter_context(tc.tile_pool(name="sps1", bufs=2, space="PSUM"))
            zqAp = p1.enter_context(tc.tile_pool(name="zqA", bufs=1, space="PSUM"))
            zqBp = p1.enter_context(tc.tile_pool(name="zqB", bufs=1, space="PSUM"))

            class _TpsPool:
                name_tag = "tps"

                @staticmethod
                def tile(shape, dt, name=None, tag=None):
                    return tps.tile(shape, dt, name=name, tag="tps")

            def emit_k_tail(pend, act_stage=False):
                """k_hat mult + packed PE transposes + khat_all copies for a
                finished segment (software-pipelined one segment behind)."""
                g, ksb, rr, jlo, jhi = pend
                n = jhi - jlo
                kg = ksb[:, jlo:jhi, 0 : 2 * D_HEAD].rearrange(
                    "p j (g d) -> p j g d", g=2
                )
                k_hat = norm.tile([P, n, 2, D_HEAD], F32R, name="khat", tag="khat")
                nc.vector.tensor_tensor(
                    k_hat[:],
                    kg,
                    rr.rearrange("p (j g) -> p j g", g=2)[
                        :, :, :, None
                    ].to_broadcast((P, n, 2, D_HEAD)),
                    ALU.mult,
                )
                last_tp = None
                for j in range(n):
                    ssl = bass.ts(g * QC + jlo + j, P)
                    pt = tps.tile([P, P], F32R, name="pt", tag="tps")
                    last_tp = nc.tensor.transpose(
                        pt[:], k_hat[:, j].rearrange("p g d -> p (g d)"), ident_r[:]
                    )
                    if act_stage:
                        nc.scalar.activation(
                            khat_all[:, ssl], pt[:], AF.Copy, scale=wkc_sb[:]
                        )
                    else:
                        nc.vector.tensor_scalar_mul(khat_all[:, ssl], pt[:], wkc_sb[:])
                return last_tp

            zqA = zqAp.tile([P, QC, D_HEAD + 1], F32, name="zqA")
            zqB = zqBp.tile([P, QC, D_HEAD + 1], F32, name="zqB")
            qsl0 = bass.ts(0, QW)

            def emit_prefix_attn(kblo, kbhi):
                """qt0 attention windows for k-blocks [kblo, kbhi), both
                heads, EB=1. Returns the last exp instruction (used as a
                scheduler ordering anchor)."""
                anchor = None
                for h, zq in ((0, zqA), (1, zqB)):
                    for kb in range(kblo, kbhi):
                        sps = sps1.tile([P, QW], F32, name="sps1", tag="sps1")
                        probs = ppool.tile([P, QW], BF16, name="probs1", tag="probs1")
                        anchor, _ = emit_score_exp(h, kb, sps[:], probs[:], qsl0)
                        emit_pv(h, kb, zq, probs[:], last=(kb == KB - 1))
                return anchor

            def emit_kv_gemms(g, ksb, sqg, jlo, jhi, act_stage=False,
                              pe_anchors=()):
                import bass_rust

                xbt = xball[g]
                for j in range(jlo, jhi):
                    sb = g * QC + j
                    kv_ps = qkps.tile([P, KV_LOCAL], F32, name="kv_ps", tag="kvps")
                    for c in range(NCH - 1):
                        mm = nc.tensor.matmul(
                            kv_ps[:],
                            lhsT=xbt[:, c, bass.ts(j, P)],
                            rhs=wkv_sb[:, c, :],
                            start=(c == 0),
                            stop=(c == NCH - 2),
                        )
                        if pe_anchors:
                            # PE->PE ordering-only deps: keep far-ahead GEMMs
                            # behind older attention windows in the static PE
                            # stream (free at runtime: same-engine order)
                            dset = bass_rust.InstructionNameOrderedSet()
                            for a in pe_anchors:
                                if a is not None:
                                    dset.add(a.ins.name)
                            mm.ins.add_nosync_dependencies_from(dset)
                            pe_anchors = ()
                    # one combined K|V staging op (PSUM->SBUF) that also adds
                    # the qkv bias (saves the 9th GEMM chunk); V' is then
                    # extracted SBUF->SBUF on the otherwise-idle Pool engine
                    nc.vector.tensor_tensor(
                        ksb[:, j, :], kv_ps[:], kvb_sb[:], ALU.add
                    )
                    nc.gpsimd.tensor_copy(
                        vp[:, :, sb, 0:D_HEAD],
                        ksb[:, j, 2 * D_HEAD : 4 * D_HEAD].rearrange(
                            "p (h d) -> p h d", h=2
                        ),
                    )
                    kgj = ksb[:, j, 0 : 2 * D_HEAD].rearrange("p (g d) -> p g d", g=2)
                    nc.vector.tensor_tensor(sqg[:, j], kgj, kgj, ALU.mult)

            def emit_k_stats(g, ksb, sqg, jlo, jhi):
                n = jhi - jlo
                ssg = norm.tile([P, n, 2], F32, name="ssg", tag="ssg")
                nc.vector.tensor_reduce(
                    ssg[:], sqg[:, jlo:jhi], axis=mybir.AxisListType.X, op=ALU.add
                )
                rr = emit_rsqrt_dve(
                    norm, ssg[:].rearrange("p j g -> p (j g)"), n * 2, tag="k"
                )
                return (g, ksb, rr, jlo, jhi)

            # group 0 is split into two pairs so the first attention windows
            # (and with them ACT's exp stream) start as early as possible
            # broadcast the bias rows (chunk 8, partition 0) across all
            # partitions via a K=1 ones-matmul; PE is idle this early
            for bias_sb, w_sb, ncols in (
                (kvb_sb, wkv_sb, KV_LOCAL),
                (qb_sb, wq_sb, Q_LOCAL),
            ):
                bps = opool.tile([P, ncols], F32, name="bps", tag="ops")
                nc.tensor.matmul(
                    bps[:],
                    lhsT=ones1[:],
                    rhs=w_sb[0:1, NCH - 1, :],
                    start=True,
                    stop=True,
                )
                nc.vector.tensor_copy(bias_sb[:], bps[:])

            segs = [(0, 0, 2), (0, 2, 4)] + [(g, 0, QC) for g in range(1, SB // QC)]
            ktiles = {}
            pending = None
            pe_anch = {}
            for si, (g, jlo, jhi) in enumerate(segs):
                if jlo == 0:
                    ktiles[g] = (
                        norm.tile([P, QC, KV_LOCAL], F32, name="ksb", tag="ksb"),
                        norm.tile([P, QC, 2, D_HEAD], F32, name="sqg", tag="sqg"),
                    )
                ksb, sqg = ktiles[g]
                # de-prioritize far-ahead kv GEMMs for the tile scheduler so
                # ready attention windows always win the PE; PE->PE nosync
                # anchors also keep them behind older windows in the static
                # stream (free at runtime: same-engine program order)
                with tc.high_priority(offset=-(si * 800)):
                    emit_kv_gemms(g, ksb, sqg, jlo, jhi, act_stage=(si == 0))
                if si == 0:
                    # q side of qt0: overlaps the first k pair's stats chain
                    emit_q_side_group(0, _OpsPool, _TpsPool, act_stage=True)
                if pending is not None:
                    pg, _, _, pjlo, pjhi = pending
                    tp_a = emit_k_tail(pending)
                    s_a = emit_prefix_attn(pg * QC + pjlo, pg * QC + pjhi)
                    pe_anch[si - 1] = (tp_a, s_a)
                pending = emit_k_stats(g, ksb, sqg, jlo, jhi)
                if si == 0:
                    # no pipelining for the very first pair: its khat (and the
                    # first exp windows) are the critical path
                    tp_a = emit_k_tail(pending, act_stage=True)
                    s_a = emit_prefix_attn(0, 2)
                    pe_anch[0] = (tp_a, s_a)
                    pending = None
                if (g, jlo) == (3, 0):
                    with tc.high_priority(offset=-2400):
                        qs1 = emit_q_side_a(1, _OpsPool)
                if (g, jlo) == (4, 0):
                    with tc.high_priority(offset=-2400):
                        emit_q_side_b(qs1, _TpsPool)
            emit_k_tail(pending)
            emit_prefix_attn(SB - QC, SB)
            ztp0 = zqBp.tile([P, QC, P], BF16, name="ztp0", tag="zqB")
            emit_qt_finish(0, ((0, zqA), (1, zqB)), ztp0)

        # ---- steady state: q-tiles 1..7, ACT-saturated exp pipeline.
        # PSUM banks: 2 score slots x3 banks, z accumulator 1, ops 1 = 8
        with ExitStack() as p2:
            spool = p2.enter_context(tc.tile_pool(name="sps", bufs=2, space="PSUM"))
            zqpool = p2.enter_context(tc.tile_pool(name="zqps", bufs=1, space="PSUM"))

            for qt in range(1, QT):
                qsl = bass.ts(qt, QW)
                zn = znpool.tile([P, QC, P], BF16, name="zn", tag="zn")
                for h in range(HEADS_LOCAL):
                    zq = zqpool.tile([P, QC, D_HEAD + 1], F32, name="zq", tag="zq")
                    for kb0 in [0] + list(range(2, KB, EXP_BATCH)):
                        nb = 2 if kb0 == 0 else min(EXP_BATCH, KB - kb0)
                        sps = spool.tile(
                            [P, EXP_BATCH, QW], F32, name="sps", tag="sps"
                        )
                        probs = ppool.tile(
                            [P, EXP_BATCH, QW], BF16, name="probs", tag="probs"
                        )
                        for j in range(nb):
                            kb = kb0 + j
                            hsl = slice(D_HEAD * h, D_HEAD * (h + 1))
                            nc.tensor.matmul(
                                sps[:, j, :],
                                lhsT=khat_all[hsl, bass.ts(kb, P)],
                                rhs=qhat_all[hsl, qsl],
                                start=True,
                                stop=True,
                            )
                        nc.scalar.activation(
                            probs[:, 0:nb, :], sps[:, 0:nb, :], AF.Exp
                        )
                        # all 128 PV matmuls form ONE PSUM accumulation group
                        # (zq spans a single 2KB zero region)
                        for j in range(nb):
                            kb = kb0 + j
                            emit_pv(h, kb, zq, probs[:, j, :], last=(kb == KB - 1))
                        # software-pipelined work emitted under the exp shadow:
                        # h0: O-projection of the previous q-tile
                        # h1: q side (GEMM+norm+transposes) of the next q-tile
                        if kb0 == 2 and h == 0:
                            emit_oproj(qt - 1)
                        if kb0 == 2 and h == 1 and qt < QT - 1:
                            emit_q_side_group(qt + 1, _OpsPool, _OpsPool)
                    # normalize in q-major: z = z / rowsum (col 64)
                    rcp = rpool.tile([P, QC], F32, name="rcp", tag="rcp")
                    nc.vector.reciprocal(rcp[:], zq[:, :, D_HEAD])
                    nc.vector.tensor_tensor(
                        zn[:, :, bass.ts(h, D_HEAD)],
                        zq[:, :, 0:D_HEAD],
                        rcp[:, :, None].to_broadcast((P, QC, D_HEAD)),
                        ALU.mult,
                    )
                # transpose both heads at once into z_nT[d_local, s]
                ztp = zqpool.tile([P, QC, P], BF16, name="ztp", tag="zq")
                for qc in range(QC):
                    nc.tensor.transpose(ztp[:, qc, :], zn[:, qc, :], ident_b[:])
                    nc.vector.tensor_copy(
                        z_nT[:, qt * QW + qc * P : qt * QW + (qc + 1) * P],
                        ztp[:, qc, :],
                    )
            emit_oproj(QT - 1, spool, final=True)

    if split_waits:
        _split_excess_waits(nc)
    return nc


def shard_inputs(x, Wqkv, bqkv, Wo, bo, wq, wk):
    import ml_dtypes

    x2 = np.ascontiguousarray(np.asarray(x, dtype=np.float32).reshape(SEQ, D_MODEL))
    Wqkv = np.asarray(Wqkv, dtype=np.float32)
    bqkv = np.asarray(bqkv, dtype=np.float32)
    Wo = np.asarray(Wo, dtype=np.float32)
    wq = np.asarray(wq, dtype=np.float32)
    wk = np.asarray(wk, dtype=np.float32)

    xta = np.zeros((DM_AUG, SEQ), np.float32)
    xta[:D_MODEL] = x2.T
    xta[D_MODEL] = 1.0
    xtb = np.ascontiguousarray(xta.astype(ml_dtypes.bfloat16))

    # per-partition scale for khat_all's packed [d0|d1] feature rows
    wkc = np.ascontiguousarray(np.tile((wq * wk).reshape(D_HEAD), 2).reshape(P, 1))

    in_maps = []
    for c in range(N_CORES):
        rows, brows = [], []
        for part in range(3):
            for h in (HEADS_LOCAL * c, HEADS_LOCAL * c + 1):
                sl = slice(part * D_MODEL + h * D_HEAD, part * D_MODEL + (h + 1) * D_HEAD)
                rows.append(Wqkv[sl])
                brows.append(bqkv[sl])
        Wl = np.concatenate(rows, 0)          # [384, 1024] rows [q0|q1|k0|k1|v0|v1]
        bl = np.concatenate(brows, 0)         # [384]
        wqkvta = np.zeros((DM_AUG, 384), np.float32)
        wqkvta[:D_MODEL] = Wl.T
        wqkvta[D_MODEL] = bl
        wkvt = np.ascontiguousarray(
            wqkvta[:, Q_LOCAL:].astype(ml_dtypes.bfloat16)
        )                                                              # [1152, 256]
        wqt = np.ascontiguousarray(
            wqkvta[:, :Q_LOCAL].astype(ml_dtypes.bfloat16)
        )                                                              # [1152, 128]
        cols = slice(HEADS_LOCAL * c * D_HEAD, (HEADS_LOCAL * c + HEADS_LOCAL) * D_HEAD)
        wotc = np.ascontiguousarray(Wo[:, cols].T.astype(ml_dtypes.bfloat16))
        in_maps.append(
            {
                "xtb": xtb,
                "wkvt": wkvt,
                "wqt": wqt,
                "wot": wotc,
                "wkc": wkc,
            }
        )
    return in_maps


_NC_CACHE = {}
LAST_RESULT = None


def kernel(x, Wqkv, bqkv, Wo, bo, wq, wk):
    import os
    from concourse.bass_utils import run_bass_kernel_spmd

    global LAST_RESULT
    assert np.asarray(x).shape == (1, SEQ, D_MODEL)
    in_maps = shard_inputs(x, Wqkv, bqkv, Wo, bo, wq, wk)
    if "nc" not in _NC_CACHE:
        _NC_CACHE["nc"] = build_core_kernel()
    nc = _NC_CACHE["nc"]
    trace = bool(int(os.environ.get("BASS_KERNEL_TRACE", "0")))
    res = run_bass_kernel_spmd(nc, in_maps, list(range(N_CORES)), trace=trace)
    LAST_RESULT = res
    acc = np.zeros((SEQ, D_MODEL), np.float64)
    for c in range(N_CORES):
        acc += res.results[c]["out"].astype(np.float64)
    acc += np.asarray(bo, dtype=np.float64)
    return acc.astype(np.float32).reshape(1, SEQ, D_MODEL)



# revision 63
# speedup vs baseline: 1.0538x; 1.0048x over previous
"""Trainium2 Bass kernel for a 16-head attention block (d_model=1024, seq=4096).

Sharding: tensor-parallel over heads. Each of the 8 cores computes QKV
projections, RMSNorm(q,k), full softmax(QK^T)V attention for its 2 heads,
and a partial O-projection (its heads' slice of the contraction). The host
sums the 8 partial outputs (bf16 partials) and adds the output bias.

Per-core dataflow (k-first, attention is ACT/exp-bound so everything else
is arranged to hide under it). x ships ONCE as bf16 and stays resident in
SBUF (8 chunk tiles, 1KB descriptor runs) — the cost model serializes all
DMA on a shared 360GB/s device, so halving x traffic halves the prefix:
  phase 1 (serial prefix): k,v projections (bf16 GEMM, moving dim 256),
           RMSNorm(k) with wq*wk folded into the k side, PE-transpose
           k_hat into [64, s] fp32r tiles, V -> bf16 [k, 65] chunks with a
           fused ones column. The q side for the first q-tile is
           interleaved into the prefix tail (ACT Sqrt path).
  phase 2 (ACT-bound steady state): per (q-tile 512, head):
           S[k,q] blocks via khatT.T @ qhatT (K=64 fp32r), exp on ACT ->
           bf16 probs, z[q,65] += probs_chunk.T @ V' with probs as the
           stationary operand (65-column moving operand halves PE time).
           Row 64 = softmax denominator; normalize in q-major on DVE,
           PE-transpose both heads at once into z_nT[d_local, s] bf16.
           The NEXT q-tile's q side runs under the exp shadow: bf16
           x @ Wq GEMM (N=128 bf16 runs at 1 cyc/row; fp32r would be 4x),
           RMSNorm(q) with a Newton-iteration rsqrt on DVE (keeps the
           ACT table on Exp), PE-transposes into qhatT.
  phase 3: out[s,dm] partial = z_nT.T @ WoT (bf16), PSUM->SBUF bf16 -> HBM,
           software-pipelined one q-tile behind attention.
"""

import numpy as np
from contextlib import ExitStack

import concourse.bass as bass
import concourse.tile as tile
from concourse import mybir
from concourse.masks import make_identity

F32 = mybir.dt.float32
F32R = mybir.dt.float32r
BF16 = mybir.dt.bfloat16
I32 = mybir.dt.int32
AF = mybir.ActivationFunctionType
ALU = mybir.AluOpType

D_MODEL = 1024
SEQ = 4096
N_HEADS = 16
D_HEAD = 64
N_CORES = 8
HEADS_LOCAL = 2
P = 128
DM_AUG = D_MODEL + P                     # 1152 rows: x^T plus ones-row block
NCH = DM_AUG // P                        # 9 contraction chunks
KV_LOCAL = 2 * HEADS_LOCAL * D_HEAD      # 256: [k0|k1|v0|v1]
Q_LOCAL = HEADS_LOCAL * D_HEAD           # 128: [q0|q1]
SB = SEQ // P                            # 32 s-blocks
QT = 8                                   # q-tiles of 512
QW = SEQ // QT                           # 512
QC = QW // P                             # 4 q-chunks of 128 per q-tile
KB = SEQ // P                            # 32 k-blocks
EXP_BATCH = 3
DISABLE_PREFETCH = True
PREFIX_SLOT_MS = 0.0025
EPS = 1e-6
RSQRT_MAGIC = 0x5F3759DF


MAX_WAITS = 1


def _split_excess_waits(nc):
    """This walrus build rejects instructions carrying more than one or two
    sync-wait commands (CTRL and pseudo-DMA structs especially). Rewrite every
    instruction with more than MAX_WAITS waits into a chain of same-engine
    NoOps each carrying MAX_WAITS waits, followed by the original."""
    import bass_rust

    n_new = 0
    for f in nc.m.functions:
        for bb in f.blocks:
            changed = False
            out = []
            for ins in bb.instructions:
                si = ins.sync_info
                waits = list(si.on_wait) if si is not None and si.on_wait else []
                if len(waits) > MAX_WAITS:
                    changed = True
                    ncar = len(waits) - MAX_WAITS
                    for i in range(0, ncar, MAX_WAITS):
                        chunk = waits[i : min(i + MAX_WAITS, ncar)]
                        nop = mybir.InstNoOp(
                            name=f"{ins.name}-wsplit{i}", ins=[], outs=[]
                        )
                        nop.engine = ins.engine
                        nop.sync_info = bass_rust.SyncInfo(
                            on_wait=chunk, on_update=[]
                        )
                        out.append(nop)
                        n_new += 1
                    ins.sync_info = bass_rust.SyncInfo(
                        on_wait=waits[ncar:], on_update=si.on_update
                    )
                out.append(ins)
            if changed:
                bb.instructions = out
    return n_new


def build_core_kernel(split_waits=True):
    nc = bass.Bass()
    xtb = nc.declare_dram_parameter("xtb", [DM_AUG, SEQ], BF16, isOutput=False)
    wkvt = nc.declare_dram_parameter("wkvt", [DM_AUG, KV_LOCAL], BF16, isOutput=False)
    wqt = nc.declare_dram_parameter("wqt", [DM_AUG, Q_LOCAL], BF16, isOutput=False)
    wot = nc.declare_dram_parameter("wot", [P, D_MODEL], BF16, isOutput=False)
    wkc = nc.declare_dram_parameter("wkc", [P, 1], F32, isOutput=False)
    out = nc.declare_dram_parameter("out", [SEQ, D_MODEL], BF16, isOutput=True)

    xtb_r = xtb.rearrange("(c p) s -> p c s", p=P)       # [128, 9, 4096]
    wkvt_r = wkvt.rearrange("(c p) f -> p c f", p=P)     # [128, 9, 256]
    wqt_r = wqt.rearrange("(c p) f -> p c f", p=P)       # [128, 9, 128]

    with ExitStack() as ctx:
        tc = ctx.enter_context(tile.TileContext(nc))

        const = ctx.enter_context(tc.tile_pool(name="const", bufs=1))
        persist = ctx.enter_context(tc.tile_pool(name="persist", bufs=1))

        # DMA order matters: q side of qt0 runs first (xb0 + wq), then the
        # k side needs the full wkv
        xb0 = const.tile([P, NCH, QW], BF16, name="xb0")
        wkv_sb = const.tile([P, NCH, KV_LOCAL], BF16)
        wq_sb = const.tile([P, NCH, Q_LOCAL], BF16)
        nc.sync.dma_start(wkv_sb[:, 0:1, :], wkvt_r[:, 0:1, :])
        nc.sync.dma_start(xb0[:, :, 0:QW // 2], xtb_r[:, :, 0 : QW // 2])
        nc.sync.dma_start(wkv_sb[:, 1:NCH, :], wkvt_r[:, 1:NCH, :])
        nc.sync.dma_start(xb0[:, :, QW // 2 : QW], xtb_r[:, :, QW // 2 : QW])
        nc.sync.dma_start(wq_sb[:], wqt_r)
        ident_f = const.tile([P, P], F32)
        make_identity(nc, ident_f)
        ident_r = const.tile([P, P], F32R)
        nc.scalar.activation(ident_r[:], ident_f[:], AF.Copy)
        ident_b = const.tile([P, P], BF16)
        nc.vector.tensor_copy(ident_b[:], ident_f[:])
        wkc_sb = const.tile([P, 1], F32)
        nc.sync.dma_start(wkc_sb[:], wkc[:])
        wot_sb = const.tile([P, D_MODEL], BF16)
        # bias rows live in chunk 8 / partition 0 of the augmented weights;
        # broadcast them across partitions once (Pool) so the projections can
        # skip the 9th GEMM chunk and fold the bias into the staging TT-add
        ones1 = const.tile([1, P], BF16, name="ones1")
        nc.gpsimd.memset(ones1[:], 1.0)
        kvb_sb = const.tile([P, KV_LOCAL], F32)
        qb_sb = const.tile([P, Q_LOCAL], F32)

        # attention operands packed 2-heads-per-tile: khat_all/qhat_all
        # [128, s] fp32r with h0 in partitions 0-63, h1 in 64-127 (scores use
        # partition-offset matmuls, K=64). V' in [k, 65] bf16 per (head,
        # k-block) with a fused ones denominator column.
        qhat_all = persist.tile([P, SEQ], F32R, name="qhat_all")
        khat_all = persist.tile([P, SEQ], F32R, name="khat_all")
        vp = persist.tile([P, HEADS_LOCAL, KB, D_HEAD + 1], BF16)
        nc.gpsimd.memset(vp[:, :, :, D_HEAD : D_HEAD + 1], 1.0)
        z_nT = persist.tile([P, SEQ], BF16)

        # resident bf16 copy of x^T (augmented): 8 chunk tiles of 512 s-cols,
        # loaded once (1KB descriptor runs; serves both kv- and q-GEMMs).
        # wq comes right after xb0 so the first q-tile's q side can run at
        # the top of the prefix.
        xball = [xb0] + [
            persist.tile([P, NCH, QW], BF16, name=f"xb{d}") for d in range(1, QT)
        ]
        for d in range(1, QT):
            nc.sync.dma_start(xball[d][:], xtb_r[:, :, bass.ts(d, QW)])
        nc.sync.dma_start(wot_sb[:], wot[:])

        qnorm = ctx.enter_context(tc.tile_pool(name="qnorm", bufs=3))

        def emit_rsqrt_dve(pool, ss, n, tag=""):
            """rr = rsqrt(ss/64 + eps) via bit-trick seed + 2 Newton steps,
            all on DVE (keeps ACT free for exp). ss/rr: [P, n] fp32."""
            ms = pool.tile([P, n], F32, name="rms" + tag, tag="rms" + tag)
            nc.vector.tensor_scalar(
                ms[:], ss, 1.0 / D_HEAD, EPS, op0=ALU.mult, op1=ALU.add
            )
            xh = pool.tile([P, n], F32, name="rxh" + tag, tag="rxh" + tag)
            nc.vector.tensor_scalar(xh[:], ms[:], 0.5, None, op0=ALU.mult)
            iy = pool.tile([P, n], I32, name="riy" + tag, tag="riy" + tag)
            nc.vector.tensor_scalar(
                iy[:], ms[:].bitcast(I32), 1, None, op0=ALU.logical_shift_right
            )
            nc.vector.tensor_scalar(
                iy[:], iy[:], -1, RSQRT_MAGIC, op0=ALU.mult, op1=ALU.add
            )
            y = iy[:].bitcast(F32)
            for it in range(2):
                y2 = pool.tile([P, n], F32, name=f"ry2_{it}" + tag, tag=f"ry2_{it}" + tag)
                nc.vector.tensor_tensor(y2[:], y, y, ALU.mult)
                nc.vector.tensor_tensor(y2[:], y2[:], xh[:], ALU.mult)
                nc.vector.tensor_scalar(
                    y2[:], y2[:], -1.0, 1.5, op0=ALU.mult, op1=ALU.add
                )
                yn = pool.tile([P, n], F32, name=f"ryn_{it}" + tag, tag=f"ryn_{it}" + tag)
                nc.vector.tensor_tensor(yn[:], y, y2[:], ALU.mult)
                y = yn[:]
            return y

        def emit_q_side_a(qt, qpool, act_stage=False):
            """part A of the q side: GEMMs + staging + stats + rsqrt."""
            xbt = xball[qt]
            qsball = qnorm.tile([P, QC, Q_LOCAL], F32, name="qsb", tag="qsb")
            for j in range(QC):
                qps = qpool.tile([P, Q_LOCAL], F32, name="qps", tag=qpool.name_tag)
                nch = NCH if act_stage else NCH - 1
                for c in range(nch):
                    nc.tensor.matmul(
                        qps[:],
                        lhsT=xbt[:, c, bass.ts(j, P)],
                        rhs=wq_sb[:, c, :],
                        start=(c == 0),
                        stop=(c == nch - 1),
                    )
                if act_stage:
                    nc.scalar.activation(qsball[:, j, :], qps[:], AF.Copy)
                else:
                    nc.vector.tensor_tensor(
                        qsball[:, j, :], qps[:], qb_sb[:], ALU.add
                    )
            qg = qsball[:].rearrange("p j (g d) -> p j g d", g=2)
            sq = qnorm.tile([P, QC, 2, D_HEAD], F32, name="qsq", tag="qsq")
            nc.vector.tensor_tensor(sq[:], qg, qg, ALU.mult)
            ss = qnorm.tile([P, QC, 2], F32, name="qss", tag="qss")
            nc.vector.tensor_reduce(
                ss[:], sq[:], axis=mybir.AxisListType.X, op=ALU.add
            )
            y = emit_rsqrt_dve(
                qnorm, ss[:].rearrange("p j g -> p (j g)"), QC * 2, tag="q"
            )
            return (qt, qsball, y)

        def emit_q_side_b(part_a, tpool, act_stage=False):
            """part B: q_hat mult + packed transposes + qhat_all copies."""
            qt, qsball, y = part_a
            qg = qsball[:].rearrange("p j (g d) -> p j g d", g=2)
            yb = y.rearrange("p (j g) -> p j g", g=2)
            q_hat = qnorm.tile([P, QC, 2, D_HEAD], F32R, name="qhat", tag="qhat")
            nc.vector.tensor_tensor(
                q_hat[:],
                qg,
                yb[:, :, :, None].to_broadcast((P, QC, 2, D_HEAD)),
                ALU.mult,
            )
            for j in range(QC):
                ssl = bass.ts(qt * QC + j, P)
                pt = tpool.tile([P, P], F32R, name="qpt", tag=tpool.name_tag)
                nc.tensor.transpose(
                    pt[:], q_hat[:, j].rearrange("p g d -> p (g d)"), ident_r[:]
                )
                if act_stage:
                    nc.scalar.activation(qhat_all[:, ssl], pt[:], AF.Copy)
                else:
                    nc.vector.tensor_copy(qhat_all[:, ssl], pt[:])

        def emit_q_side_group(qt, qpool, tpool, act_stage=False):
            emit_q_side_b(
                emit_q_side_a(qt, qpool, act_stage), tpool, act_stage
            )

        def _unused_q_side(qt, qpool, tpool, act_stage=False):
            """bf16 x @ Wq for one q-tile (4 s-blocks) + RMSNorm(q) + PE
            transposes into qhatT. Stats batched across the 4 s-blocks so the
            DVE Newton-rsqrt chain runs once on [P, 8]. act_stage: route the
            PSUM->SBUF staging copies through ACT (only safe before the exp
            stream starts, when ACT is idle)."""
            xbt = xball[qt]
            qsball = qnorm.tile([P, QC, Q_LOCAL], F32, name="qsb", tag="qsb")
            for j in range(QC):
                qps = qpool.tile([P, Q_LOCAL], F32, name="qps", tag=qpool.name_tag)
                nch = NCH if act_stage else NCH - 1
                for c in range(nch):
                    nc.tensor.matmul(
                        qps[:],
                        lhsT=xbt[:, c, bass.ts(j, P)],
                        rhs=wq_sb[:, c, :],
                        start=(c == 0),
                        stop=(c == nch - 1),
                    )
                if act_stage:
                    nc.scalar.activation(qsball[:, j, :], qps[:], AF.Copy)
                else:
                    nc.vector.tensor_tensor(
                        qsball[:, j, :], qps[:], qb_sb[:], ALU.add
                    )
            qg = qsball[:].rearrange("p j (g d) -> p j g d", g=2)
            sq = qnorm.tile([P, QC, 2, D_HEAD], F32, name="qsq", tag="qsq")
            nc.vector.tensor_tensor(sq[:], qg, qg, ALU.mult)
            ss = qnorm.tile([P, QC, 2], F32, name="qss", tag="qss")
            nc.vector.tensor_reduce(
                ss[:], sq[:], axis=mybir.AxisListType.X, op=ALU.add
            )
            y = emit_rsqrt_dve(
                qnorm, ss[:].rearrange("p j g -> p (j g)"), QC * 2, tag="q"
            )
            yb = y.rearrange("p (j g) -> p j g", g=2)
            q_hat = qnorm.tile([P, QC, 2, D_HEAD], F32R, name="qhat", tag="qhat")
            nc.vector.tensor_tensor(
                q_hat[:],
                qg,
                yb[:, :, :, None].to_broadcast((P, QC, 2, D_HEAD)),
                ALU.mult,
            )
            for j in range(QC):
                ssl = bass.ts(qt * QC + j, P)
                pt = tpool.tile([P, P], F32R, name="qpt", tag=tpool.name_tag)
                nc.tensor.transpose(
                    pt[:], q_hat[:, j].rearrange("p g d -> p (g d)"), ident_r[:]
                )
                if act_stage:
                    nc.scalar.activation(qhat_all[:, ssl], pt[:], AF.Copy)
                else:
                    nc.vector.tensor_copy(qhat_all[:, ssl], pt[:])

        # shared pools alive for the whole kernel
        opool = ctx.enter_context(tc.tile_pool(name="ops", bufs=1, space="PSUM"))
        ppool = ctx.enter_context(tc.tile_pool(name="probs", bufs=4))
        znpool = ctx.enter_context(tc.tile_pool(name="zn", bufs=3))
        rpool = ctx.enter_context(tc.tile_pool(name="rcp", bufs=3))
        osb = ctx.enter_context(tc.tile_pool(name="osb", bufs=4))
        norm = ctx.enter_context(tc.tile_pool(name="norm", bufs=3))

        class _OpsPool:
            name_tag = "ops"

            @staticmethod
            def tile(shape, dt, name=None, tag=None):
                return opool.tile(shape, dt, name=name, tag="ops")

        def emit_score_exp(h, kb, sps, probs, qsl):
            """S matmul (partition-offset by head) + ACT exp -> bf16 probs.
            sps/probs: [P, QW] APs. Returns the exp instruction."""
            hsl = slice(D_HEAD * h, D_HEAD * (h + 1))
            smm = nc.tensor.matmul(
                sps,
                lhsT=khat_all[hsl, bass.ts(kb, P)],
                rhs=qhat_all[hsl, qsl],
                start=True,
                stop=True,
            )
            return smm, nc.scalar.activation(probs, sps, AF.Exp)

        def emit_pv(h, kb, zq, probs, last):
            """4 PV matmuls accumulating [P, QC, 65] into zq (one PSUM
            zero-region per (qt, h))."""
            for qc in range(QC):
                nc.tensor.matmul(
                    zq[:, qc, :],
                    lhsT=probs[:, bass.ts(qc, P)],
                    rhs=vp[:, h, kb, :],
                    start=(kb == 0 and qc == 0),
                    stop=(last and qc == QC - 1),
                    skip_group_check=True,
                )

        def emit_qt_finish(qt, h_zq_pairs, ztp):
            """normalize z (q-major), pack both heads, PE-transpose into
            z_nT[d_local, s]."""
            zn = znpool.tile([P, QC, P], BF16, name="zn", tag="zn")
            for h, zq in h_zq_pairs:
                rcp = rpool.tile([P, QC], F32, name="rcp", tag="rcp")
                nc.vector.reciprocal(rcp[:], zq[:, :, D_HEAD])
                nc.vector.tensor_tensor(
                    zn[:, :, bass.ts(h, D_HEAD)],
                    zq[:, :, 0:D_HEAD],
                    rcp[:, :, None].to_broadcast((P, QC, D_HEAD)),
                    ALU.mult,
                )
            for qc in range(QC):
                nc.tensor.transpose(ztp[:, qc, :], zn[:, qc, :], ident_b[:])
                nc.vector.tensor_copy(
                    z_nT[:, qt * QW + qc * P : qt * QW + (qc + 1) * P],
                    ztp[:, qc, :],
                )

        def emit_oproj(qt, spool=None, final=False):
            # final q-tile: S slots are free, so pipeline the matmuls
            # 2-wide through them and put half the copies on the idle ACT
            for sbl in range(QC):
                sb = qt * QC + sbl
                ot = osb.tile([P, D_MODEL], BF16, name="ot", tag="ot")
                for half in range(2):
                    if final:
                        ops = spool.tile([P, QW], F32, name="ops", tag="sps")
                    else:
                        ops = opool.tile([P, QW], F32, name="ops", tag="ops")
                    nc.tensor.matmul(
                        ops[:],
                        lhsT=z_nT[:, bass.ts(sb, P)],
                        rhs=wot_sb[:, bass.ts(half, QW)],
                        start=True,
                        stop=True,
                    )
                    if final and half == 0:
                        nc.scalar.activation(
                            ot[:, bass.ts(half, QW)], ops[:], AF.Copy
                        )
                    elif final:
                        nc.vector.tensor_copy(ot[:, bass.ts(half, QW)], ops[:])
                    else:
                        nc.vector.tensor_copy(ot[:, bass.ts(half, QW)], ops[:])
                    if final:
                        nc.sync.dma_start(
                            out[bass.ts(sb, P), bass.ts(half, QW)],
                            ot[:, bass.ts(half, QW)],
                        )
                if not final:
                    nc.sync.dma_start(out[bass.ts(sb, P), :], ot[:])

        # ---- streaming prefix: K/V projections with qt0's attention (both
        # heads) interleaved so ACT starts exp within a few us of launch.
        # PSUM banks: kvps 2 + tps 1 + sps1 2 + zqA 1 + zqB 1 + ops 1 = 8
        with ExitStack() as p1:
            qkps = p1.enter_context(tc.tile_pool(name="kvps", bufs=2, space="PSUM"))
            tps = p1.enter_context(tc.tile_pool(name="tps", bufs=1, space="PSUM"))
            sps1 = p1.enter_context(tc.tile_pool(name="sps1", bufs=2, space="PSUM"))
            zqAp = p1.enter_context(tc.tile_pool(name="zqA", bufs=1, space="PSUM"))
            zqBp = p1.enter_context(tc.tile_pool(name="zqB", bufs=1, space="PSUM"))

            class _TpsPool:
                name_tag = "tps"

                @staticmethod
                def tile(shape, dt, name=None, tag=None):
                    return tps.tile(shape, dt, name=name, tag="tps")

            def emit_k_tail(pend, act_stage=False):
                """k_hat mult + packed PE transposes + khat_all copies for a
                finished segment (software-pipelined one segment behind)."""
                g, ksb, rr, jlo, jhi = pend
                n = jhi - jlo
                kg = ksb[:, jlo:jhi, 0 : 2 * D_HEAD].rearrange(
                    "p j (g d) -> p j g d", g=2
                )
                k_hat = norm.tile([P, n, 2, D_HEAD], F32R, name="khat", tag="khat")
                nc.vector.tensor_tensor(
                    k_hat[:],
                    kg,
                    rr.rearrange("p (j g) -> p j g", g=2)[
                        :, :, :, None
                    ].to_broadcast((P, n, 2, D_HEAD)),
                    ALU.mult,
                )
                last_tp = None
                for j in range(n):
                    ssl = bass.ts(g * QC + jlo + j, P)
                    pt = tps.tile([P, P], F32R, name="pt", tag="tps")
                    last_tp = nc.tensor.transpose(
                        pt[:], k_hat[:, j].rearrange("p g d -> p (g d)"), ident_r[:]
                    )
                    if act_stage:
                        nc.scalar.activation(
                            khat_all[:, ssl], pt[:], AF.Copy, scale=wkc_sb[:]
                        )
                    else:
                        nc.vector.tensor_scalar_mul(khat_all[:, ssl], pt[:], wkc_sb[:])
                return last_tp

            zqA = zqAp.tile([P, QC, D_HEAD + 1], F32, name="zqA")
            zqB = zqBp.tile([P, QC, D_HEAD + 1], F32, name="zqB")
            qsl0 = bass.ts(0, QW)

            def emit_prefix_attn(kblo, kbhi):
                """qt0 attention windows for k-blocks [kblo, kbhi), both
                heads, EB=1. Returns the last exp instruction (used as a
                scheduler ordering anchor)."""
                anchor = None
                for h, zq in ((0, zqA), (1, zqB)):
                    for kb in range(kblo, kbhi):
                        sps = sps1.tile([P, QW], F32, name="sps1", tag="sps1")
                        probs = ppool.tile([P, QW], BF16, name="probs1", tag="probs1")
                        anchor, _ = emit_score_exp(h, kb, sps[:], probs[:], qsl0)
                        emit_pv(h, kb, zq, probs[:], last=(kb == KB - 1))
                return anchor

            def emit_kv_gemms(g, ksb, sqg, jlo, jhi, act_stage=False,
                              pe_anchors=()):
                import bass_rust

                xbt = xball[g]
                for j in range(jlo, jhi):
                    sb = g * QC + j
                    kv_ps = qkps.tile([P, KV_LOCAL], F32, name="kv_ps", tag="kvps")
                    nch = NCH if act_stage else NCH - 1
                    for c in range(nch):
                        mm = nc.tensor.matmul(
                            kv_ps[:],
                            lhsT=xbt[:, c, bass.ts(j, P)],
                            rhs=wkv_sb[:, c, :],
                            start=(c == 0),
                            stop=(c == nch - 1),
                        )
                        if pe_anchors:
                            # PE->PE ordering-only deps: keep far-ahead GEMMs
                            # behind older attention windows in the static PE
                            # stream (free at runtime: same-engine order)
                            dset = bass_rust.InstructionNameOrderedSet()
                            for a in pe_anchors:
                                if a is not None:
                                    dset.add(a.ins.name)
                            mm.ins.add_nosync_dependencies_from(dset)
                            pe_anchors = ()
                    # one combined K|V staging op (PSUM->SBUF) that also adds
                    # the qkv bias (saves the 9th GEMM chunk); V' is then
                    # extracted SBUF->SBUF on the otherwise-idle Pool engine.
                    # act_stage (head of pipeline): bias came from the aug-row
                    # chunk instead, so the idle ACT can do a plain copy
                    if act_stage:
                        nc.scalar.activation(ksb[:, j, :], kv_ps[:], AF.Copy)
                    else:
                        nc.vector.tensor_tensor(
                            ksb[:, j, :], kv_ps[:], kvb_sb[:], ALU.add
                        )
                    nc.gpsimd.tensor_copy(
                        vp[:, :, sb, 0:D_HEAD],
                        ksb[:, j, 2 * D_HEAD : 4 * D_HEAD].rearrange(
                            "p (h d) -> p h d", h=2
                        ),
                    )
                    kgj = ksb[:, j, 0 : 2 * D_HEAD].rearrange("p (g d) -> p g d", g=2)
                    nc.vector.tensor_tensor(sqg[:, j], kgj, kgj, ALU.mult)

            def emit_k_stats(g, ksb, sqg, jlo, jhi):
                n = jhi - jlo
                ssg = norm.tile([P, n, 2], F32, name="ssg", tag="ssg")
                nc.vector.tensor_reduce(
                    ssg[:], sqg[:, jlo:jhi], axis=mybir.AxisListType.X, op=ALU.add
                )
                rr = emit_rsqrt_dve(
                    norm, ssg[:].rearrange("p j g -> p (j g)"), n * 2, tag="k"
                )
                return (g, ksb, rr, jlo, jhi)

            # group 0 is split into two pairs so the first attention windows
            # (and with them ACT's exp stream) start as early as possible
            # broadcast the bias rows (chunk 8, partition 0) across all
            # partitions via a K=1 ones-matmul; PE is idle this early
            for bias_sb, w_sb, ncols in (
                (kvb_sb, wkv_sb, KV_LOCAL),
                (qb_sb, wq_sb, Q_LOCAL),
            ):
                bps = opool.tile([P, ncols], F32, name="bps", tag="ops")
                nc.tensor.matmul(
                    bps[:],
                    lhsT=ones1[:],
                    rhs=w_sb[0:1, NCH - 1, :],
                    start=True,
                    stop=True,
                )
                nc.vector.tensor_copy(bias_sb[:], bps[:])

            segs = [(0, 0, 2), (0, 2, 4)] + [(g, 0, QC) for g in range(1, SB // QC)]
            ktiles = {}
            pending = None
            pe_anch = {}
            for si, (g, jlo, jhi) in enumerate(segs):
                if jlo == 0:
                    ktiles[g] = (
                        norm.tile([P, QC, KV_LOCAL], F32, name="ksb", tag="ksb"),
                        norm.tile([P, QC, 2, D_HEAD], F32, name="sqg", tag="sqg"),
                    )
                ksb, sqg = ktiles[g]
                # de-prioritize far-ahead kv GEMMs for the tile scheduler so
                # ready attention windows always win the PE; PE->PE nosync
                # anchors also keep them behind older windows in the static
                # stream (free at runtime: same-engine program order)
                with tc.high_priority(offset=-(si * 800)):
                    emit_kv_gemms(g, ksb, sqg, jlo, jhi, act_stage=(si == 0))
                if si == 0:
                    # q side of qt0: overlaps the first k pair's stats chain
                    emit_q_side_group(0, _OpsPool, _TpsPool, act_stage=True)
                if pending is not None:
                    pg, _, _, pjlo, pjhi = pending
                    tp_a = emit_k_tail(pending)
                    s_a = emit_prefix_attn(pg * QC + pjlo, pg * QC + pjhi)
                    pe_anch[si - 1] = (tp_a, s_a)
                pending = emit_k_stats(g, ksb, sqg, jlo, jhi)
                if si == 0:
                    # no pipelining for the very first pair: its khat (and the
                    # first exp windows) are the critical path
                    tp_a = emit_k_tail(pending, act_stage=True)
                    s_a = emit_prefix_attn(0, 2)
                    pe_anch[0] = (tp_a, s_a)
                    pending = None
                if (g, jlo) == (3, 0):
                    with tc.high_priority(offset=-2400):
                        qs1 = emit_q_side_a(1, _OpsPool)
                if (g, jlo) == (4, 0):
                    with tc.high_priority(offset=-2400):
                        emit_q_side_b(qs1, _TpsPool)
            emit_k_tail(pending)
            emit_prefix_attn(SB - QC, SB)
            ztp0 = zqBp.tile([P, QC, P], BF16, name="ztp0", tag="zqB")
            emit_qt_finish(0, ((0, zqA), (1, zqB)), ztp0)

        # ---- steady state: q-tiles 1..7, ACT-saturated exp pipeline.
        # PSUM banks: 2 score slots x3 banks, z accumulator 1, ops 1 = 8
        with ExitStack() as p2:
            spool = p2.enter_context(tc.tile_pool(name="sps", bufs=2, space="PSUM"))
            zqpool = p2.enter_context(tc.tile_pool(name="zqps", bufs=1, space="PSUM"))

            def emit_sexp(qt, h, kb0, nb):
                """S matmuls + exp for one steady batch; returns the probs."""
                qsl = bass.ts(qt, QW)
                sps = spool.tile([P, EXP_BATCH, QW], F32, name="sps", tag="sps")
                probs = ppool.tile(
                    [P, EXP_BATCH, QW], BF16, name="probs", tag="probs"
                )
                hsl = slice(D_HEAD * h, D_HEAD * (h + 1))
                for j in range(nb):
                    kb = kb0 + j
                    nc.tensor.matmul(
                        sps[:, j, :],
                        lhsT=khat_all[hsl, bass.ts(kb, P)],
                        rhs=qhat_all[hsl, qsl],
                        start=True,
                        stop=True,
                    )
                nc.scalar.activation(probs[:, 0:nb, :], sps[:, 0:nb, :], AF.Exp)
                return probs

            # (qt, h) tile-head stream with the next head's first S+exp batch
            # prefetched before the current head's normalize, so ACT never
            # waits on the zq bank turnaround at head/tile boundaries
            heads = [(qt, h) for qt in range(1, QT) for h in range(HEADS_LOCAL)]
            prefetch = None
            zn = None
            for qt, h in heads:
                if h == 0:
                    zn = znpool.tile([P, QC, P], BF16, name="zn", tag="zn")
                zq = zqpool.tile([P, QC, D_HEAD + 1], F32, name="zq", tag="zq")
                for kb0 in [0] + list(range(2, KB, EXP_BATCH)):
                    nb = 2 if kb0 == 0 else min(EXP_BATCH, KB - kb0)
                    if kb0 == 0 and prefetch is not None:
                        probs = prefetch
                        prefetch = None
                    else:
                        probs = emit_sexp(qt, h, kb0, nb)
                    # all 128 PV matmuls form ONE PSUM accumulation group
                    # (zq spans a single 2KB zero region)
                    for j in range(nb):
                        kb = kb0 + j
                        emit_pv(h, kb, zq, probs[:, j, :], last=(kb == KB - 1))
                    # software-pipelined work emitted under the exp shadow:
                    # h0: O-projection of the previous q-tile
                    # h1: q side (GEMM+norm+transposes) of the next q-tile
                    if kb0 == 2 and h == 0:
                        emit_oproj(qt - 1)
                    if kb0 == 2 and h == 1 and qt < QT - 1:
                        emit_q_side_group(qt + 1, _OpsPool, _OpsPool)
                # prefetch the next tile-head's first batch (its qhat/khat are
                # ready well before this point)
                if (qt, h) != heads[-1] and not DISABLE_PREFETCH:
                    nqt, nh = heads[heads.index((qt, h)) + 1]
                    prefetch = emit_sexp(nqt, nh, 0, 2)
                # normalize in q-major: z = z / rowsum (col 64)
                rcp = rpool.tile([P, QC], F32, name="rcp", tag="rcp")
                nc.vector.reciprocal(rcp[:], zq[:, :, D_HEAD])
                nc.vector.tensor_tensor(
                    zn[:, :, bass.ts(h, D_HEAD)],
                    zq[:, :, 0:D_HEAD],
                    rcp[:, :, None].to_broadcast((P, QC, D_HEAD)),
                    ALU.mult,
                )
                if h == 1:
                    # transpose both heads at once into z_nT[d_local, s]
                    ztp = zqpool.tile([P, QC, P], BF16, name="ztp", tag="zq")
                    for qc in range(QC):
                        nc.tensor.transpose(ztp[:, qc, :], zn[:, qc, :], ident_b[:])
                        nc.vector.tensor_copy(
                            z_nT[:, qt * QW + qc * P : qt * QW + (qc + 1) * P],
                            ztp[:, qc, :],
                        )
            emit_oproj(QT - 1, spool, final=True)

    if split_waits:
        _split_excess_waits(nc)
    return nc


def shard_inputs(x, Wqkv, bqkv, Wo, bo, wq, wk):
    import ml_dtypes

    x2 = np.ascontiguousarray(np.asarray(x, dtype=np.float32).reshape(SEQ, D_MODEL))
    Wqkv = np.asarray(Wqkv, dtype=np.float32)
    bqkv = np.asarray(bqkv, dtype=np.float32)
    Wo = np.asarray(Wo, dtype=np.float32)
    wq = np.asarray(wq, dtype=np.float32)
    wk = np.asarray(wk, dtype=np.float32)

    xta = np.zeros((DM_AUG, SEQ), np.float32)
    xta[:D_MODEL] = x2.T
    xta[D_MODEL] = 1.0
    xtb = np.ascontiguousarray(xta.astype(ml_dtypes.bfloat16))

    # per-partition scale for khat_all's packed [d0|d1] feature rows
    wkc = np.ascontiguousarray(np.tile((wq * wk).reshape(D_HEAD), 2).reshape(P, 1))

    in_maps = []
    for c in range(N_CORES):
        rows, brows = [], []
        for part in range(3):
            for h in (HEADS_LOCAL * c, HEADS_LOCAL * c + 1):
                sl = slice(part * D_MODEL + h * D_HEAD, part * D_MODEL + (h + 1) * D_HEAD)
                rows.append(Wqkv[sl])
                brows.append(bqkv[sl])
        Wl = np.concatenate(rows, 0)          # [384, 1024] rows [q0|q1|k0|k1|v0|v1]
        bl = np.concatenate(brows, 0)         # [384]
        wqkvta = np.zeros((DM_AUG, 384), np.float32)
        wqkvta[:D_MODEL] = Wl.T
        wqkvta[D_MODEL] = bl
        wkvt = np.ascontiguousarray(
            wqkvta[:, Q_LOCAL:].astype(ml_dtypes.bfloat16)
        )                                                              # [1152, 256]
        wqt = np.ascontiguousarray(
            wqkvta[:, :Q_LOCAL].astype(ml_dtypes.bfloat16)
        )                                                              # [1152, 128]
        cols = slice(HEADS_LOCAL * c * D_HEAD, (HEADS_LOCAL * c + HEADS_LOCAL) * D_HEAD)
        wotc = np.ascontiguousarray(Wo[:, cols].T.astype(ml_dtypes.bfloat16))
        in_maps.append(
            {
                "xtb": xtb,
                "wkvt": wkvt,
                "wqt": wqt,
                "wot": wotc,
                "wkc": wkc,
            }
        )
    return in_maps


_NC_CACHE = {}
LAST_RESULT = None


def kernel(x, Wqkv, bqkv, Wo, bo, wq, wk):
    import os
    from concourse.bass_utils import run_bass_kernel_spmd

    global LAST_RESULT
    assert np.asarray(x).shape == (1, SEQ, D_MODEL)
    in_maps = shard_inputs(x, Wqkv, bqkv, Wo, bo, wq, wk)
    if "nc" not in _NC_CACHE:
        _NC_CACHE["nc"] = build_core_kernel()
    nc = _NC_CACHE["nc"]
    trace = bool(int(os.environ.get("BASS_KERNEL_TRACE", "0")))
    res = run_bass_kernel_spmd(nc, in_maps, list(range(N_CORES)), trace=trace)
    LAST_RESULT = res
    acc = np.zeros((SEQ, D_MODEL), np.float64)
    for c in range(N_CORES):
        acc += res.results[c]["out"].astype(np.float64)
    acc += np.asarray(bo, dtype=np.float64)
    return acc.astype(np.float32).reshape(1, SEQ, D_MODEL)



# revision 64
# speedup vs baseline: 1.0543x; 1.0005x over previous
"""Trainium2 Bass kernel for a 16-head attention block (d_model=1024, seq=4096).

Sharding: tensor-parallel over heads. Each of the 8 cores computes QKV
projections, RMSNorm(q,k), full softmax(QK^T)V attention for its 2 heads,
and a partial O-projection (its heads' slice of the contraction). The host
sums the 8 partial outputs (bf16 partials) and adds the output bias.

Per-core dataflow (k-first, attention is ACT/exp-bound so everything else
is arranged to hide under it). x ships ONCE as bf16 and stays resident in
SBUF (8 chunk tiles, 1KB descriptor runs) — the cost model serializes all
DMA on a shared 360GB/s device, so halving x traffic halves the prefix:
  phase 1 (serial prefix): k,v projections (bf16 GEMM, moving dim 256),
           RMSNorm(k) with wq*wk folded into the k side, PE-transpose
           k_hat into [64, s] fp32r tiles, V -> bf16 [k, 65] chunks with a
           fused ones column. The q side for the first q-tile is
           interleaved into the prefix tail (ACT Sqrt path).
  phase 2 (ACT-bound steady state): per (q-tile 512, head):
           S[k,q] blocks via khatT.T @ qhatT (K=64 fp32r), exp on ACT ->
           bf16 probs, z[q,65] += probs_chunk.T @ V' with probs as the
           stationary operand (65-column moving operand halves PE time).
           Row 64 = softmax denominator; normalize in q-major on DVE,
           PE-transpose both heads at once into z_nT[d_local, s] bf16.
           The NEXT q-tile's q side runs under the exp shadow: bf16
           x @ Wq GEMM (N=128 bf16 runs at 1 cyc/row; fp32r would be 4x),
           RMSNorm(q) with a Newton-iteration rsqrt on DVE (keeps the
           ACT table on Exp), PE-transposes into qhatT.
  phase 3: out[s,dm] partial = z_nT.T @ WoT (bf16), PSUM->SBUF bf16 -> HBM,
           software-pipelined one q-tile behind attention.
"""

import numpy as np
from contextlib import ExitStack

import concourse.bass as bass
import concourse.tile as tile
from concourse import mybir
from concourse.masks import make_identity

F32 = mybir.dt.float32
F32R = mybir.dt.float32r
BF16 = mybir.dt.bfloat16
I32 = mybir.dt.int32
AF = mybir.ActivationFunctionType
ALU = mybir.AluOpType

D_MODEL = 1024
SEQ = 4096
N_HEADS = 16
D_HEAD = 64
N_CORES = 8
HEADS_LOCAL = 2
P = 128
DM_AUG = D_MODEL + P                     # 1152 rows: x^T plus ones-row block
NCH = DM_AUG // P                        # 9 contraction chunks
KV_LOCAL = 2 * HEADS_LOCAL * D_HEAD      # 256: [k0|k1|v0|v1]
Q_LOCAL = HEADS_LOCAL * D_HEAD           # 128: [q0|q1]
SB = SEQ // P                            # 32 s-blocks
QT = 8                                   # q-tiles of 512
QW = SEQ // QT                           # 512
QC = QW // P                             # 4 q-chunks of 128 per q-tile
KB = SEQ // P                            # 32 k-blocks
EXP_BATCH = 3
DISABLE_PREFETCH = True
PREFIX_SLOT_MS = 0.0025
EPS = 1e-6
RSQRT_MAGIC = 0x5F3759DF


MAX_WAITS = 1


def _split_excess_waits(nc):
    """This walrus build rejects instructions carrying more than one or two
    sync-wait commands (CTRL and pseudo-DMA structs especially). Rewrite every
    instruction with more than MAX_WAITS waits into a chain of same-engine
    NoOps each carrying MAX_WAITS waits, followed by the original."""
    import bass_rust

    n_new = 0
    for f in nc.m.functions:
        for bb in f.blocks:
            changed = False
            out = []
            for ins in bb.instructions:
                si = ins.sync_info
                waits = list(si.on_wait) if si is not None and si.on_wait else []
                if len(waits) > MAX_WAITS:
                    changed = True
                    ncar = len(waits) - MAX_WAITS
                    for i in range(0, ncar, MAX_WAITS):
                        chunk = waits[i : min(i + MAX_WAITS, ncar)]
                        nop = mybir.InstNoOp(
                            name=f"{ins.name}-wsplit{i}", ins=[], outs=[]
                        )
                        nop.engine = ins.engine
                        nop.sync_info = bass_rust.SyncInfo(
                            on_wait=chunk, on_update=[]
                        )
                        out.append(nop)
                        n_new += 1
                    ins.sync_info = bass_rust.SyncInfo(
                        on_wait=waits[ncar:], on_update=si.on_update
                    )
                out.append(ins)
            if changed:
                bb.instructions = out
    return n_new


def build_core_kernel(split_waits=True):
    nc = bass.Bass()
    xtb = nc.declare_dram_parameter("xtb", [DM_AUG, SEQ], BF16, isOutput=False)
    wkvt = nc.declare_dram_parameter("wkvt", [DM_AUG, KV_LOCAL], BF16, isOutput=False)
    wqt = nc.declare_dram_parameter("wqt", [DM_AUG, Q_LOCAL], BF16, isOutput=False)
    wot = nc.declare_dram_parameter("wot", [P, D_MODEL], BF16, isOutput=False)
    wkc = nc.declare_dram_parameter("wkc", [P, 1], F32, isOutput=False)
    out = nc.declare_dram_parameter("out", [SEQ, D_MODEL], BF16, isOutput=True)

    xtb_r = xtb.rearrange("(c p) s -> p c s", p=P)       # [128, 9, 4096]
    wkvt_r = wkvt.rearrange("(c p) f -> p c f", p=P)     # [128, 9, 256]
    wqt_r = wqt.rearrange("(c p) f -> p c f", p=P)       # [128, 9, 128]

    with ExitStack() as ctx:
        tc = ctx.enter_context(tile.TileContext(nc))

        const = ctx.enter_context(tc.tile_pool(name="const", bufs=1))
        persist = ctx.enter_context(tc.tile_pool(name="persist", bufs=1))

        # DMA order matters: q side of qt0 runs first (xb0 + wq), then the
        # k side needs the full wkv
        xb0 = const.tile([P, NCH, QW], BF16, name="xb0")
        wkv_sb = const.tile([P, NCH, KV_LOCAL], BF16)
        wq_sb = const.tile([P, NCH, Q_LOCAL], BF16)
        nc.sync.dma_start(wkv_sb[:], wkvt_r)
        nc.sync.dma_start(xb0[:, :, 0:QW // 2], xtb_r[:, :, 0 : QW // 2])
        nc.sync.dma_start(xb0[:, :, QW // 2 : QW], xtb_r[:, :, QW // 2 : QW])
        nc.sync.dma_start(wq_sb[:], wqt_r)
        ident_f = const.tile([P, P], F32)
        make_identity(nc, ident_f)
        ident_r = const.tile([P, P], F32R)
        nc.scalar.activation(ident_r[:], ident_f[:], AF.Copy)
        ident_b = const.tile([P, P], BF16)
        nc.vector.tensor_copy(ident_b[:], ident_f[:])
        wkc_sb = const.tile([P, 1], F32)
        nc.sync.dma_start(wkc_sb[:], wkc[:])
        wot_sb = const.tile([P, D_MODEL], BF16)
        # bias rows live in chunk 8 / partition 0 of the augmented weights;
        # broadcast them across partitions once (Pool) so the projections can
        # skip the 9th GEMM chunk and fold the bias into the staging TT-add
        ones1 = const.tile([1, P], BF16, name="ones1")
        nc.gpsimd.memset(ones1[:], 1.0)
        kvb_sb = const.tile([P, KV_LOCAL], F32)
        qb_sb = const.tile([P, Q_LOCAL], F32)

        # attention operands packed 2-heads-per-tile: khat_all/qhat_all
        # [128, s] fp32r with h0 in partitions 0-63, h1 in 64-127 (scores use
        # partition-offset matmuls, K=64). V' in [k, 65] bf16 per (head,
        # k-block) with a fused ones denominator column.
        qhat_all = persist.tile([P, SEQ], F32R, name="qhat_all")
        khat_all = persist.tile([P, SEQ], F32R, name="khat_all")
        vp = persist.tile([P, HEADS_LOCAL, KB, D_HEAD + 1], BF16)
        nc.gpsimd.memset(vp[:, :, :, D_HEAD : D_HEAD + 1], 1.0)
        z_nT = persist.tile([P, SEQ], BF16)

        # resident bf16 copy of x^T (augmented): 8 chunk tiles of 512 s-cols,
        # loaded once (1KB descriptor runs; serves both kv- and q-GEMMs).
        # wq comes right after xb0 so the first q-tile's q side can run at
        # the top of the prefix.
        xball = [xb0] + [
            persist.tile([P, NCH, QW], BF16, name=f"xb{d}") for d in range(1, QT)
        ]
        for d in range(1, QT):
            nc.sync.dma_start(xball[d][:], xtb_r[:, :, bass.ts(d, QW)])
        nc.sync.dma_start(wot_sb[:], wot[:])

        qnorm = ctx.enter_context(tc.tile_pool(name="qnorm", bufs=3))

        def emit_rsqrt_dve(pool, ss, n, tag=""):
            """rr = rsqrt(ss/64 + eps) via bit-trick seed + 2 Newton steps,
            all on DVE (keeps ACT free for exp). ss/rr: [P, n] fp32."""
            ms = pool.tile([P, n], F32, name="rms" + tag, tag="rms" + tag)
            nc.vector.tensor_scalar(
                ms[:], ss, 1.0 / D_HEAD, EPS, op0=ALU.mult, op1=ALU.add
            )
            iy = pool.tile([P, n], I32, name="riy" + tag, tag="riy" + tag)
            nc.vector.tensor_scalar(
                iy[:], ms[:].bitcast(I32), 1, None, op0=ALU.logical_shift_right
            )
            nc.vector.tensor_scalar(
                iy[:], iy[:], -1, RSQRT_MAGIC, op0=ALU.mult, op1=ALU.add
            )
            y = iy[:].bitcast(F32)
            for it in range(2):
                y2 = pool.tile([P, n], F32, name=f"ry2_{it}" + tag, tag=f"ry2_{it}" + tag)
                nc.vector.tensor_tensor(y2[:], y, y, ALU.mult)
                nc.vector.tensor_tensor(y2[:], y2[:], ms[:], ALU.mult)
                nc.vector.tensor_scalar(
                    y2[:], y2[:], -0.5, 1.5, op0=ALU.mult, op1=ALU.add
                )
                yn = pool.tile([P, n], F32, name=f"ryn_{it}" + tag, tag=f"ryn_{it}" + tag)
                nc.vector.tensor_tensor(yn[:], y, y2[:], ALU.mult)
                y = yn[:]
            return y

        def emit_q_side_a(qt, qpool, act_stage=False):
            """part A of the q side: GEMMs + staging + stats + rsqrt."""
            xbt = xball[qt]
            qsball = qnorm.tile([P, QC, Q_LOCAL], F32, name="qsb", tag="qsb")
            for j in range(QC):
                qps = qpool.tile([P, Q_LOCAL], F32, name="qps", tag=qpool.name_tag)
                nch = NCH if act_stage else NCH - 1
                for c in range(nch):
                    nc.tensor.matmul(
                        qps[:],
                        lhsT=xbt[:, c, bass.ts(j, P)],
                        rhs=wq_sb[:, c, :],
                        start=(c == 0),
                        stop=(c == nch - 1),
                    )
                if act_stage:
                    nc.scalar.activation(qsball[:, j, :], qps[:], AF.Copy)
                else:
                    nc.vector.tensor_tensor(
                        qsball[:, j, :], qps[:], qb_sb[:], ALU.add
                    )
            qg = qsball[:].rearrange("p j (g d) -> p j g d", g=2)
            sq = qnorm.tile([P, QC, 2, D_HEAD], F32, name="qsq", tag="qsq")
            nc.vector.tensor_tensor(sq[:], qg, qg, ALU.mult)
            ss = qnorm.tile([P, QC, 2], F32, name="qss", tag="qss")
            nc.vector.tensor_reduce(
                ss[:], sq[:], axis=mybir.AxisListType.X, op=ALU.add
            )
            y = emit_rsqrt_dve(
                qnorm, ss[:].rearrange("p j g -> p (j g)"), QC * 2, tag="q"
            )
            return (qt, qsball, y)

        def emit_q_side_b(part_a, tpool, act_stage=False):
            """part B: q_hat mult + packed transposes + qhat_all copies."""
            qt, qsball, y = part_a
            qg = qsball[:].rearrange("p j (g d) -> p j g d", g=2)
            yb = y.rearrange("p (j g) -> p j g", g=2)
            q_hat = qnorm.tile([P, QC, 2, D_HEAD], F32R, name="qhat", tag="qhat")
            nc.vector.tensor_tensor(
                q_hat[:],
                qg,
                yb[:, :, :, None].to_broadcast((P, QC, 2, D_HEAD)),
                ALU.mult,
            )
            for j in range(QC):
                ssl = bass.ts(qt * QC + j, P)
                pt = tpool.tile([P, P], F32R, name="qpt", tag=tpool.name_tag)
                nc.tensor.transpose(
                    pt[:], q_hat[:, j].rearrange("p g d -> p (g d)"), ident_r[:]
                )
                if act_stage:
                    nc.scalar.activation(qhat_all[:, ssl], pt[:], AF.Copy)
                else:
                    nc.vector.tensor_copy(qhat_all[:, ssl], pt[:])

        def emit_q_side_group(qt, qpool, tpool, act_stage=False):
            emit_q_side_b(
                emit_q_side_a(qt, qpool, act_stage), tpool, act_stage
            )

        def _unused_q_side(qt, qpool, tpool, act_stage=False):
            """bf16 x @ Wq for one q-tile (4 s-blocks) + RMSNorm(q) + PE
            transposes into qhatT. Stats batched across the 4 s-blocks so the
            DVE Newton-rsqrt chain runs once on [P, 8]. act_stage: route the
            PSUM->SBUF staging copies through ACT (only safe before the exp
            stream starts, when ACT is idle)."""
            xbt = xball[qt]
            qsball = qnorm.tile([P, QC, Q_LOCAL], F32, name="qsb", tag="qsb")
            for j in range(QC):
                qps = qpool.tile([P, Q_LOCAL], F32, name="qps", tag=qpool.name_tag)
                nch = NCH if act_stage else NCH - 1
                for c in range(nch):
                    nc.tensor.matmul(
                        qps[:],
                        lhsT=xbt[:, c, bass.ts(j, P)],
                        rhs=wq_sb[:, c, :],
                        start=(c == 0),
                        stop=(c == nch - 1),
                    )
                if act_stage:
                    nc.scalar.activation(qsball[:, j, :], qps[:], AF.Copy)
                else:
                    nc.vector.tensor_tensor(
                        qsball[:, j, :], qps[:], qb_sb[:], ALU.add
                    )
            qg = qsball[:].rearrange("p j (g d) -> p j g d", g=2)
            sq = qnorm.tile([P, QC, 2, D_HEAD], F32, name="qsq", tag="qsq")
            nc.vector.tensor_tensor(sq[:], qg, qg, ALU.mult)
            ss = qnorm.tile([P, QC, 2], F32, name="qss", tag="qss")
            nc.vector.tensor_reduce(
                ss[:], sq[:], axis=mybir.AxisListType.X, op=ALU.add
            )
            y = emit_rsqrt_dve(
                qnorm, ss[:].rearrange("p j g -> p (j g)"), QC * 2, tag="q"
            )
            yb = y.rearrange("p (j g) -> p j g", g=2)
            q_hat = qnorm.tile([P, QC, 2, D_HEAD], F32R, name="qhat", tag="qhat")
            nc.vector.tensor_tensor(
                q_hat[:],
                qg,
                yb[:, :, :, None].to_broadcast((P, QC, 2, D_HEAD)),
                ALU.mult,
            )
            for j in range(QC):
                ssl = bass.ts(qt * QC + j, P)
                pt = tpool.tile([P, P], F32R, name="qpt", tag=tpool.name_tag)
                nc.tensor.transpose(
                    pt[:], q_hat[:, j].rearrange("p g d -> p (g d)"), ident_r[:]
                )
                if act_stage:
                    nc.scalar.activation(qhat_all[:, ssl], pt[:], AF.Copy)
                else:
                    nc.vector.tensor_copy(qhat_all[:, ssl], pt[:])

        # shared pools alive for the whole kernel
        opool = ctx.enter_context(tc.tile_pool(name="ops", bufs=1, space="PSUM"))
        ppool = ctx.enter_context(tc.tile_pool(name="probs", bufs=4))
        znpool = ctx.enter_context(tc.tile_pool(name="zn", bufs=3))
        rpool = ctx.enter_context(tc.tile_pool(name="rcp", bufs=3))
        osb = ctx.enter_context(tc.tile_pool(name="osb", bufs=4))
        norm = ctx.enter_context(tc.tile_pool(name="norm", bufs=3))

        class _OpsPool:
            name_tag = "ops"

            @staticmethod
            def tile(shape, dt, name=None, tag=None):
                return opool.tile(shape, dt, name=name, tag="ops")

        def emit_score_exp(h, kb, sps, probs, qsl):
            """S matmul (partition-offset by head) + ACT exp -> bf16 probs.
            sps/probs: [P, QW] APs. Returns the exp instruction."""
            hsl = slice(D_HEAD * h, D_HEAD * (h + 1))
            smm = nc.tensor.matmul(
                sps,
                lhsT=khat_all[hsl, bass.ts(kb, P)],
                rhs=qhat_all[hsl, qsl],
                start=True,
                stop=True,
            )
            return smm, nc.scalar.activation(probs, sps, AF.Exp)

        def emit_pv(h, kb, zq, probs, last):
            """4 PV matmuls accumulating [P, QC, 65] into zq (one PSUM
            zero-region per (qt, h))."""
            for qc in range(QC):
                nc.tensor.matmul(
                    zq[:, qc, :],
                    lhsT=probs[:, bass.ts(qc, P)],
                    rhs=vp[:, h, kb, :],
                    start=(kb == 0 and qc == 0),
                    stop=(last and qc == QC - 1),
                    skip_group_check=True,
                )

        def emit_qt_finish(qt, h_zq_pairs, ztp):
            """normalize z (q-major), pack both heads, PE-transpose into
            z_nT[d_local, s]."""
            zn = znpool.tile([P, QC, P], BF16, name="zn", tag="zn")
            for h, zq in h_zq_pairs:
                rcp = rpool.tile([P, QC], F32, name="rcp", tag="rcp")
                nc.vector.reciprocal(rcp[:], zq[:, :, D_HEAD])
                nc.vector.tensor_tensor(
                    zn[:, :, bass.ts(h, D_HEAD)],
                    zq[:, :, 0:D_HEAD],
                    rcp[:, :, None].to_broadcast((P, QC, D_HEAD)),
                    ALU.mult,
                )
            for qc in range(QC):
                nc.tensor.transpose(ztp[:, qc, :], zn[:, qc, :], ident_b[:])
                nc.vector.tensor_copy(
                    z_nT[:, qt * QW + qc * P : qt * QW + (qc + 1) * P],
                    ztp[:, qc, :],
                )

        def emit_oproj(qt, spool=None, final=False):
            # final q-tile: S slots are free, so pipeline the matmuls
            # 2-wide through them and put half the copies on the idle ACT
            for sbl in range(QC):
                sb = qt * QC + sbl
                ot = osb.tile([P, D_MODEL], BF16, name="ot", tag="ot")
                for half in range(2):
                    if final:
                        ops = spool.tile([P, QW], F32, name="ops", tag="sps")
                    else:
                        ops = opool.tile([P, QW], F32, name="ops", tag="ops")
                    nc.tensor.matmul(
                        ops[:],
                        lhsT=z_nT[:, bass.ts(sb, P)],
                        rhs=wot_sb[:, bass.ts(half, QW)],
                        start=True,
                        stop=True,
                    )
                    if final and half == 0:
                        nc.scalar.activation(
                            ot[:, bass.ts(half, QW)], ops[:], AF.Copy
                        )
                    elif final:
                        nc.vector.tensor_copy(ot[:, bass.ts(half, QW)], ops[:])
                    else:
                        nc.vector.tensor_copy(ot[:, bass.ts(half, QW)], ops[:])
                    if final:
                        nc.sync.dma_start(
                            out[bass.ts(sb, P), bass.ts(half, QW)],
                            ot[:, bass.ts(half, QW)],
                        )
                if not final:
                    nc.sync.dma_start(out[bass.ts(sb, P), :], ot[:])

        # ---- streaming prefix: K/V projections with qt0's attention (both
        # heads) interleaved so ACT starts exp within a few us of launch.
        # PSUM banks: kvps 2 + tps 1 + sps1 2 + zqA 1 + zqB 1 + ops 1 = 8
        with ExitStack() as p1:
            qkps = p1.enter_context(tc.tile_pool(name="kvps", bufs=2, space="PSUM"))
            tps = p1.enter_context(tc.tile_pool(name="tps", bufs=1, space="PSUM"))
            sps1 = p1.enter_context(tc.tile_pool(name="sps1", bufs=2, space="PSUM"))
            zqAp = p1.enter_context(tc.tile_pool(name="zqA", bufs=1, space="PSUM"))
            zqBp = p1.enter_context(tc.tile_pool(name="zqB", bufs=1, space="PSUM"))

            class _TpsPool:
                name_tag = "tps"

                @staticmethod
                def tile(shape, dt, name=None, tag=None):
                    return tps.tile(shape, dt, name=name, tag="tps")

            def emit_k_tail(pend, act_stage=False):
                """k_hat mult + packed PE transposes + khat_all copies for a
                finished segment (software-pipelined one segment behind)."""
                g, ksb, rr, jlo, jhi = pend
                n = jhi - jlo
                kg = ksb[:, jlo:jhi, 0 : 2 * D_HEAD].rearrange(
                    "p j (g d) -> p j g d", g=2
                )
                k_hat = norm.tile([P, n, 2, D_HEAD], F32R, name="khat", tag="khat")
                nc.vector.tensor_tensor(
                    k_hat[:],
                    kg,
                    rr.rearrange("p (j g) -> p j g", g=2)[
                        :, :, :, None
                    ].to_broadcast((P, n, 2, D_HEAD)),
                    ALU.mult,
                )
                last_tp = None
                for j in range(n):
                    ssl = bass.ts(g * QC + jlo + j, P)
                    pt = tps.tile([P, P], F32R, name="pt", tag="tps")
                    last_tp = nc.tensor.transpose(
                        pt[:], k_hat[:, j].rearrange("p g d -> p (g d)"), ident_r[:]
                    )
                    if act_stage:
                        nc.scalar.activation(
                            khat_all[:, ssl], pt[:], AF.Copy, scale=wkc_sb[:]
                        )
                    else:
                        nc.vector.tensor_scalar_mul(khat_all[:, ssl], pt[:], wkc_sb[:])
                return last_tp

            zqA = zqAp.tile([P, QC, D_HEAD + 1], F32, name="zqA")
            zqB = zqBp.tile([P, QC, D_HEAD + 1], F32, name="zqB")
            qsl0 = bass.ts(0, QW)

            def emit_prefix_attn(kblo, kbhi):
                """qt0 attention windows for k-blocks [kblo, kbhi), both
                heads, EB=1. Returns the last exp instruction (used as a
                scheduler ordering anchor)."""
                anchor = None
                for h, zq in ((0, zqA), (1, zqB)):
                    for kb in range(kblo, kbhi):
                        sps = sps1.tile([P, QW], F32, name="sps1", tag="sps1")
                        probs = ppool.tile([P, QW], BF16, name="probs1", tag="probs1")
                        anchor, _ = emit_score_exp(h, kb, sps[:], probs[:], qsl0)
                        emit_pv(h, kb, zq, probs[:], last=(kb == KB - 1))
                return anchor

            def emit_kv_gemms(g, ksb, sqg, jlo, jhi, act_stage=False,
                              pe_anchors=()):
                import bass_rust

                xbt = xball[g]
                for j in range(jlo, jhi):
                    sb = g * QC + j
                    kv_ps = qkps.tile([P, KV_LOCAL], F32, name="kv_ps", tag="kvps")
                    nch = NCH if act_stage else NCH - 1
                    for c in range(nch):
                        mm = nc.tensor.matmul(
                            kv_ps[:],
                            lhsT=xbt[:, c, bass.ts(j, P)],
                            rhs=wkv_sb[:, c, :],
                            start=(c == 0),
                            stop=(c == nch - 1),
                        )
                        if pe_anchors:
                            # PE->PE ordering-only deps: keep far-ahead GEMMs
                            # behind older attention windows in the static PE
                            # stream (free at runtime: same-engine order)
                            dset = bass_rust.InstructionNameOrderedSet()
                            for a in pe_anchors:
                                if a is not None:
                                    dset.add(a.ins.name)
                            mm.ins.add_nosync_dependencies_from(dset)
                            pe_anchors = ()
                    # one combined K|V staging op (PSUM->SBUF) that also adds
                    # the qkv bias (saves the 9th GEMM chunk); V' is then
                    # extracted SBUF->SBUF on the otherwise-idle Pool engine.
                    # act_stage (head of pipeline): bias came from the aug-row
                    # chunk instead, so the idle ACT can do a plain copy
                    if act_stage:
                        nc.scalar.activation(ksb[:, j, :], kv_ps[:], AF.Copy)
                    else:
                        nc.vector.tensor_tensor(
                            ksb[:, j, :], kv_ps[:], kvb_sb[:], ALU.add
                        )
                    nc.gpsimd.tensor_copy(
                        vp[:, :, sb, 0:D_HEAD],
                        ksb[:, j, 2 * D_HEAD : 4 * D_HEAD].rearrange(
                            "p (h d) -> p h d", h=2
                        ),
                    )
                    kgj = ksb[:, j, 0 : 2 * D_HEAD].rearrange("p (g d) -> p g d", g=2)
                    nc.vector.tensor_tensor(sqg[:, j], kgj, kgj, ALU.mult)

            def emit_k_stats(g, ksb, sqg, jlo, jhi):
                n = jhi - jlo
                ssg = norm.tile([P, n, 2], F32, name="ssg", tag="ssg")
                nc.vector.tensor_reduce(
                    ssg[:], sqg[:, jlo:jhi], axis=mybir.AxisListType.X, op=ALU.add
                )
                rr = emit_rsqrt_dve(
                    norm, ssg[:].rearrange("p j g -> p (j g)"), n * 2, tag="k"
                )
                return (g, ksb, rr, jlo, jhi)

            # group 0 is split into two pairs so the first attention windows
            # (and with them ACT's exp stream) start as early as possible
            # broadcast the bias rows (chunk 8, partition 0) across all
            # partitions via a K=1 ones-matmul; PE is idle this early
            for bias_sb, w_sb, ncols in (
                (kvb_sb, wkv_sb, KV_LOCAL),
                (qb_sb, wq_sb, Q_LOCAL),
            ):
                bps = opool.tile([P, ncols], F32, name="bps", tag="ops")
                nc.tensor.matmul(
                    bps[:],
                    lhsT=ones1[:],
                    rhs=w_sb[0:1, NCH - 1, :],
                    start=True,
                    stop=True,
                )
                nc.vector.tensor_copy(bias_sb[:], bps[:])

            segs = [(0, 0, 2), (0, 2, 4)] + [(g, 0, QC) for g in range(1, SB // QC)]
            ktiles = {}
            pending = None
            pe_anch = {}
            for si, (g, jlo, jhi) in enumerate(segs):
                if jlo == 0:
                    ktiles[g] = (
                        norm.tile([P, QC, KV_LOCAL], F32, name="ksb", tag="ksb"),
                        norm.tile([P, QC, 2, D_HEAD], F32, name="sqg", tag="sqg"),
                    )
                ksb, sqg = ktiles[g]
                # de-prioritize far-ahead kv GEMMs for the tile scheduler so
                # ready attention windows always win the PE; PE->PE nosync
                # anchors also keep them behind older windows in the static
                # stream (free at runtime: same-engine program order)
                with tc.high_priority(offset=-(si * 800)):
                    emit_kv_gemms(g, ksb, sqg, jlo, jhi, act_stage=(si == 0))
                if si == 0:
                    # q side of qt0: overlaps the first k pair's stats chain
                    emit_q_side_group(0, _OpsPool, _TpsPool, act_stage=True)
                if pending is not None:
                    pg, _, _, pjlo, pjhi = pending
                    tp_a = emit_k_tail(pending)
                    s_a = emit_prefix_attn(pg * QC + pjlo, pg * QC + pjhi)
                    pe_anch[si - 1] = (tp_a, s_a)
                pending = emit_k_stats(g, ksb, sqg, jlo, jhi)
                if si == 0:
                    # no pipelining for the very first pair: its khat (and the
                    # first exp windows) are the critical path
                    tp_a = emit_k_tail(pending, act_stage=True)
                    s_a = emit_prefix_attn(0, 2)
                    pe_anch[0] = (tp_a, s_a)
                    pending = None
                if (g, jlo) == (3, 0):
                    with tc.high_priority(offset=-2400):
                        qs1 = emit_q_side_a(1, _OpsPool)
                if (g, jlo) == (4, 0):
                    with tc.high_priority(offset=-2400):
                        emit_q_side_b(qs1, _TpsPool)
            emit_k_tail(pending)
            emit_prefix_attn(SB - QC, SB)
            ztp0 = zqBp.tile([P, QC, P], BF16, name="ztp0", tag="zqB")
            emit_qt_finish(0, ((0, zqA), (1, zqB)), ztp0)

        # ---- steady state: q-tiles 1..7, ACT-saturated exp pipeline.
        # PSUM banks: 2 score slots x3 banks, z accumulator 1, ops 1 = 8
        with ExitStack() as p2:
            spool = p2.enter_context(tc.tile_pool(name="sps", bufs=2, space="PSUM"))
            zqpool = p2.enter_context(tc.tile_pool(name="zqps", bufs=1, space="PSUM"))

            def emit_sexp(qt, h, kb0, nb):
                """S matmuls + exp for one steady batch; returns the probs."""
                qsl = bass.ts(qt, QW)
                sps = spool.tile([P, EXP_BATCH, QW], F32, name="sps", tag="sps")
                probs = ppool.tile(
                    [P, EXP_BATCH, QW], BF16, name="probs", tag="probs"
                )
                hsl = slice(D_HEAD * h, D_HEAD * (h + 1))
                for j in range(nb):
                    kb = kb0 + j
                    nc.tensor.matmul(
                        sps[:, j, :],
                        lhsT=khat_all[hsl, bass.ts(kb, P)],
                        rhs=qhat_all[hsl, qsl],
                        start=True,
                        stop=True,
                    )
                nc.scalar.activation(probs[:, 0:nb, :], sps[:, 0:nb, :], AF.Exp)
                return probs

            # (qt, h) tile-head stream with the next head's first S+exp batch
            # prefetched before the current head's normalize, so ACT never
            # waits on the zq bank turnaround at head/tile boundaries
            heads = [(qt, h) for qt in range(1, QT) for h in range(HEADS_LOCAL)]
            prefetch = None
            zn = None
            for qt, h in heads:
                if h == 0:
                    zn = znpool.tile([P, QC, P], BF16, name="zn", tag="zn")
                zq = zqpool.tile([P, QC, D_HEAD + 1], F32, name="zq", tag="zq")
                for kb0 in [0] + list(range(2, KB, EXP_BATCH)):
                    nb = 2 if kb0 == 0 else min(EXP_BATCH, KB - kb0)
                    if kb0 == 0 and prefetch is not None:
                        probs = prefetch
                        prefetch = None
                    else:
                        probs = emit_sexp(qt, h, kb0, nb)
                    # all 128 PV matmuls form ONE PSUM accumulation group
                    # (zq spans a single 2KB zero region)
                    for j in range(nb):
                        kb = kb0 + j
                        emit_pv(h, kb, zq, probs[:, j, :], last=(kb == KB - 1))
                    # software-pipelined work emitted under the exp shadow:
                    # h0: O-projection of the previous q-tile
                    # h1: q side (GEMM+norm+transposes) of the next q-tile
                    if kb0 == 2 and h == 0:
                        emit_oproj(qt - 1)
                    if kb0 == 2 and h == 1 and qt < QT - 1:
                        emit_q_side_group(qt + 1, _OpsPool, _OpsPool)
                # prefetch the next tile-head's first batch (its qhat/khat are
                # ready well before this point)
                if (qt, h) != heads[-1] and not DISABLE_PREFETCH:
                    nqt, nh = heads[heads.index((qt, h)) + 1]
                    prefetch = emit_sexp(nqt, nh, 0, 2)
                # normalize in q-major: z = z / rowsum (col 64)
                rcp = rpool.tile([P, QC], F32, name="rcp", tag="rcp")
                nc.vector.reciprocal(rcp[:], zq[:, :, D_HEAD])
                nc.vector.tensor_tensor(
                    zn[:, :, bass.ts(h, D_HEAD)],
                    zq[:, :, 0:D_HEAD],
                    rcp[:, :, None].to_broadcast((P, QC, D_HEAD)),
                    ALU.mult,
                )
                if h == 1:
                    # transpose both heads at once into z_nT[d_local, s]
                    ztp = zqpool.tile([P, QC, P], BF16, name="ztp", tag="zq")
                    for qc in range(QC):
                        nc.tensor.transpose(ztp[:, qc, :], zn[:, qc, :], ident_b[:])
                        nc.vector.tensor_copy(
                            z_nT[:, qt * QW + qc * P : qt * QW + (qc + 1) * P],
                            ztp[:, qc, :],
                        )
            emit_oproj(QT - 1, spool, final=True)

    if split_waits:
        _split_excess_waits(nc)
    return nc


def shard_inputs(x, Wqkv, bqkv, Wo, bo, wq, wk):
    import ml_dtypes

    x2 = np.ascontiguousarray(np.asarray(x, dtype=np.float32).reshape(SEQ, D_MODEL))
    Wqkv = np.asarray(Wqkv, dtype=np.float32)
    bqkv = np.asarray(bqkv, dtype=np.float32)
    Wo = np.asarray(Wo, dtype=np.float32)
    wq = np.asarray(wq, dtype=np.float32)
    wk = np.asarray(wk, dtype=np.float32)

    xta = np.zeros((DM_AUG, SEQ), np.float32)
    xta[:D_MODEL] = x2.T
    xta[D_MODEL] = 1.0
    xtb = np.ascontiguousarray(xta.astype(ml_dtypes.bfloat16))

    # per-partition scale for khat_all's packed [d0|d1] feature rows
    wkc = np.ascontiguousarray(np.tile((wq * wk).reshape(D_HEAD), 2).reshape(P, 1))

    in_maps = []
    for c in range(N_CORES):
        rows, brows = [], []
        for part in range(3):
            for h in (HEADS_LOCAL * c, HEADS_LOCAL * c + 1):
                sl = slice(part * D_MODEL + h * D_HEAD, part * D_MODEL + (h + 1) * D_HEAD)
                rows.append(Wqkv[sl])
                brows.append(bqkv[sl])
        Wl = np.concatenate(rows, 0)          # [384, 1024] rows [q0|q1|k0|k1|v0|v1]
        bl = np.concatenate(brows, 0)         # [384]
        wqkvta = np.zeros((DM_AUG, 384), np.float32)
        wqkvta[:D_MODEL] = Wl.T
        wqkvta[D_MODEL] = bl
        wkvt = np.ascontiguousarray(
            wqkvta[:, Q_LOCAL:].astype(ml_dtypes.bfloat16)
        )                                                              # [1152, 256]
        wqt = np.ascontiguousarray(
            wqkvta[:, :Q_LOCAL].astype(ml_dtypes.bfloat16)
        )                                                              # [1152, 128]
        cols = slice(HEADS_LOCAL * c * D_HEAD, (HEADS_LOCAL * c + HEADS_LOCAL) * D_HEAD)
        wotc = np.ascontiguousarray(Wo[:, cols].T.astype(ml_dtypes.bfloat16))
        in_maps.append(
            {
                "xtb": xtb,
                "wkvt": wkvt,
                "wqt": wqt,
                "wot": wotc,
                "wkc": wkc,
            }
        )
    return in_maps


_NC_CACHE = {}
LAST_RESULT = None


def kernel(x, Wqkv, bqkv, Wo, bo, wq, wk):
    import os
    from concourse.bass_utils import run_bass_kernel_spmd

    global LAST_RESULT
    assert np.asarray(x).shape == (1, SEQ, D_MODEL)
    in_maps = shard_inputs(x, Wqkv, bqkv, Wo, bo, wq, wk)
    if "nc" not in _NC_CACHE:
        _NC_CACHE["nc"] = build_core_kernel()
    nc = _NC_CACHE["nc"]
    trace = bool(int(os.environ.get("BASS_KERNEL_TRACE", "0")))
    res = run_bass_kernel_spmd(nc, in_maps, list(range(N_CORES)), trace=trace)
    LAST_RESULT = res
    acc = np.zeros((SEQ, D_MODEL), np.float64)
    for c in range(N_CORES):
        acc += res.results[c]["out"].astype(np.float64)
    acc += np.asarray(bo, dtype=np.float64)
    return acc.astype(np.float32).reshape(1, SEQ, D_MODEL)

